# revision 2
# baseline (speedup 1.0000x reference)
"""Trainium2 Bass kernel for nn_AttentionSimilarity.

Contract: kernel(**inputs) takes the FULL unsharded inputs (numpy) and
returns the FULL [64, 64] similarity matrix, distributing work across 8
NeuronCores internally.

Structure:
  prog1 (projections, sharded by batch): each core projects its 8
    a-batches and 8 b-batches through the three two-layer MLPs,
    emitting qaT/kaT/vaT/qbT/kbT/vbT chunks in [inner, (batch, n)]
    layout. Host gathers the a-side to full tensors.
  prog2 (attention, sharded by p = b-side batch): each core computes
    both attention paths for its 8 p's against all 64 q's, the cosine
    numerators/denominators via selector matmuls on the PE, and the
    per-(p,q) sums over n. Host assembles the [64, 64] output.

Math notes:
  - softmax feeds only cosine similarity, which is scale-invariant in
    the aligned vector, so the softmax max-shift and denominator cancel:
    softmax reduces to exp(scores/8).
  - the x-side cosine norm is folded on the host (vhat = v / max(|v|, eps)).
  - 1/max(|y|, eps) and the dot with vhat are applied on the host from
    the streamed-out aligned values.

Performance notes (vs the first working version):
  - prog1 W1 layer and both programs' score matmuls run in fp8e4 with
    MatmulPerfMode.DoubleRow (2 contraction rows per PE partition, 0.5
    cycles/output column): weights/features/q/k are DR-packed on the
    host ([K/2, 2, M] with k = (K/2)*s + p; lhsT m-blocks padded to
    MP=112 so the DR pair-stride stays 16-byte aligned).
  - the entire cosine stage (dot, squared-norm, rsqrt, mean over n)
    is computed on the HOST: the aligned values (As, bf16) stream out
    over the otherwise-idle DMA engines, deleting the M/SQ multiplies,
    all selector-reduce matmuls, the P1/P2 PSUM accumulators (freeing
    banks for aligned double-buffering), and the device epilogues.
    The device does projections, scores, softmax-exp and the aligned
    matmuls -- all of the O(B^2 N^2) compute.
  - warmup/tail: weight DMAs are split/consolidated so the first matmul
    starts as early as possible; prog1's W2 PSUM/copy/DMA pipeline is
    chunked per bank so stores drain during compute; path2 score tiles
    are 1536 columns (3 PSUM banks) to amortize the fixed per-
    instruction ACT access latency on the softmax exp, which is the
    saturated engine (~98% busy) in the final balance.
  - measured rel err vs fp32 reference: ~1.7e-3.

Dead end (measured): packing score tiles to 128 partitions by mixing
(q, m) across rows would cut exp columns 100/128, but the follow-up
aligned matmuls need operand slices at arbitrary partition offsets and
the PE requires base partition 0/32/64 (bass matmul assert); since
100 is not a multiple of 32, per-q slices of a packed layout are
unaddressable. The [m<=100, cols] score layout is forced.
"""

import os
import sys

sys.path.insert(0, "/opt/trn_rl_repo")
os.environ.setdefault("NEURON_RT_RESET_CORES", "1")

import numpy as np
import ml_dtypes  # noqa: F401  (bf16 host arrays)

import bass_rust
import concourse.bass as bass
import concourse.mybir as mybir
import concourse.tile as tile
from concourse.bass_utils import run_bass_kernel_spmd

F32 = mybir.dt.float32
F32R = mybir.dt.float32r
BF16 = mybir.dt.bfloat16
F16 = mybir.dt.float16
F8E4 = mybir.dt.float8e4
AF = mybir.ActivationFunctionType
DR = mybir.MatmulPerfMode.DoubleRow

B = 64          # batches per side
C = 512         # channels
N = 100         # H*W tokens per batch
INNER = 64      # projected dim
CORES = 8
PB = B // CORES  # batches per core (8)
BN = PB * N      # 800: (batch, n) columns per core chunk
EPS = 1e-8
KT1 = C // 128   # prog1 contraction tiles (4)
MP = 112         # fp8-DR padded m stride (112 % 16 == 0, >= N)

E1_BUFS = int(os.environ.get("K_E1_BUFS", "5"))
SEL_LAG = int(os.environ.get("K_SEL_LAG", "4"))
POOL_MOD1 = int(os.environ.get("K_POOL_MOD1", os.environ.get("K_POOL_MOD", "3")))
POOL_MOD2 = int(os.environ.get("K_POOL_MOD2", os.environ.get("K_POOL_MOD", "2")))
SEL_LAG2 = int(os.environ.get("K_SEL_LAG2", "4"))
M2_BUFS = int(os.environ.get("K_M2_BUFS", "8"))
MPOOL_MOD = int(os.environ.get("K_MPOOL_MOD", "0"))  # 0=never, k=every kth M on pool
M_BUFS = int(os.environ.get("K_M_BUFS", "8"))
E2_BUFS = int(os.environ.get("K_E2_BUFS", "3"))
S1_BUFS = int(os.environ.get("K_S1_BUFS", "2"))
A1_BUFS = int(os.environ.get("K_A1_BUFS", "1"))

_waitsplit_ctr = [0]


def _split_multi_waits(nc, max_waits=1):
    """This container's walrus build accepts at most ONE sync wait per
    instruction; Tile attaches several. Move extras onto preceding
    same-engine NoOps (engines are in-order, so semantics hold)."""
    n_split = 0
    for f in nc.m.functions:
        for blk in f.blocks:
            insts = list(blk.instructions)
            new_list = []
            changed = False
            for inst in insts:
                si = inst.sync_info
                waits = list(si.on_wait) if (si is not None and si.on_wait) else []
                if len(waits) > max_waits:
                    for w in waits[:-max_waits]:
                        _waitsplit_ctr[0] += 1
                        nop = mybir.InstNoOp(
                            name=f"I-waitsplit-{_waitsplit_ctr[0]}",
                            engine=inst.engine,
                            ins=[],
                            outs=[],
                            sync_info=bass_rust.SyncInfo(on_wait=[w], on_update=[]),
                        )
                        nc.register_instruction(nop, overwrite=True)
                        new_list.append(nop)
                        n_split += 1
                    si.on_wait = waits[-max_waits:]
                    inst.sync_info = si
                    changed = True
                new_list.append(inst)
            if changed:
                blk.instructions = new_list
    return n_split


# ---------------------------------------------------------------- prog1

def build_prog1():
    """Projection program. Per-core inputs:
      fa8dr, fb8dr: [KT*64, 2*BN] f8e4 DoubleRow-packed features
        (row kt*64+p, col (s, (b n)) holds feat[c = 128*kt + 64*s + p])
      wq1dr/...: [KT*64, 2*C] f8e4 DR weights (col (s, c_out))
      wq2/...: [C, INNER] bf16
    Outputs: qaT8/kaT8/vaT8/qbT8/kbT8/vbT8: [INNER, BN]  ([i, (b n)])
    """
    nc = bass.Bass("TRN2", target_bir_lowering=False, debug=False,
                   num_devices=CORES)
    fa8 = nc.dram_tensor("fa8dr", [KT1 * 64, 2 * BN], F8E4,
                         kind="ExternalInput").ap()
    fb8 = nc.dram_tensor("fb8dr", [KT1 * 64, 2 * BN], F8E4,
                         kind="ExternalInput").ap()
    w1 = {p: nc.dram_tensor(f"w{p}1dr", [KT1 * 64, 2 * C], F8E4,
                            kind="ExternalInput").ap()
          for p in "qkv"}
    w2 = {p: nc.dram_tensor(f"w{p}2", [C, INNER], BF16, kind="ExternalInput").ap()
          for p in "qkv"}
    outs = {(s, p): nc.dram_tensor(f"{p}{s}T8", [INNER, BN], F16,
                                   kind="ExternalOutput").ap()
            for s in "ab" for p in "qkv"}

    KT = KT1  # 4 contraction tiles of 128 (64 partitions x 2 DR)
    CT = C // 128  # 4 c_out tiles
    CH = [(0, 512), (512, BN)]  # psum-bank-aligned column chunks of BN

    with tile.TileContext(nc) as tc:
        with (
            tc.tile_pool(name="wpool", bufs=1) as wpool,
            tc.tile_pool(name="fpool", bufs=int(os.environ.get("K_F_BUFS", "3"))) as fpool,
            tc.tile_pool(name="hpool", bufs=int(os.environ.get("K_H_BUFS", "5"))) as hpool,
            tc.tile_pool(name="opool", bufs=int(os.environ.get("K_O_BUFS", "4"))) as opool,
            tc.tile_pool(name="psH", bufs=int(os.environ.get("K_PSH_BUFS", "3")), space="PSUM") as psHp,
            tc.tile_pool(name="psO", bufs=int(os.environ.get("K_PSO_BUFS", "1")), space="PSUM") as psOp,
        ):
            w1sb, w2sb = {}, {}

            def load_w1(p):
                wt = wpool.tile([64, KT * 2 * C], F8E4, tag=f"w1{p}",
                                name=f"w1{p}sb")
                wv = wt[:].rearrange("p (kt x) -> p kt x", kt=KT)
                dv = w1[p].rearrange("(kt p) x -> p kt x", p=64)
                nc.sync.dma_start(wv[:, 0:2], dv[:, 0:2])
                nc.sync.dma_start(wv[:, 2:KT], dv[:, 2:KT])
                w1sb[p] = wt

            def load_w(p):
                load_w1(p)
                w2sb[p] = wpool.tile([128, KT * INNER], BF16, tag=f"w2{p}",
                                     name=f"w2{p}sb")
                nc.sync.dma_start(
                    w2sb[p][:].rearrange("p (kt i) -> p kt i", kt=KT),
                    w2[p].rearrange("(kt p) i -> p kt i", p=128))

            for s, feat in (("a", fa8), ("b", fb8)):
                fts = []
                for kt in range(KT):
                    if s == "a" and kt == 0:
                        load_w1("q")
                    ft = fpool.tile([64, 2 * BN], F8E4, tag=f"f{kt}")
                    nc.sync.dma_start(ft[:], feat[64 * kt:64 * (kt + 1), :])
                    fts.append(ft)
                if s == "a":
                    w2sb["q"] = wpool.tile([128, KT * INNER], BF16, tag="w2q",
                                           name="w2qsb")
                    nc.sync.dma_start(
                        w2sb["q"][:].rearrange("p (kt i) -> p kt i", kt=KT),
                        w2["q"].rearrange("(kt p) i -> p kt i", p=128))
                    load_w("k")
                    load_w("v")
                for p in "qkv":
                    hts = []
                    for t in range(CT):
                        psH = psHp.tile([128, 1024], F32, tag="psH")
                        for lo, hi in CH:
                            for kt in range(KT):
                                nc.tensor.matmul(
                                    psH[:, lo:hi],
                                    w1sb[p][:].rearrange(
                                        "p (kt two c) -> p kt two c",
                                        kt=KT, two=2)[
                                        :, kt, :, 128 * t:128 * t + 128],
                                    fts[kt][:].rearrange(
                                        "p (two n) -> p two n", two=2)[
                                        :, :, lo:hi],
                                    start=(kt == 0), stop=(kt == KT - 1),
                                    perf_mode=mybir.MatmulPerfMode.DoubleRow)
                        ht = hpool.tile([128, BN], BF16, tag=f"h{t}")
                        if t % 2 == 0:
                            nc.scalar.activation(ht[:], psH[:, 0:BN], AF.Relu)
                        else:
                            nc.vector.tensor_scalar_max(ht[:], psH[:, 0:BN],
                                                        0.0)
                        hts.append(ht)
                    psOs = [psOp.tile([INNER, 512], F32, tag="psOa",
                                      name="psOa"),
                            psOp.tile([INNER, 512], F32, tag="psOb",
                                      name="psOb")]
                    ot = opool.tile([INNER, BN], F16, tag="out")
                    for ci, (lo, hi) in enumerate(CH):
                        for kt in range(KT):
                            nc.tensor.matmul(
                                psOs[ci][:, 0:hi - lo],
                                w2sb[p][:, INNER * kt:INNER * (kt + 1)],
                                hts[kt][:, lo:hi],
                                start=(kt == 0), stop=(kt == KT - 1))
                        nc.scalar.copy(ot[:, lo:hi], psOs[ci][:, 0:hi - lo])
                        nc.sync.dma_start(outs[(s, p)][:, lo:hi],
                                          ot[:, lo:hi])

    _split_multi_waits(nc)
    return nc


# ---------------------------------------------------------------- prog2

def build_prog2():
    """Attention program, sharded over p (this core's 8 b-batches).

    Unified 64-stage software pipeline; every stage produces 1600 score
    columns in a [128, 2048] PSUM tile (4 banks, double-buffered = all 8
    banks), does ONE 1600-wide exp on ACT (the bottleneck engine), then
    reuses the exp-consumed banks of the same tile as the aligned-matmul
    accumulator (carve-after-read; subtile deps order the WAR hazard).
    Stage t+1's score matmuls are emitted before stage t's aligned
    matmuls so PE always has score work ready when ACT finishes an exp.

      path1 stage j (32): scores for q-pair (2j, 2j+1) over this core's
        800 (p, n) columns; q0 at S cols 0:800, q1 at 1024:1824; exp via
        a strided [100, 2, 800] AP; aligned A at cols 0:800.
      path2 stage (p, k) (32): scores for 1600 (q n) columns
        [1600k, 1600k+1600) against kb[p]; aligned A groups at cols
        0:400 and 512:912; strided copy out.

    Outputs (identical layout to the previous version; host unchanged):
      as1o [128, 32*800] bf16, as2o [128, 32*800] bf16
    """
    nc = bass.Bass("TRN2", target_bir_lowering=False, debug=False,
                   num_devices=CORES)
    din = {}
    for name, shape, dt in [
        ("kaTdr", [32, 2 * B * MP], F8E4), ("qaTdr", [32, 2 * B * N], F8E4),
        ("qbTdr", [32, 2 * BN], F8E4), ("kbTdr", [32, 2 * PB * MP], F8E4),
        ("vaL", [N, (B // 2) * 128], F16), ("vaR", [N, (B // 2) * 128], F16),
        ("vbL", [N, PB * 128], F16), ("vbR", [N, PB * 128], F16),
    ]:
        din[name] = nc.dram_tensor(name, shape, dt, kind="ExternalInput").ap()
    as1o = nc.dram_tensor("as1o", [128, 32 * BN], BF16,
                          kind="ExternalOutput").ap()
    as2o = nc.dram_tensor("as2o", [128, 32 * 800], BF16,
                          kind="ExternalOutput").ap()

    with tile.TileContext(nc) as tc:
        from contextlib import ExitStack
        with ExitStack() as ctx:
            inp = ctx.enter_context(tc.tile_pool(name="inp", bufs=1))
            sb = {}

            def load(name):
                ap = din[name]
                t = inp.tile(list(ap.shape), ap.dtype, tag=name,
                             name=f"sb_{name}")
                nc.sync.dma_start(t[:], ap[:])
                sb[name] = t

            # Input DMAs, hot-first. All on the SP (sync) queue, issued
            # before any output DMA so no wait ever blocks the SP SEQ.
            ka_t = inp.tile([32, 2 * B * MP], F8E4, tag="kaTdr",
                            name="sb_kaTdr")
            sb["kaTdr"] = ka_t
            ka3d = din["kaTdr"].rearrange("p (two q m) -> p two q m",
                                          two=2, q=B)
            ka3s = ka_t[:].rearrange("p (two q m) -> p two q m", two=2, q=B)
            load("qbTdr")
            nc.sync.dma_start(ka3s[:, :, 0:8, :], ka3d[:, :, 0:8, :])
            va_t = {}
            for nm in ("vaL", "vaR"):
                va_t[nm] = inp.tile([N, (B // 2) * 128], F16, tag=nm,
                                    name=f"sb_{nm}")
                nc.sync.dma_start(va_t[nm][:, 0:512], din[nm][:, 0:512])
            for nm in ("vaL", "vaR"):
                nc.sync.dma_start(va_t[nm][:, 512:2048], din[nm][:, 512:2048])
            nc.sync.dma_start(ka3s[:, :, 8:32, :], ka3d[:, :, 8:32, :])
            for nm in ("vaL", "vaR"):
                nc.sync.dma_start(va_t[nm][:, 2048:4096],
                                  din[nm][:, 2048:4096])
            nc.sync.dma_start(ka3s[:, :, 32:64, :], ka3d[:, :, 32:64, :])
            qa_t = inp.tile([32, 2 * B * N], F8E4, tag="qaTdr",
                            name="sb_qaTdr")
            sb["qaTdr"] = qa_t
            qa3d = din["qaTdr"].rearrange("p (two n) -> p two n", two=2)
            qa3s = qa_t[:].rearrange("p (two n) -> p two n", two=2)
            nc.sync.dma_start(qa3s[:, :, 0:3200], qa3d[:, :, 0:3200])
            nc.sync.dma_start(qa3s[:, :, 3200:6400], qa3d[:, :, 3200:6400])
            for name in ("kbTdr", "vbL", "vbR"):
                load(name)

            epool = ctx.enter_context(tc.tile_pool(name="epool", bufs=3))
            mpool = ctx.enter_context(tc.tile_pool(name="mpool", bufs=8))
            psum = ctx.enter_context(
                tc.tile_pool(name="psum", bufs=2, space="PSUM"))

            ka3 = sb["kaTdr"][:].rearrange("p (two q m) -> p two q m",
                                           two=2, q=B)
            qb3 = sb["qbTdr"][:].rearrange("p (two n) -> p two n", two=2)
            kb3 = sb["kbTdr"][:].rearrange("p (two b m) -> p two b m",
                                           two=2, b=PB)
            qa3 = sb["qaTdr"][:].rearrange("p (two n) -> p two n", two=2)

            NS = 64  # 32 path1 pair-stages + 32 path2 (p, k) chunk-stages
            live = {}  # stage -> (sa_tile, E_tile)

            def emit_front(t):
                """Score matmuls + exp for stage t (into a fresh SA tile)."""
                sa = psum.tile([128, 2048], F32, tag="SA", name=f"SA{t % 2}")
                E = epool.tile([100, 1600], F16, tag="E")
                sa_ap = sa[:]
                if t < 32:
                    j = t
                    for qi, q in enumerate((2 * j, 2 * j + 1)):
                        base = 1024 * qi
                        for lo, hi in ((0, 512), (512, 800)):
                            nc.tensor.matmul(
                                sa_ap[0:100, base + lo:base + hi],
                                ka3[:, :, q, 0:N], qb3[:, :, lo:hi],
                                start=True, stop=True, perf_mode=DR)
                    sview = sa_ap[0:100].rearrange("p (two c) -> p two c",
                                                   two=2)[:, :, 0:800]
                    nc.scalar.activation(
                        E[:].rearrange("p (two c) -> p two c", two=2),
                        sview, AF.Exp, scale=0.125)
                else:
                    p, k = (t - 32) // 4, (t - 32) % 4
                    c0 = 1600 * k
                    for lo, hi in ((0, 512), (512, 1024), (1024, 1536),
                                   (1536, 1600)):
                        nc.tensor.matmul(
                            sa_ap[0:100, lo:hi], kb3[:, :, p, 0:N],
                            qa3[:, :, c0 + lo:c0 + hi],
                            start=True, stop=True, perf_mode=DR)
                    nc.scalar.activation(E[:], sa_ap[0:100, 0:1600],
                                         AF.Exp, scale=0.125)
                live[t] = (sa, E)

            def emit_back(t):
                """Aligned matmuls into stage t's exp-consumed banks, copy
                out, DMA."""
                sa, E = live.pop(t)
                sa_ap = sa[:]
                As = mpool.tile([128, 800], BF16, tag="As")
                if t < 32:
                    j = t
                    E3 = E[:].rearrange("p (two c) -> p two c", two=2)
                    vaLs = va_t["vaL"][:, 128 * j:128 * (j + 1)]
                    vaRs = va_t["vaR"][:, 128 * j:128 * (j + 1)]
                    for lo, hi in ((0, 512), (512, 800)):
                        nc.tensor.matmul(sa_ap[0:128, lo:hi], vaLs,
                                         E3[:, 0, lo:hi],
                                         start=True, stop=False)
                        nc.tensor.matmul(sa_ap[0:128, lo:hi], vaRs,
                                         E3[:, 1, lo:hi],
                                         start=False, stop=True)
                        nc.vector.tensor_copy(As[:, lo:hi],
                                              sa_ap[0:128, lo:hi])
                    nc.sync.dma_start(as1o[:, BN * j:BN * (j + 1)], As[:])
                else:
                    p, k = (t - 32) // 4, (t - 32) % 4
                    vbLs = sb["vbL"][:, 128 * p:128 * (p + 1)]
                    vbRs = sb["vbR"][:, 128 * p:128 * (p + 1)]
                    for g in range(2):
                        off = 512 * g
                        nc.tensor.matmul(sa_ap[0:128, off:off + 400], vbLs,
                                         E[:, 800 * g:800 * g + 400],
                                         start=True, stop=False)
                        nc.tensor.matmul(sa_ap[0:128, off:off + 400], vbRs,
                                         E[:, 800 * g + 400:800 * (g + 1)],
                                         start=False, stop=True)
                        nc.vector.tensor_copy(
                            As[:, 400 * g:400 * (g + 1)],
                            sa_ap[0:128, off:off + 400])
                    nc.sync.dma_start(
                        as2o[:, 3200 * p + 800 * k:3200 * p + 800 * (k + 1)],
                        As[:])

            for t in range(NS + 1):
                if t < NS:
                    emit_front(t)
                if t >= 1:
                    emit_back(t - 1)

    _split_multi_waits(nc)
    return nc


# ---------------------------------------------------------------- host

_progs = {}


def _install_compile_cache():
    """Persist compiled NEFF-wrapped custom calls across processes: walrus
    compilation takes tens of seconds per program and bass2jax recompiles
    in every fresh process otherwise."""
    import hashlib
    import pathlib
    from concourse import bass2jax
    if getattr(bass2jax, "_ant_disk_cache", False):
        return
    bass2jax._ant_disk_cache = True
    orig = bass2jax.neuronx_cc_hook
    cdir = pathlib.Path(os.environ.get("BASS_NEFF_CACHE",
                                       "/tmp/bass_neff_cache"))
    try:
        cdir.mkdir(parents=True, exist_ok=True)
    except OSError:
        return

    def cached_hook(code, code_format, platform_version, file_prefix):
        try:
            key = hashlib.sha256(
                bytes(code) + b"|" + bytes(code_format)).hexdigest()
            path = cdir / f"{key}.neffcall"
            if path.exists():
                return 0, path.read_bytes()
        except Exception:
            return orig(code, code_format, platform_version, file_prefix)
        rc, blob = orig(code, code_format, platform_version, file_prefix)
        if rc == 0:
            try:
                tmp = path.with_suffix(f".tmp{os.getpid()}")
                tmp.write_bytes(blob)
                tmp.rename(path)
            except OSError:
                pass
        return rc, blob

    bass2jax.neuronx_cc_hook = cached_hook
    try:
        import libneuronxla
        if libneuronxla.neuronx_cc is orig:
            libneuronxla.neuronx_cc = cached_hook
    except ImportError:
        pass


def _get_progs():
    if "p1" not in _progs:
        _install_compile_cache()
        _progs["p1"] = build_prog1()
        _progs["p2"] = build_prog2()
    return _progs["p1"], _progs["p2"]


def _masters():
    import ml_dtypes
    m1 = np.zeros((128, 320), ml_dtypes.bfloat16)
    m1[0:64, 128] = 1.0   # up-plane (rows 0:64 of rhs) -> out row q
    m1[64:128, 129] = 1.0  # down-plane -> out row q+1
    m8 = np.zeros((128, 320), ml_dtypes.bfloat16)
    m8[0:64, 128] = 1.0
    m8[64:128, 136] = 1.0  # down-plane -> out row r0+8
    return m1, m8


def _dr_pack_k(x, pad_to=None):
    """Pack [K, M] (K contraction, even) into DoubleRow layout
    [K//2, 2*M] fp8e4 with k = (K//2)*s + p."""
    import ml_dtypes
    K = x.shape[0]
    h = K // 2
    arr = x.reshape(2, h, *x.shape[1:]).transpose(1, 0, *range(2, x.ndim + 1))
    return np.ascontiguousarray(arr.reshape(h, -1).astype(
        ml_dtypes.float8_e4m3fn))


def _dr_pack_k_padded(x, nblk, blk, pad):
    """[K, nblk*blk] -> DR fp8 [K//2, 2*nblk*pad] with each blk padded."""
    import ml_dtypes
    K = x.shape[0]
    h = K // 2
    a = x.reshape(2, h, nblk, blk).transpose(1, 0, 2, 3)
    z = np.zeros((h, 2, nblk, pad), np.float32)
    z[:, :, :, 0:blk] = a
    return np.ascontiguousarray(z.reshape(h, -1).astype(
        ml_dtypes.float8_e4m3fn))


def kernel(features_a, features_b, Wq1, Wq2, Wk1, Wk2, Wv1, Wv2):
    import ml_dtypes
    nc1, nc2 = _get_progs()
    cc = np.ascontiguousarray

    fa = np.asarray(features_a, np.float32).reshape(B, C, N)
    fb = np.asarray(features_b, np.float32).reshape(B, C, N)

    def feat_dr(f_core):  # [PB, C, N] -> [KT1*64, 2*BN] fp8 DR
        fT = f_core.transpose(1, 0, 2).reshape(C, BN)
        a = fT.reshape(KT1, 2, 64, BN).transpose(0, 2, 1, 3)
        return cc(a.reshape(KT1 * 64, 2 * BN).astype(ml_dtypes.float8_e4m3fn))

    def w1_dr(W):  # [C, C] -> [KT1*64, 2*C] fp8 DR
        a = np.asarray(W, np.float32).reshape(KT1, 2, 64, C).transpose(
            0, 2, 1, 3)
        return cc(a.reshape(KT1 * 64, 2 * C).astype(ml_dtypes.float8_e4m3fn))

    ws = {"wq1dr": w1_dr(Wq1), "wk1dr": w1_dr(Wk1), "wv1dr": w1_dr(Wv1)}
    ws.update({k: cc(np.asarray(v, np.float32).astype(ml_dtypes.bfloat16))
               for k, v in (("wq2", Wq2), ("wk2", Wk2), ("wv2", Wv2))})

    in1 = [dict(fa8dr=feat_dr(fa[PB * i:PB * (i + 1)]),
                fb8dr=feat_dr(fb[PB * i:PB * (i + 1)]), **ws)
           for i in range(CORES)]
    res1 = run_bass_kernel_spmd(nc1, in1, core_ids=list(range(CORES)))

    def gather(name):
        return np.concatenate([res1.results[i][name] for i in range(CORES)],
                              axis=1)

    qaT, kaT, vaT = gather("qaT8"), gather("kaT8"), gather("vaT8")
    qbT = [res1.results[i]["qbT8"] for i in range(CORES)]
    kbT = [res1.results[i]["kbT8"] for i in range(CORES)]
    vbT = [res1.results[i]["vbT8"] for i in range(CORES)]

    # a-side derived tensors (shared by all cores)
    vaT32 = vaT.astype(np.float32)
    va_nm = cc(vaT.T)                       # [B*N, INNER] fp16
    na = np.maximum(np.sqrt((vaT32 * vaT32).sum(0)), EPS)
    vhat_aT = vaT32 / na[None, :]
    vaL = np.zeros((N, (B // 2) * 128), np.float16)
    vaR = np.zeros((N, (B // 2) * 128), np.float16)
    for j in range(B // 2):
        vaL[:, 128 * j:128 * j + 64] = va_nm[N * 2 * j:N * (2 * j + 1)]
        vaR[:, 128 * j + 64:128 * (j + 1)] = va_nm[N * (2 * j + 1):
                                                   N * (2 * j + 2)]
    vhat_aT2 = np.zeros((128, B * N // 2), np.float32)
    for j2 in range(8):
        vhat_aT2[0:64, 400 * j2:400 * (j2 + 1)] = \
            vhat_aT[:, 800 * j2:800 * j2 + 400]
        vhat_aT2[64:128, 400 * j2:400 * (j2 + 1)] = \
            vhat_aT[:, 800 * j2 + 400:800 * (j2 + 1)]
    m1, m8 = _masters()

    kaTdr = _dr_pack_k_padded(kaT.astype(np.float32), B, N, MP)
    qaTdr = _dr_pack_k(qaT.astype(np.float32))
    in2 = []
    vhat_bTs = []
    for i in range(CORES):
        vbT32 = vbT[i].astype(np.float32)
        vb_nm = cc(vbT[i].T)                # [BN, INNER] fp16
        nb = np.maximum(np.sqrt((vbT32 * vbT32).sum(0)), EPS)
        vhat_bT = vbT32 / nb[None, :]
        vbL = np.zeros((N, PB * 128), np.float16)
        vbR = np.zeros((N, PB * 128), np.float16)
        for p in range(PB):
            vbL[:, 128 * p:128 * p + 64] = vb_nm[N * p:N * (p + 1)]
            vbR[:, 128 * p + 64:128 * (p + 1)] = vb_nm[N * p:N * (p + 1)]
        vhat_bTs.append(vhat_bT)
        in2.append(dict(
            kaTdr=kaTdr, qaTdr=qaTdr,
            qbTdr=_dr_pack_k(qbT[i].astype(np.float32)),
            kbTdr=_dr_pack_k_padded(kbT[i].astype(np.float32), PB, N, MP),
            vaL=vaL, vaR=vaR, vbL=vbL, vbR=vbR))
    res2 = run_bass_kernel_spmd(nc2, in2, core_ids=list(range(CORES)))

    sim = np.zeros((B, B), np.float32)
    for i in range(CORES):
        r = res2.results[i]
        # path1: As1 col-block 800j = pair j (rows 0:64 -> q=2j,
        # rows 64:128 -> q=2j+1, cols (p, n)); dot/ny2 on host
        as1 = np.asarray(r["as1o"], np.float32).reshape(128, 32, 800)
        vb_h = vhat_bTs[i]                              # [64 i, 800 (p n)]
        ny2_1 = np.empty((64, 800), np.float32)
        dot1 = np.empty((64, 800), np.float32)
        ny2_1[0::2] = (as1[0:64] ** 2).sum(0)
        ny2_1[1::2] = (as1[64:128] ** 2).sum(0)
        dot1[0::2] = np.einsum('ijc,ic->jc', as1[0:64], vb_h)
        dot1[1::2] = np.einsum('ijc,ic->jc', as1[64:128], vb_h)
        cos1 = dot1 / np.maximum(np.sqrt(ny2_1), EPS)
        sim1 = cos1.reshape(64, PB, N).sum(-1)          # [q, p]

        # path2: As2 cols 3200p + 800g + 400h + c; rows 0:64 ->
        # qn = 800*(2g+h)+c, rows 64:128 -> +400; vhat_a [64, (g,h,half,c)]
        as2 = np.asarray(r["as2o"], np.float32).reshape(128, PB, 4, 2, 400)
        va4 = vhat_aT.reshape(64, 4, 2, 2, 400)         # [i, g, h, half, c]
        ny_lo = (as2[0:64] ** 2).sum(0).reshape(PB, 8, 400)
        ny_hi = (as2[64:128] ** 2).sum(0).reshape(PB, 8, 400)
        ny2_2 = np.concatenate([ny_lo, ny_hi], axis=2).reshape(PB, B * N)
        d_lo = np.einsum('ipghc,ighc->pghc', as2[0:64], va4[:, :, :, 0])
        d_hi = np.einsum('ipghc,ighc->pghc', as2[64:128], va4[:, :, :, 1])
        dot2 = np.concatenate([d_lo.reshape(PB, 8, 400),
                               d_hi.reshape(PB, 8, 400)],
                              axis=2).reshape(PB, B * N)
        cos2 = dot2 / np.maximum(np.sqrt(ny2_2), EPS)
        sim2 = cos2.reshape(PB, B, N).sum(-1)           # [p, q]

        sim[PB * i:PB * (i + 1)] = (sim1.T + sim2) / N
    return sim



# revision 3
# speedup vs baseline: 1.0120x; 1.0120x over previous
"""Trainium2 Bass kernel for nn_AttentionSimilarity.

Contract: kernel(**inputs) takes the FULL unsharded inputs (numpy) and
returns the FULL [64, 64] similarity matrix, distributing work across 8
NeuronCores internally.

Structure:
  prog1 (projections, sharded by batch): each core projects its 8
    a-batches and 8 b-batches through the three two-layer MLPs,
    emitting qaT/kaT/vaT/qbT/kbT/vbT chunks in [inner, (batch, n)]
    layout. Host gathers the a-side to full tensors.
  prog2 (attention, sharded by p = b-side batch): each core computes
    both attention paths for its 8 p's against all 64 q's, the cosine
    numerators/denominators via selector matmuls on the PE, and the
    per-(p,q) sums over n. Host assembles the [64, 64] output.

Math notes:
  - softmax feeds only cosine similarity, which is scale-invariant in
    the aligned vector, so the softmax max-shift and denominator cancel:
    softmax reduces to exp(scores/8).
  - the x-side cosine norm is folded on the host (vhat = v / max(|v|, eps)).
  - 1/max(|y|, eps) and the dot with vhat are applied on the host from
    the streamed-out aligned values.

Performance notes (vs the first working version):
  - prog1 W1 layer and both programs' score matmuls run in fp8e4 with
    MatmulPerfMode.DoubleRow (2 contraction rows per PE partition, 0.5
    cycles/output column): weights/features/q/k are DR-packed on the
    host ([K/2, 2, M] with k = (K/2)*s + p; lhsT m-blocks padded to
    MP=112 so the DR pair-stride stays 16-byte aligned).
  - the entire cosine stage (dot, squared-norm, rsqrt, mean over n)
    is computed on the HOST: the aligned values (As, bf16) stream out
    over the otherwise-idle DMA engines, deleting the M/SQ multiplies,
    all selector-reduce matmuls, the P1/P2 PSUM accumulators (freeing
    banks for aligned double-buffering), and the device epilogues.
    The device does projections, scores, softmax-exp and the aligned
    matmuls -- all of the O(B^2 N^2) compute.
  - warmup/tail: weight DMAs are split/consolidated so the first matmul
    starts as early as possible; prog1's W2 PSUM/copy/DMA pipeline is
    chunked per bank so stores drain during compute; path2 score tiles
    are 1536 columns (3 PSUM banks) to amortize the fixed per-
    instruction ACT access latency on the softmax exp, which is the
    saturated engine (~98% busy) in the final balance.
  - measured rel err vs fp32 reference: ~1.7e-3.

Dead end (measured): packing score tiles to 128 partitions by mixing
(q, m) across rows would cut exp columns 100/128, but the follow-up
aligned matmuls need operand slices at arbitrary partition offsets and
the PE requires base partition 0/32/64 (bass matmul assert); since
100 is not a multiple of 32, per-q slices of a packed layout are
unaddressable. The [m<=100, cols] score layout is forced.
"""

import os
import sys

sys.path.insert(0, "/opt/trn_rl_repo")
os.environ.setdefault("NEURON_RT_RESET_CORES", "1")

import numpy as np
import ml_dtypes  # noqa: F401  (bf16 host arrays)

import bass_rust
import concourse.bass as bass
import concourse.mybir as mybir
import concourse.tile as tile
from concourse.bass_utils import run_bass_kernel_spmd

F32 = mybir.dt.float32
F32R = mybir.dt.float32r
BF16 = mybir.dt.bfloat16
F16 = mybir.dt.float16
F8E4 = mybir.dt.float8e4
AF = mybir.ActivationFunctionType
DR = mybir.MatmulPerfMode.DoubleRow

B = 64          # batches per side
C = 512         # channels
N = 100         # H*W tokens per batch
INNER = 64      # projected dim
CORES = 8
PB = B // CORES  # batches per core (8)
BN = PB * N      # 800: (batch, n) columns per core chunk
EPS = 1e-8
KT1 = C // 128   # prog1 contraction tiles (4)
MP = 112         # fp8-DR padded m stride (112 % 16 == 0, >= N)

E1_BUFS = int(os.environ.get("K_E1_BUFS", "5"))
SEL_LAG = int(os.environ.get("K_SEL_LAG", "4"))
POOL_MOD1 = int(os.environ.get("K_POOL_MOD1", os.environ.get("K_POOL_MOD", "3")))
POOL_MOD2 = int(os.environ.get("K_POOL_MOD2", os.environ.get("K_POOL_MOD", "2")))
SEL_LAG2 = int(os.environ.get("K_SEL_LAG2", "4"))
M2_BUFS = int(os.environ.get("K_M2_BUFS", "8"))
MPOOL_MOD = int(os.environ.get("K_MPOOL_MOD", "0"))  # 0=never, k=every kth M on pool
M_BUFS = int(os.environ.get("K_M_BUFS", "8"))
E2_BUFS = int(os.environ.get("K_E2_BUFS", "3"))
S1_BUFS = int(os.environ.get("K_S1_BUFS", "2"))
A1_BUFS = int(os.environ.get("K_A1_BUFS", "1"))

_waitsplit_ctr = [0]


def _split_multi_waits(nc, max_waits=1):
    """This container's walrus build accepts at most ONE sync wait per
    instruction; Tile attaches several. Move extras onto preceding
    same-engine NoOps (engines are in-order, so semantics hold)."""
    n_split = 0
    for f in nc.m.functions:
        for blk in f.blocks:
            insts = list(blk.instructions)
            new_list = []
            changed = False
            for inst in insts:
                si = inst.sync_info
                waits = list(si.on_wait) if (si is not None and si.on_wait) else []
                if len(waits) > max_waits:
                    for w in waits[:-max_waits]:
                        _waitsplit_ctr[0] += 1
                        nop = mybir.InstNoOp(
                            name=f"I-waitsplit-{_waitsplit_ctr[0]}",
                            engine=inst.engine,
                            ins=[],
                            outs=[],
                            sync_info=bass_rust.SyncInfo(on_wait=[w], on_update=[]),
                        )
                        nc.register_instruction(nop, overwrite=True)
                        new_list.append(nop)
                        n_split += 1
                    si.on_wait = waits[-max_waits:]
                    inst.sync_info = si
                    changed = True
                new_list.append(inst)
            if changed:
                blk.instructions = new_list
    return n_split


# ---------------------------------------------------------------- prog1

def build_prog1():
    """Projection program. Per-core inputs:
      fa8dr, fb8dr: [KT*64, 2*BN] f8e4 DoubleRow-packed features
        (row kt*64+p, col (s, (b n)) holds feat[c = 128*kt + 64*s + p])
      wq1dr/...: [KT*64, 2*C] f8e4 DR weights (col (s, c_out))
      wq2/...: [C, INNER] bf16
    Outputs: qaT8/kaT8/vaT8/qbT8/kbT8/vbT8: [INNER, BN]  ([i, (b n)])
    """
    nc = bass.Bass("TRN2", target_bir_lowering=False, debug=False,
                   num_devices=CORES)
    fa8 = nc.dram_tensor("fa8dr", [KT1 * 64, 2 * BN], F8E4,
                         kind="ExternalInput").ap()
    fb8 = nc.dram_tensor("fb8dr", [KT1 * 64, 2 * BN], F8E4,
                         kind="ExternalInput").ap()
    w1 = {p: nc.dram_tensor(f"w{p}1dr", [KT1 * 64, 2 * C], F8E4,
                            kind="ExternalInput").ap()
          for p in "qkv"}
    w2 = {p: nc.dram_tensor(f"w{p}2", [C, INNER], BF16, kind="ExternalInput").ap()
          for p in "qkv"}
    outs = {(s, p): nc.dram_tensor(f"{p}{s}T8", [INNER, BN], F16,
                                   kind="ExternalOutput").ap()
            for s in "ab" for p in "qkv"}

    KT = KT1  # 4 contraction tiles of 128 (64 partitions x 2 DR)
    CT = C // 128  # 4 c_out tiles
    CH = [(0, 512), (512, BN)]  # psum-bank-aligned column chunks of BN

    with tile.TileContext(nc) as tc:
        with (
            tc.tile_pool(name="wpool", bufs=1) as wpool,
            tc.tile_pool(name="fpool", bufs=int(os.environ.get("K_F_BUFS", "3"))) as fpool,
            tc.tile_pool(name="hpool", bufs=int(os.environ.get("K_H_BUFS", "5"))) as hpool,
            tc.tile_pool(name="opool", bufs=int(os.environ.get("K_O_BUFS", "4"))) as opool,
            tc.tile_pool(name="psH", bufs=int(os.environ.get("K_PSH_BUFS", "3")), space="PSUM") as psHp,
            tc.tile_pool(name="psO", bufs=int(os.environ.get("K_PSO_BUFS", "1")), space="PSUM") as psOp,
        ):
            w1sb, w2sb = {}, {}

            def load_w1(p):
                wt = wpool.tile([64, KT * 2 * C], F8E4, tag=f"w1{p}",
                                name=f"w1{p}sb")
                wv = wt[:].rearrange("p (kt x) -> p kt x", kt=KT)
                dv = w1[p].rearrange("(kt p) x -> p kt x", p=64)
                nc.sync.dma_start(wv[:, 0:2], dv[:, 0:2])
                nc.sync.dma_start(wv[:, 2:KT], dv[:, 2:KT])
                w1sb[p] = wt

            def load_w(p):
                load_w1(p)
                w2sb[p] = wpool.tile([128, KT * INNER], BF16, tag=f"w2{p}",
                                     name=f"w2{p}sb")
                nc.sync.dma_start(
                    w2sb[p][:].rearrange("p (kt i) -> p kt i", kt=KT),
                    w2[p].rearrange("(kt p) i -> p kt i", p=128))

            for s, feat in (("a", fa8), ("b", fb8)):
                fts = []
                for kt in range(KT):
                    if s == "a" and kt == 0:
                        load_w1("q")
                    ft = fpool.tile([64, 2 * BN], F8E4, tag=f"f{kt}")
                    nc.sync.dma_start(ft[:], feat[64 * kt:64 * (kt + 1), :])
                    fts.append(ft)
                if s == "a":
                    w2sb["q"] = wpool.tile([128, KT * INNER], BF16, tag="w2q",
                                           name="w2qsb")
                    nc.sync.dma_start(
                        w2sb["q"][:].rearrange("p (kt i) -> p kt i", kt=KT),
                        w2["q"].rearrange("(kt p) i -> p kt i", p=128))
                    load_w("k")
                    load_w("v")
                for p in "qkv":
                    hts = []
                    for t in range(CT):
                        psH = psHp.tile([128, 1024], F32, tag="psH")
                        for lo, hi in CH:
                            for kt in range(KT):
                                nc.tensor.matmul(
                                    psH[:, lo:hi],
                                    w1sb[p][:].rearrange(
                                        "p (kt two c) -> p kt two c",
                                        kt=KT, two=2)[
                                        :, kt, :, 128 * t:128 * t + 128],
                                    fts[kt][:].rearrange(
                                        "p (two n) -> p two n", two=2)[
                                        :, :, lo:hi],
                                    start=(kt == 0), stop=(kt == KT - 1),
                                    perf_mode=mybir.MatmulPerfMode.DoubleRow)
                        ht = hpool.tile([128, BN], BF16, tag=f"h{t}")
                        if t % 2 == 0:
                            nc.scalar.activation(ht[:], psH[:, 0:BN], AF.Relu)
                        else:
                            nc.vector.tensor_scalar_max(ht[:], psH[:, 0:BN],
                                                        0.0)
                        hts.append(ht)
                    psOs = [psOp.tile([INNER, 512], F32, tag="psOa",
                                      name="psOa"),
                            psOp.tile([INNER, 512], F32, tag="psOb",
                                      name="psOb")]
                    ot = opool.tile([INNER, BN], F16, tag="out")
                    for ci, (lo, hi) in enumerate(CH):
                        for kt in range(KT):
                            nc.tensor.matmul(
                                psOs[ci][:, 0:hi - lo],
                                w2sb[p][:, INNER * kt:INNER * (kt + 1)],
                                hts[kt][:, lo:hi],
                                start=(kt == 0), stop=(kt == KT - 1))
                        nc.scalar.copy(ot[:, lo:hi], psOs[ci][:, 0:hi - lo])
                        nc.sync.dma_start(outs[(s, p)][:, lo:hi],
                                          ot[:, lo:hi])

    _split_multi_waits(nc)
    return nc


# ---------------------------------------------------------------- prog2

def build_prog2():
    """Attention program, sharded over p (this core's 8 b-batches).

    Unified 64-stage software pipeline; every stage produces 1600 score
    columns in a [128, 2048] PSUM tile (4 banks, double-buffered = all 8
    banks), does ONE 1600-wide exp on ACT (the bottleneck engine), then
    reuses the exp-consumed banks of the same tile as the aligned-matmul
    accumulator (carve-after-read; subtile deps order the WAR hazard).
    Stage t+1's score matmuls are emitted before stage t's aligned
    matmuls so PE always has score work ready when ACT finishes an exp.

      path1 stage j (32): scores for q-pair (2j, 2j+1) over this core's
        800 (p, n) columns; q0 at S cols 0:800, q1 at 1024:1824; exp via
        a strided [100, 2, 800] AP; aligned A at cols 0:800.
      path2 stage (p, k) (32): scores for 1600 (q n) columns
        [1600k, 1600k+1600) against kb[p]; aligned A groups at cols
        0:400 and 512:912; strided copy out.

    Outputs (identical layout to the previous version; host unchanged):
      as1o [128, 32*800] bf16, as2o [128, 32*800] bf16
    """
    nc = bass.Bass("TRN2", target_bir_lowering=False, debug=False,
                   num_devices=CORES)
    din = {}
    for name, shape, dt in [
        ("kaTdr", [32, 2 * B * MP], F8E4), ("qaTdr", [32, 2 * B * N], F8E4),
        ("qbTdr", [32, 2 * BN], F8E4), ("kbTdr", [32, 2 * PB * MP], F8E4),
        ("vaL", [N, (B // 2) * 128], F16), ("vaR", [N, (B // 2) * 128], F16),
        ("vbL", [N, PB * 128], F16), ("vbR", [N, PB * 128], F16),
    ]:
        din[name] = nc.dram_tensor(name, shape, dt, kind="ExternalInput").ap()
    as1o = nc.dram_tensor("as1o", [128, 32 * BN], BF16,
                          kind="ExternalOutput").ap()
    as2o = nc.dram_tensor("as2o", [128, 32 * 800], BF16,
                          kind="ExternalOutput").ap()

    with tile.TileContext(nc) as tc:
        from contextlib import ExitStack
        with ExitStack() as ctx:
            inp = ctx.enter_context(tc.tile_pool(name="inp", bufs=1))
            sb = {}

            def load(name):
                ap = din[name]
                t = inp.tile(list(ap.shape), ap.dtype, tag=name,
                             name=f"sb_{name}")
                nc.sync.dma_start(t[:], ap[:])
                sb[name] = t

            # Input DMAs, hot-first. All on the SP (sync) queue, issued
            # before any output DMA so no wait ever blocks the SP SEQ.
            ka_t = inp.tile([32, 2 * B * MP], F8E4, tag="kaTdr",
                            name="sb_kaTdr")
            sb["kaTdr"] = ka_t
            ka3d = din["kaTdr"].rearrange("p (two q m) -> p two q m",
                                          two=2, q=B)
            ka3s = ka_t[:].rearrange("p (two q m) -> p two q m", two=2, q=B)
            load("qbTdr")
            nc.sync.dma_start(ka3s[:, :, 0:8, :], ka3d[:, :, 0:8, :])
            va_t = {}
            for nm in ("vaL", "vaR"):
                va_t[nm] = inp.tile([N, (B // 2) * 128], F16, tag=nm,
                                    name=f"sb_{nm}")
                nc.sync.dma_start(va_t[nm][:, 0:512], din[nm][:, 0:512])
            for nm in ("vaL", "vaR"):
                nc.sync.dma_start(va_t[nm][:, 512:2048], din[nm][:, 512:2048])
            nc.sync.dma_start(ka3s[:, :, 8:32, :], ka3d[:, :, 8:32, :])
            for nm in ("vaL", "vaR"):
                nc.sync.dma_start(va_t[nm][:, 2048:4096],
                                  din[nm][:, 2048:4096])
            nc.sync.dma_start(ka3s[:, :, 32:64, :], ka3d[:, :, 32:64, :])
            qa_t = inp.tile([32, 2 * B * N], F8E4, tag="qaTdr",
                            name="sb_qaTdr")
            sb["qaTdr"] = qa_t
            qa3d = din["qaTdr"].rearrange("p (two n) -> p two n", two=2)
            qa3s = qa_t[:].rearrange("p (two n) -> p two n", two=2)
            nc.sync.dma_start(qa3s[:, :, 0:3200], qa3d[:, :, 0:3200])
            nc.sync.dma_start(qa3s[:, :, 3200:6400], qa3d[:, :, 3200:6400])
            for name in ("kbTdr", "vbL", "vbR"):
                load(name)

            epool = ctx.enter_context(tc.tile_pool(name="epool", bufs=3))
            mpool = ctx.enter_context(tc.tile_pool(name="mpool", bufs=8))
            psum = ctx.enter_context(
                tc.tile_pool(name="psum", bufs=2, space="PSUM"))

            ka3 = sb["kaTdr"][:].rearrange("p (two q m) -> p two q m",
                                           two=2, q=B)
            qb3 = sb["qbTdr"][:].rearrange("p (two n) -> p two n", two=2)
            kb3 = sb["kbTdr"][:].rearrange("p (two b m) -> p two b m",
                                           two=2, b=PB)
            qa3 = sb["qaTdr"][:].rearrange("p (two n) -> p two n", two=2)

            NS = 64  # 32 path1 pair-stages + 32 path2 (p, k) chunk-stages
            live = {}  # stage -> (sa_tile, E_tile)

            def emit_front(t):
                """Score matmuls + exp for stage t (into a fresh SA tile)."""
                sa = psum.tile([128, 2048], F32, tag="SA", name=f"SA{t % 2}")
                E = epool.tile([100, 1600], F16, tag="E")
                sa_ap = sa[:]
                if t < 32:
                    j = t
                    for qi, q in enumerate((2 * j, 2 * j + 1)):
                        base = 1024 * qi
                        for lo, hi in ((0, 512), (512, 800)):
                            nc.tensor.matmul(
                                sa_ap[0:100, base + lo:base + hi],
                                ka3[:, :, q, 0:N], qb3[:, :, lo:hi],
                                start=True, stop=True, perf_mode=DR)
                    sview = sa_ap[0:100].rearrange("p (two c) -> p two c",
                                                   two=2)[:, :, 0:800]
                    nc.scalar.activation(
                        E[:].rearrange("p (two c) -> p two c", two=2),
                        sview, AF.Exp, scale=0.125)
                else:
                    p, k = (t - 32) // 4, (t - 32) % 4
                    c0 = 1600 * k
                    for lo, hi in ((0, 512), (512, 1024), (1024, 1536),
                                   (1536, 1600)):
                        nc.tensor.matmul(
                            sa_ap[0:100, lo:hi], kb3[:, :, p, 0:N],
                            qa3[:, :, c0 + lo:c0 + hi],
                            start=True, stop=True, perf_mode=DR)
                    nc.scalar.activation(E[:], sa_ap[0:100, 0:1600],
                                         AF.Exp, scale=0.125)
                live[t] = (sa, E)

            def emit_back(t):
                """Aligned matmuls into stage t's exp-consumed banks, copy
                out, DMA."""
                sa, E = live.pop(t)
                sa_ap = sa[:]
                As = mpool.tile([128, 800], BF16, tag="As")
                if t < 32:
                    j = t
                    E3 = E[:].rearrange("p (two c) -> p two c", two=2)
                    vaLs = va_t["vaL"][:, 128 * j:128 * (j + 1)]
                    vaRs = va_t["vaR"][:, 128 * j:128 * (j + 1)]
                    for lo, hi in ((0, 512), (512, 800)):
                        nc.tensor.matmul(sa_ap[0:128, lo:hi], vaLs,
                                         E3[:, 0, lo:hi],
                                         start=True, stop=False)
                        nc.tensor.matmul(sa_ap[0:128, lo:hi], vaRs,
                                         E3[:, 1, lo:hi],
                                         start=False, stop=True)
                    nc.vector.tensor_copy(As[:], sa_ap[0:128, 0:800])
                    nc.sync.dma_start(as1o[:, BN * j:BN * (j + 1)], As[:])
                else:
                    p, k = (t - 32) // 4, (t - 32) % 4
                    vbLs = sb["vbL"][:, 128 * p:128 * (p + 1)]
                    vbRs = sb["vbR"][:, 128 * p:128 * (p + 1)]
                    for g in range(2):
                        off = 512 * g
                        nc.tensor.matmul(sa_ap[0:128, off:off + 400], vbLs,
                                         E[:, 800 * g:800 * g + 400],
                                         start=True, stop=False)
                        nc.tensor.matmul(sa_ap[0:128, off:off + 400], vbRs,
                                         E[:, 800 * g + 400:800 * (g + 1)],
                                         start=False, stop=True)
                    nc.vector.tensor_copy(
                        As[:].rearrange("p (g c) -> p g c", g=2),
                        sa_ap[0:128].rearrange("p (g c) -> p g c",
                                               g=4)[:, 0:2, 0:400])
                    nc.sync.dma_start(
                        as2o[:, 3200 * p + 800 * k:3200 * p + 800 * (k + 1)],
                        As[:])

            for t in range(NS + 1):
                if t < NS:
                    emit_front(t)
                if t >= 1:
                    emit_back(t - 1)

    _split_multi_waits(nc)
    return nc


# ---------------------------------------------------------------- host

_progs = {}


def _install_compile_cache():
    """Persist compiled NEFF-wrapped custom calls across processes: walrus
    compilation takes tens of seconds per program and bass2jax recompiles
    in every fresh process otherwise."""
    import hashlib
    import pathlib
    from concourse import bass2jax
    if getattr(bass2jax, "_ant_disk_cache", False):
        return
    bass2jax._ant_disk_cache = True
    orig = bass2jax.neuronx_cc_hook
    cdir = pathlib.Path(os.environ.get("BASS_NEFF_CACHE",
                                       "/tmp/bass_neff_cache"))
    try:
        cdir.mkdir(parents=True, exist_ok=True)
    except OSError:
        return

    def cached_hook(code, code_format, platform_version, file_prefix):
        try:
            key = hashlib.sha256(
                bytes(code) + b"|" + bytes(code_format)).hexdigest()
            path = cdir / f"{key}.neffcall"
            if path.exists():
                return 0, path.read_bytes()
        except Exception:
            return orig(code, code_format, platform_version, file_prefix)
        rc, blob = orig(code, code_format, platform_version, file_prefix)
        if rc == 0:
            try:
                tmp = path.with_suffix(f".tmp{os.getpid()}")
                tmp.write_bytes(blob)
                tmp.rename(path)
            except OSError:
                pass
        return rc, blob

    bass2jax.neuronx_cc_hook = cached_hook
    try:
        import libneuronxla
        if libneuronxla.neuronx_cc is orig:
            libneuronxla.neuronx_cc = cached_hook
    except ImportError:
        pass


def _get_progs():
    if "p1" not in _progs:
        _install_compile_cache()
        _progs["p1"] = build_prog1()
        _progs["p2"] = build_prog2()
    return _progs["p1"], _progs["p2"]


def _masters():
    import ml_dtypes
    m1 = np.zeros((128, 320), ml_dtypes.bfloat16)
    m1[0:64, 128] = 1.0   # up-plane (rows 0:64 of rhs) -> out row q
    m1[64:128, 129] = 1.0  # down-plane -> out row q+1
    m8 = np.zeros((128, 320), ml_dtypes.bfloat16)
    m8[0:64, 128] = 1.0
    m8[64:128, 136] = 1.0  # down-plane -> out row r0+8
    return m1, m8


def _dr_pack_k(x, pad_to=None):
    """Pack [K, M] (K contraction, even) into DoubleRow layout
    [K//2, 2*M] fp8e4 with k = (K//2)*s + p."""
    import ml_dtypes
    K = x.shape[0]
    h = K // 2
    arr = x.reshape(2, h, *x.shape[1:]).transpose(1, 0, *range(2, x.ndim + 1))
    return np.ascontiguousarray(arr.reshape(h, -1).astype(
        ml_dtypes.float8_e4m3fn))


def _dr_pack_k_padded(x, nblk, blk, pad):
    """[K, nblk*blk] -> DR fp8 [K//2, 2*nblk*pad] with each blk padded."""
    import ml_dtypes
    K = x.shape[0]
    h = K // 2
    a = x.reshape(2, h, nblk, blk).transpose(1, 0, 2, 3)
    z = np.zeros((h, 2, nblk, pad), np.float32)
    z[:, :, :, 0:blk] = a
    return np.ascontiguousarray(z.reshape(h, -1).astype(
        ml_dtypes.float8_e4m3fn))


def kernel(features_a, features_b, Wq1, Wq2, Wk1, Wk2, Wv1, Wv2):
    import ml_dtypes
    nc1, nc2 = _get_progs()
    cc = np.ascontiguousarray

    fa = np.asarray(features_a, np.float32).reshape(B, C, N)
    fb = np.asarray(features_b, np.float32).reshape(B, C, N)

    def feat_dr(f_core):  # [PB, C, N] -> [KT1*64, 2*BN] fp8 DR
        fT = f_core.transpose(1, 0, 2).reshape(C, BN)
        a = fT.reshape(KT1, 2, 64, BN).transpose(0, 2, 1, 3)
        return cc(a.reshape(KT1 * 64, 2 * BN).astype(ml_dtypes.float8_e4m3fn))

    def w1_dr(W):  # [C, C] -> [KT1*64, 2*C] fp8 DR
        a = np.asarray(W, np.float32).reshape(KT1, 2, 64, C).transpose(
            0, 2, 1, 3)
        return cc(a.reshape(KT1 * 64, 2 * C).astype(ml_dtypes.float8_e4m3fn))

    ws = {"wq1dr": w1_dr(Wq1), "wk1dr": w1_dr(Wk1), "wv1dr": w1_dr(Wv1)}
    ws.update({k: cc(np.asarray(v, np.float32).astype(ml_dtypes.bfloat16))
               for k, v in (("wq2", Wq2), ("wk2", Wk2), ("wv2", Wv2))})

    in1 = [dict(fa8dr=feat_dr(fa[PB * i:PB * (i + 1)]),
                fb8dr=feat_dr(fb[PB * i:PB * (i + 1)]), **ws)
           for i in range(CORES)]
    res1 = run_bass_kernel_spmd(nc1, in1, core_ids=list(range(CORES)))

    def gather(name):
        return np.concatenate([res1.results[i][name] for i in range(CORES)],
                              axis=1)

    qaT, kaT, vaT = gather("qaT8"), gather("kaT8"), gather("vaT8")
    qbT = [res1.results[i]["qbT8"] for i in range(CORES)]
    kbT = [res1.results[i]["kbT8"] for i in range(CORES)]
    vbT = [res1.results[i]["vbT8"] for i in range(CORES)]

    # a-side derived tensors (shared by all cores)
    vaT32 = vaT.astype(np.float32)
    va_nm = cc(vaT.T)                       # [B*N, INNER] fp16
    na = np.maximum(np.sqrt((vaT32 * vaT32).sum(0)), EPS)
    vhat_aT = vaT32 / na[None, :]
    vaL = np.zeros((N, (B // 2) * 128), np.float16)
    vaR = np.zeros((N, (B // 2) * 128), np.float16)
    for j in range(B // 2):
        vaL[:, 128 * j:128 * j + 64] = va_nm[N * 2 * j:N * (2 * j + 1)]
        vaR[:, 128 * j + 64:128 * (j + 1)] = va_nm[N * (2 * j + 1):
                                                   N * (2 * j + 2)]
    vhat_aT2 = np.zeros((128, B * N // 2), np.float32)
    for j2 in range(8):
        vhat_aT2[0:64, 400 * j2:400 * (j2 + 1)] = \
            vhat_aT[:, 800 * j2:800 * j2 + 400]
        vhat_aT2[64:128, 400 * j2:400 * (j2 + 1)] = \
            vhat_aT[:, 800 * j2 + 400:800 * (j2 + 1)]
    m1, m8 = _masters()

    kaTdr = _dr_pack_k_padded(kaT.astype(np.float32), B, N, MP)
    qaTdr = _dr_pack_k(qaT.astype(np.float32))
    in2 = []
    vhat_bTs = []
    for i in range(CORES):
        vbT32 = vbT[i].astype(np.float32)
        vb_nm = cc(vbT[i].T)                # [BN, INNER] fp16
        nb = np.maximum(np.sqrt((vbT32 * vbT32).sum(0)), EPS)
        vhat_bT = vbT32 / nb[None, :]
        vbL = np.zeros((N, PB * 128), np.float16)
        vbR = np.zeros((N, PB * 128), np.float16)
        for p in range(PB):
            vbL[:, 128 * p:128 * p + 64] = vb_nm[N * p:N * (p + 1)]
            vbR[:, 128 * p + 64:128 * (p + 1)] = vb_nm[N * p:N * (p + 1)]
        vhat_bTs.append(vhat_bT)
        in2.append(dict(
            kaTdr=kaTdr, qaTdr=qaTdr,
            qbTdr=_dr_pack_k(qbT[i].astype(np.float32)),
            kbTdr=_dr_pack_k_padded(kbT[i].astype(np.float32), PB, N, MP),
            vaL=vaL, vaR=vaR, vbL=vbL, vbR=vbR))
    res2 = run_bass_kernel_spmd(nc2, in2, core_ids=list(range(CORES)))

    sim = np.zeros((B, B), np.float32)
    for i in range(CORES):
        r = res2.results[i]
        # path1: As1 col-block 800j = pair j (rows 0:64 -> q=2j,
        # rows 64:128 -> q=2j+1, cols (p, n)); dot/ny2 on host
        as1 = np.asarray(r["as1o"], np.float32).reshape(128, 32, 800)
        vb_h = vhat_bTs[i]                              # [64 i, 800 (p n)]
        ny2_1 = np.empty((64, 800), np.float32)
        dot1 = np.empty((64, 800), np.float32)
        ny2_1[0::2] = (as1[0:64] ** 2).sum(0)
        ny2_1[1::2] = (as1[64:128] ** 2).sum(0)
        dot1[0::2] = np.einsum('ijc,ic->jc', as1[0:64], vb_h)
        dot1[1::2] = np.einsum('ijc,ic->jc', as1[64:128], vb_h)
        cos1 = dot1 / np.maximum(np.sqrt(ny2_1), EPS)
        sim1 = cos1.reshape(64, PB, N).sum(-1)          # [q, p]

        # path2: As2 cols 3200p + 800g + 400h + c; rows 0:64 ->
        # qn = 800*(2g+h)+c, rows 64:128 -> +400; vhat_a [64, (g,h,half,c)]
        as2 = np.asarray(r["as2o"], np.float32).reshape(128, PB, 4, 2, 400)
        va4 = vhat_aT.reshape(64, 4, 2, 2, 400)         # [i, g, h, half, c]
        ny_lo = (as2[0:64] ** 2).sum(0).reshape(PB, 8, 400)
        ny_hi = (as2[64:128] ** 2).sum(0).reshape(PB, 8, 400)
        ny2_2 = np.concatenate([ny_lo, ny_hi], axis=2).reshape(PB, B * N)
        d_lo = np.einsum('ipghc,ighc->pghc', as2[0:64], va4[:, :, :, 0])
        d_hi = np.einsum('ipghc,ighc->pghc', as2[64:128], va4[:, :, :, 1])
        dot2 = np.concatenate([d_lo.reshape(PB, 8, 400),
                               d_hi.reshape(PB, 8, 400)],
                              axis=2).reshape(PB, B * N)
        cos2 = dot2 / np.maximum(np.sqrt(ny2_2), EPS)
        sim2 = cos2.reshape(PB, B, N).sum(-1)           # [p, q]

        sim[PB * i:PB * (i + 1)] = (sim1.T + sim2) / N
    return sim



# revision 6
# speedup vs baseline: 1.3266x; 1.3109x over previous
"""Trainium2 Bass kernel for nn_AttentionSimilarity.

Contract: kernel(**inputs) takes the FULL unsharded inputs (numpy) and
returns the FULL [64, 64] similarity matrix, distributing work across 8
NeuronCores internally.

Structure:
  prog1 (projections, sharded by batch): each core projects its 8
    a-batches and 8 b-batches through the three two-layer MLPs,
    emitting qaT/kaT/vaT/qbT/kbT/vbT chunks in [inner, (batch, n)]
    layout. Host gathers the a-side to full tensors.
  prog2 (attention, sharded by p = b-side batch): each core computes
    both attention paths for its 8 p's against all 64 q's, the cosine
    numerators/denominators via selector matmuls on the PE, and the
    per-(p,q) sums over n. Host assembles the [64, 64] output.

Math notes:
  - softmax feeds only cosine similarity, which is scale-invariant in
    the aligned vector, so the softmax max-shift and denominator cancel:
    softmax reduces to exp(scores/8).
  - the x-side cosine norm is folded on the host (vhat = v / max(|v|, eps)).
  - 1/max(|y|, eps) and the dot with vhat are applied on the host from
    the streamed-out aligned values.

Performance notes (vs the first working version):
  - prog1 W1 layer and both programs' score matmuls run in fp8e4 with
    MatmulPerfMode.DoubleRow (2 contraction rows per PE partition, 0.5
    cycles/output column): weights/features/q/k are DR-packed on the
    host ([K/2, 2, M] with k = (K/2)*s + p; lhsT m-blocks padded to
    MP=112 so the DR pair-stride stays 16-byte aligned).
  - the entire cosine stage (dot, squared-norm, rsqrt, mean over n)
    is computed on the HOST: the aligned values (As, bf16) stream out
    over the otherwise-idle DMA engines, deleting the M/SQ multiplies,
    all selector-reduce matmuls, the P1/P2 PSUM accumulators (freeing
    banks for aligned double-buffering), and the device epilogues.
    The device does projections, scores, softmax-exp and the aligned
    matmuls -- all of the O(B^2 N^2) compute.
  - warmup/tail: weight DMAs are split/consolidated so the first matmul
    starts as early as possible; prog1's W2 PSUM/copy/DMA pipeline is
    chunked per bank so stores drain during compute; path2 score tiles
    are 1536 columns (3 PSUM banks) to amortize the fixed per-
    instruction ACT access latency on the softmax exp, which is the
    saturated engine (~98% busy) in the final balance.
  - measured rel err vs fp32 reference: ~1.7e-3.

Dead end (measured): packing score tiles to 128 partitions by mixing
(q, m) across rows would cut exp columns 100/128, but the follow-up
aligned matmuls need operand slices at arbitrary partition offsets and
the PE requires base partition 0/32/64 (bass matmul assert); since
100 is not a multiple of 32, per-q slices of a packed layout are
unaddressable. The [m<=100, cols] score layout is forced.
"""

import os
import sys

sys.path.insert(0, "/opt/trn_rl_repo")
os.environ.setdefault("NEURON_RT_RESET_CORES", "1")

import numpy as np
import ml_dtypes  # noqa: F401  (bf16 host arrays)

import bass_rust
import concourse.bass as bass
import concourse.mybir as mybir
import concourse.tile as tile
from concourse.bass_utils import run_bass_kernel_spmd

F32 = mybir.dt.float32
F32R = mybir.dt.float32r
BF16 = mybir.dt.bfloat16
F16 = mybir.dt.float16
F8E4 = mybir.dt.float8e4
AF = mybir.ActivationFunctionType
DR = mybir.MatmulPerfMode.DoubleRow

B = 64          # batches per side
C = 512         # channels
N = 100         # H*W tokens per batch
INNER = 64      # projected dim
CORES = 8
PB = B // CORES  # batches per core (8)
BN = PB * N      # 800: (batch, n) columns per core chunk
EPS = 1e-8
KT1 = C // 128   # prog1 contraction tiles (4)
MP = 112         # fp8-DR padded m stride (112 % 16 == 0, >= N)

E1_BUFS = int(os.environ.get("K_E1_BUFS", "5"))
SEL_LAG = int(os.environ.get("K_SEL_LAG", "4"))
POOL_MOD1 = int(os.environ.get("K_POOL_MOD1", os.environ.get("K_POOL_MOD", "3")))
POOL_MOD2 = int(os.environ.get("K_POOL_MOD2", os.environ.get("K_POOL_MOD", "2")))
SEL_LAG2 = int(os.environ.get("K_SEL_LAG2", "4"))
M2_BUFS = int(os.environ.get("K_M2_BUFS", "8"))
MPOOL_MOD = int(os.environ.get("K_MPOOL_MOD", "0"))  # 0=never, k=every kth M on pool
M_BUFS = int(os.environ.get("K_M_BUFS", "8"))
E2_BUFS = int(os.environ.get("K_E2_BUFS", "3"))
S1_BUFS = int(os.environ.get("K_S1_BUFS", "2"))
A1_BUFS = int(os.environ.get("K_A1_BUFS", "1"))

_waitsplit_ctr = [0]


def _split_multi_waits(nc, max_waits=1):
    """This container's walrus build accepts at most ONE sync wait per
    instruction; Tile attaches several. Move extras onto preceding
    same-engine NoOps (engines are in-order, so semantics hold)."""
    n_split = 0
    for f in nc.m.functions:
        for blk in f.blocks:
            insts = list(blk.instructions)
            new_list = []
            changed = False
            for inst in insts:
                si = inst.sync_info
                waits = list(si.on_wait) if (si is not None and si.on_wait) else []
                if len(waits) > max_waits:
                    for w in waits[:-max_waits]:
                        _waitsplit_ctr[0] += 1
                        nop = mybir.InstNoOp(
                            name=f"I-waitsplit-{_waitsplit_ctr[0]}",
                            engine=inst.engine,
                            ins=[],
                            outs=[],
                            sync_info=bass_rust.SyncInfo(on_wait=[w], on_update=[]),
                        )
                        nc.register_instruction(nop, overwrite=True)
                        new_list.append(nop)
                        n_split += 1
                    si.on_wait = waits[-max_waits:]
                    inst.sync_info = si
                    changed = True
                new_list.append(inst)
            if changed:
                blk.instructions = new_list
    return n_split


# ---------------------------------------------------------------- prog1

def build_prog1():
    """Projection program. Per-core inputs:
      fa8dr, fb8dr: [KT*64, 2*BN] f8e4 DoubleRow-packed features
        (row kt*64+p, col (s, (b n)) holds feat[c = 128*kt + 64*s + p])
      wq1dr/...: [KT*64, 2*C] f8e4 DR weights (col (s, c_out))
      wq2/...: [C, INNER] bf16
    Outputs: qaT8/kaT8/vaT8/qbT8/kbT8/vbT8: [INNER, BN]  ([i, (b n)])
    """
    nc = bass.Bass("TRN2", target_bir_lowering=False, debug=False,
                   num_devices=CORES)
    fa8 = nc.dram_tensor("fa8dr", [KT1 * 64, 2 * BN], F8E4,
                         kind="ExternalInput").ap()
    fb8 = nc.dram_tensor("fb8dr", [KT1 * 64, 2 * BN], F8E4,
                         kind="ExternalInput").ap()
    w1 = {p: nc.dram_tensor(f"w{p}1dr", [KT1 * 64, 2 * C], F8E4,
                            kind="ExternalInput").ap()
          for p in "qkv"}
    w2 = {p: nc.dram_tensor(f"w{p}2", [C, INNER], BF16, kind="ExternalInput").ap()
          for p in "qkv"}
    outs = {(s, p): nc.dram_tensor(f"{p}{s}T8", [INNER, BN], F16,
                                   kind="ExternalOutput").ap()
            for s in "ab" for p in "qkv"}

    KT = KT1  # 4 contraction tiles of 128 (64 partitions x 2 DR)
    CT = C // 128  # 4 c_out tiles
    CH = [(0, 512), (512, BN)]  # psum-bank-aligned column chunks of BN

    with tile.TileContext(nc) as tc:
        with (
            tc.tile_pool(name="wpool", bufs=1) as wpool,
            tc.tile_pool(name="fpool", bufs=int(os.environ.get("K_F_BUFS", "3"))) as fpool,
            tc.tile_pool(name="hpool", bufs=int(os.environ.get("K_H_BUFS", "5"))) as hpool,
            tc.tile_pool(name="opool", bufs=int(os.environ.get("K_O_BUFS", "4"))) as opool,
            tc.tile_pool(name="psH", bufs=int(os.environ.get("K_PSH_BUFS", "3")), space="PSUM") as psHp,
            tc.tile_pool(name="psO", bufs=int(os.environ.get("K_PSO_BUFS", "1")), space="PSUM") as psOp,
        ):
            w1sb, w2sb = {}, {}

            def load_w1(p):
                wt = wpool.tile([64, KT * 2 * C], F8E4, tag=f"w1{p}",
                                name=f"w1{p}sb")
                wv = wt[:].rearrange("p (kt x) -> p kt x", kt=KT)
                dv = w1[p].rearrange("(kt p) x -> p kt x", p=64)
                nc.sync.dma_start(wv[:, 0:2], dv[:, 0:2])
                nc.sync.dma_start(wv[:, 2:KT], dv[:, 2:KT])
                w1sb[p] = wt

            def load_w(p):
                load_w1(p)
                w2sb[p] = wpool.tile([128, KT * INNER], BF16, tag=f"w2{p}",
                                     name=f"w2{p}sb")
                nc.sync.dma_start(
                    w2sb[p][:].rearrange("p (kt i) -> p kt i", kt=KT),
                    w2[p].rearrange("(kt p) i -> p kt i", p=128))

            for s, feat in (("a", fa8), ("b", fb8)):
                fts = []
                for kt in range(KT):
                    if s == "a" and kt == 0:
                        load_w1("q")
                    ft = fpool.tile([64, 2 * BN], F8E4, tag=f"f{kt}")
                    nc.sync.dma_start(ft[:], feat[64 * kt:64 * (kt + 1), :])
                    fts.append(ft)
                if s == "a":
                    w2sb["q"] = wpool.tile([128, KT * INNER], BF16, tag="w2q",
                                           name="w2qsb")
                    nc.sync.dma_start(
                        w2sb["q"][:].rearrange("p (kt i) -> p kt i", kt=KT),
                        w2["q"].rearrange("(kt p) i -> p kt i", p=128))
                    load_w("k")
                    load_w("v")
                for p in "qkv":
                    hts = []
                    for t in range(CT):
                        psH = psHp.tile([128, 1024], F32, tag="psH")
                        for lo, hi in CH:
                            for kt in range(KT):
                                nc.tensor.matmul(
                                    psH[:, lo:hi],
                                    w1sb[p][:].rearrange(
                                        "p (kt two c) -> p kt two c",
                                        kt=KT, two=2)[
                                        :, kt, :, 128 * t:128 * t + 128],
                                    fts[kt][:].rearrange(
                                        "p (two n) -> p two n", two=2)[
                                        :, :, lo:hi],
                                    start=(kt == 0), stop=(kt == KT - 1),
                                    perf_mode=mybir.MatmulPerfMode.DoubleRow)
                        ht = hpool.tile([128, BN], BF16, tag=f"h{t}")
                        if t % 2 == 0:
                            nc.scalar.activation(ht[:], psH[:, 0:BN], AF.Relu)
                        else:
                            nc.vector.tensor_scalar_max(ht[:], psH[:, 0:BN],
                                                        0.0)
                        hts.append(ht)
                    psOs = [psOp.tile([INNER, 512], F32, tag="psOa",
                                      name="psOa"),
                            psOp.tile([INNER, 512], F32, tag="psOb",
                                      name="psOb")]
                    ot = opool.tile([INNER, BN], F16, tag="out")
                    for ci, (lo, hi) in enumerate(CH):
                        for kt in range(KT):
                            nc.tensor.matmul(
                                psOs[ci][:, 0:hi - lo],
                                w2sb[p][:, INNER * kt:INNER * (kt + 1)],
                                hts[kt][:, lo:hi],
                                start=(kt == 0), stop=(kt == KT - 1))
                        nc.scalar.copy(ot[:, lo:hi], psOs[ci][:, 0:hi - lo])
                        nc.sync.dma_start(outs[(s, p)][:, lo:hi],
                                          ot[:, lo:hi])

    _split_multi_waits(nc)
    return nc


# ---------------------------------------------------------------- prog2

def build_prog2():
    """Attention program, sharded over p (this core's 8 b-batches).

    Unified 64-stage software pipeline; every stage produces 1600 score
    columns in a [128, 2048] PSUM tile (4 banks, double-buffered = all 8
    banks), does ONE 1600-wide exp on ACT (the bottleneck engine), then
    reuses the exp-consumed banks of the same tile as the aligned-matmul
    accumulator (carve-after-read; subtile deps order the WAR hazard).
    Stage t+1's score matmuls are emitted before stage t's aligned
    matmuls so PE always has score work ready when ACT finishes an exp.

      path1 stage j (32): scores for q-pair (2j, 2j+1) over this core's
        800 (p, n) columns; q0 at S cols 0:800, q1 at 1024:1824; exp via
        a strided [100, 2, 800] AP; aligned A at cols 0:800.
      path2 stage (p, k) (32): scores for 1600 (q n) columns
        [1600k, 1600k+1600) against kb[p]; aligned A groups at cols
        0:400 and 512:912; strided copy out.

    Outputs (identical layout to the previous version; host unchanged):
      as1o [128, 32*800] bf16, as2o [128, 32*800] bf16
    """
    nc = bass.Bass("TRN2", target_bir_lowering=False, debug=False,
                   num_devices=CORES)
    din = {}
    for name, shape, dt in [
        ("kaTdr", [32, 2 * B * MP], F8E4), ("qaTdr", [32, 2 * B * N], F8E4),
        ("qbTdr", [32, 2 * BN], F8E4), ("kbTdr", [32, 2 * PB * MP], F8E4),
        ("vaL", [N, (B // 2) * 128], F16), ("vaR", [N, (B // 2) * 128], F16),
        ("vbL", [N, PB * 128], F16), ("vbR", [N, PB * 128], F16),
    ]:
        din[name] = nc.dram_tensor(name, shape, dt, kind="ExternalInput").ap()
    as1o = nc.dram_tensor("as1o", [128, 32 * BN], BF16,
                          kind="ExternalOutput").ap()
    as2o = nc.dram_tensor("as2o", [128, 32 * 800], BF16,
                          kind="ExternalOutput").ap()

    with tile.TileContext(nc) as tc:
        from contextlib import ExitStack
        with ExitStack() as ctx:
            inp = ctx.enter_context(tc.tile_pool(name="inp", bufs=1))
            sb = {}

            def load(name):
                ap = din[name]
                t = inp.tile(list(ap.shape), ap.dtype, tag=name,
                             name=f"sb_{name}")
                nc.sync.dma_start(t[:], ap[:])
                sb[name] = t

            # Input DMAs, hot-first. All on the SP (sync) queue, issued
            # before any output DMA so no wait ever blocks the SP SEQ.
            ka_t = inp.tile([32, 2 * B * MP], F8E4, tag="kaTdr",
                            name="sb_kaTdr")
            sb["kaTdr"] = ka_t
            ka3d = din["kaTdr"].rearrange("p (two q m) -> p two q m",
                                          two=2, q=B)
            ka3s = ka_t[:].rearrange("p (two q m) -> p two q m", two=2, q=B)
            load("qbTdr")
            nc.sync.dma_start(ka3s[:, :, 0:8, :], ka3d[:, :, 0:8, :])
            va_t = {}
            for nm in ("vaL", "vaR"):
                va_t[nm] = inp.tile([N, (B // 2) * 128], F16, tag=nm,
                                    name=f"sb_{nm}")
                nc.sync.dma_start(va_t[nm][:, 0:512], din[nm][:, 0:512])
            for nm in ("vaL", "vaR"):
                nc.sync.dma_start(va_t[nm][:, 512:2048], din[nm][:, 512:2048])
            nc.sync.dma_start(ka3s[:, :, 8:32, :], ka3d[:, :, 8:32, :])
            for nm in ("vaL", "vaR"):
                nc.sync.dma_start(va_t[nm][:, 2048:4096],
                                  din[nm][:, 2048:4096])
            nc.sync.dma_start(ka3s[:, :, 32:64, :], ka3d[:, :, 32:64, :])
            qa_t = inp.tile([32, 2 * B * N], F8E4, tag="qaTdr",
                            name="sb_qaTdr")
            sb["qaTdr"] = qa_t
            qa3d = din["qaTdr"].rearrange("p (two n) -> p two n", two=2)
            qa3s = qa_t[:].rearrange("p (two n) -> p two n", two=2)
            nc.sync.dma_start(qa3s[:, :, 0:3200], qa3d[:, :, 0:3200])
            nc.sync.dma_start(qa3s[:, :, 3200:6400], qa3d[:, :, 3200:6400])
            for name in ("kbTdr", "vbL", "vbR"):
                load(name)

            epool = ctx.enter_context(tc.tile_pool(name="epool", bufs=4))
            mpool = ctx.enter_context(tc.tile_pool(name="mpool", bufs=8))
            spool = ctx.enter_context(
                tc.tile_pool(name="spool", bufs=2, space="PSUM"))
            apool = ctx.enter_context(
                tc.tile_pool(name="apool", bufs=2, space="PSUM"))

            ka3 = sb["kaTdr"][:].rearrange("p (two q m) -> p two q m",
                                           two=2, q=B)
            qb3 = sb["qbTdr"][:].rearrange("p (two n) -> p two n", two=2)
            kb3 = sb["kbTdr"][:].rearrange("p (two b m) -> p two b m",
                                           two=2, b=PB)
            qa3 = sb["qaTdr"][:].rearrange("p (two n) -> p two n", two=2)

            # The whole attention is one score stream of 102,400 columns:
            #   cols [1600j + 800h, +800)          = path1 pair j, q = 2j+h
            #   cols [51200 + 6400p + o, ...)      = path2 batch p
            # chunked into CW-wide exp stages (3-bank PSUM S tiles).
            CW = 1536
            SL = 102400
            NT = (SL + CW - 1) // CW  # 67 chunks (last 1024)
            segs = []  # (base, length, lhsT, rhs3)
            for j in range(B // 2):
                for h in range(2):
                    segs.append((1600 * j + 800 * h, 800,
                                 ka3[:, :, 2 * j + h, 0:N], qb3))
            for p in range(PB):
                segs.append((51200 + 6400 * p, 6400, kb3[:, :, p, 0:N], qa3))

            etiles = {}  # chunk index -> E tile

            def eslices(a, b):
                """Stream range [a, b) as a list of E-tile slices."""
                out = []
                while a < b:
                    t = a // CW
                    e = min(b, (t + 1) * CW)
                    out.append(etiles[t][:][:, a - t * CW:e - t * CW])
                    a = e
                return out

            def emit_front(t):
                """Score matmuls + one exp for stream chunk t."""
                c0, c1 = CW * t, min(CW * (t + 1), SL)
                sa = spool.tile([100, CW], F32, tag="S", name=f"S{t % 2}")
                E = epool.tile([100, CW], F16, tag="E")
                for base, ln, lhsT, rhs3 in segs:
                    a, b = max(c0, base), min(c1, base + ln)
                    while a < b:  # split at this S tile's 512-col banks
                        e = min(b, c0 + ((a - c0) // 512 + 1) * 512)
                        nc.tensor.matmul(
                            sa[:][:, a - c0:e - c0], lhsT,
                            rhs3[:, :, a - base:e - base],
                            start=True, stop=True, perf_mode=DR)
                        a = e
                nc.scalar.activation(E[:][:, 0:c1 - c0], sa[:][:, 0:c1 - c0],
                                     AF.Exp, scale=0.125)
                etiles[t] = E

            def emit_aligned(At, dcols, pairs):
                """At[:, d] = sum_i lhsT_i.T @ E[stream a_i + d] for
                d in [0, dcols). Dest is split at every E-chunk boundary of
                either source range so each dest interval is a complete
                start/stop accumulation group."""
                cuts = {0, dcols}
                for _, a in pairs:
                    c = (a // CW + 1) * CW
                    while c < a + dcols:
                        cuts.add(c - a)
                        c += CW
                cs = sorted(cuts)
                for d0, d1 in zip(cs, cs[1:]):
                    for i, (lhsT, a) in enumerate(pairs):
                        (sl,) = eslices(a + d0, a + d1)
                        nc.tensor.matmul(At[:][:, d0:d1], lhsT, sl,
                                         start=(i == 0),
                                         stop=(i == len(pairs) - 1),
                                         skip_group_check=True)

            as2_live = {}

            def emit_back(g):
                """Aligned matmuls + copy (+DMA) for finished group g."""
                if g < B // 2:  # path1 pair j
                    j = g
                    vaLs = va_t["vaL"][:, 128 * j:128 * (j + 1)]
                    vaRs = va_t["vaR"][:, 128 * j:128 * (j + 1)]
                    As = mpool.tile([128, 800], BF16, tag="As")
                    for lo, w in ((0, 512), (512, 288)):
                        At = apool.tile([128, 512], F32, tag="A")
                        emit_aligned(At, w,
                                     [(vaLs, 1600 * j + lo),
                                      (vaRs, 1600 * j + 800 + lo)])
                        nc.vector.tensor_copy(As[:][:, lo:lo + w],
                                              At[:][:, 0:w])
                    nc.sync.dma_start(as1o[:, BN * j:BN * (j + 1)], As[:])
                else:  # path2 800-col group
                    gg = g - B // 2
                    p, o8 = gg // 8, (gg % 8) * 800
                    base = 51200 + 6400 * p + o8
                    vbLs = sb["vbL"][:, 128 * p:128 * (p + 1)]
                    vbRs = sb["vbR"][:, 128 * p:128 * (p + 1)]
                    At = apool.tile([128, 512], F32, tag="A")
                    emit_aligned(At, 400, [(vbLs, base), (vbRs, base + 400)])
                    u = (gg % 8) // 2
                    if gg % 2 == 0:
                        as2_live[p] = mpool.tile([128, 800], BF16, tag="As", name="As2")
                    As2 = as2_live[p]
                    nc.vector.tensor_copy(
                        As2[:][:, 400 * (gg % 2):400 * (gg % 2) + 400],
                        At[:][:, 0:400])
                    if gg % 2 == 1:
                        nc.sync.dma_start(
                            as2o[:, 3200 * p + 800 * u:
                                 3200 * p + 800 * (u + 1)], As2[:])

            # group g ready once its last stream column's chunk is emitted
            ends = [1600 * (j + 1) for j in range(B // 2)] + \
                   [51200 + 6400 * (gg // 8) + 800 * (gg % 8) + 800
                    for gg in range(64)]
            ready = [(e + CW - 1) // CW - 1 for e in ends]
            for t in range(NT + 1):
                if t < NT:
                    emit_front(t)
                for g in range(len(ends)):
                    if ready[g] == t - 1:
                        emit_back(g)

    _split_multi_waits(nc)
    return nc


# ---------------------------------------------------------------- host

_progs = {}


def _install_compile_cache():
    """Persist compiled NEFF-wrapped custom calls across processes: walrus
    compilation takes tens of seconds per program and bass2jax recompiles
    in every fresh process otherwise."""
    import hashlib
    import pathlib
    from concourse import bass2jax
    if getattr(bass2jax, "_ant_disk_cache", False):
        return
    bass2jax._ant_disk_cache = True
    orig = bass2jax.neuronx_cc_hook
    cdir = pathlib.Path(os.environ.get("BASS_NEFF_CACHE",
                                       "/tmp/bass_neff_cache"))
    try:
        cdir.mkdir(parents=True, exist_ok=True)
    except OSError:
        return

    def cached_hook(code, code_format, platform_version, file_prefix):
        try:
            key = hashlib.sha256(
                bytes(code) + b"|" + bytes(code_format)).hexdigest()
            path = cdir / f"{key}.neffcall"
            if path.exists():
                return 0, path.read_bytes()
        except Exception:
            return orig(code, code_format, platform_version, file_prefix)
        rc, blob = orig(code, code_format, platform_version, file_prefix)
        if rc == 0:
            try:
                tmp = path.with_suffix(f".tmp{os.getpid()}")
                tmp.write_bytes(blob)
                tmp.rename(path)
            except OSError:
                pass
        return rc, blob

    bass2jax.neuronx_cc_hook = cached_hook
    try:
        import libneuronxla
        if libneuronxla.neuronx_cc is orig:
            libneuronxla.neuronx_cc = cached_hook
    except ImportError:
        pass


def _get_progs():
    if "p1" not in _progs:
        _install_compile_cache()
        _progs["p1"] = build_prog1()
        _progs["p2"] = build_prog2()
    return _progs["p1"], _progs["p2"]


def _masters():
    import ml_dtypes
    m1 = np.zeros((128, 320), ml_dtypes.bfloat16)
    m1[0:64, 128] = 1.0   # up-plane (rows 0:64 of rhs) -> out row q
    m1[64:128, 129] = 1.0  # down-plane -> out row q+1
    m8 = np.zeros((128, 320), ml_dtypes.bfloat16)
    m8[0:64, 128] = 1.0
    m8[64:128, 136] = 1.0  # down-plane -> out row r0+8
    return m1, m8


def _dr_pack_k(x, pad_to=None):
    """Pack [K, M] (K contraction, even) into DoubleRow layout
    [K//2, 2*M] fp8e4 with k = (K//2)*s + p."""
    import ml_dtypes
    K = x.shape[0]
    h = K // 2
    arr = x.reshape(2, h, *x.shape[1:]).transpose(1, 0, *range(2, x.ndim + 1))
    return np.ascontiguousarray(arr.reshape(h, -1).astype(
        ml_dtypes.float8_e4m3fn))


def _dr_pack_k_padded(x, nblk, blk, pad):
    """[K, nblk*blk] -> DR fp8 [K//2, 2*nblk*pad] with each blk padded."""
    import ml_dtypes
    K = x.shape[0]
    h = K // 2
    a = x.reshape(2, h, nblk, blk).transpose(1, 0, 2, 3)
    z = np.zeros((h, 2, nblk, pad), np.float32)
    z[:, :, :, 0:blk] = a
    return np.ascontiguousarray(z.reshape(h, -1).astype(
        ml_dtypes.float8_e4m3fn))


def kernel(features_a, features_b, Wq1, Wq2, Wk1, Wk2, Wv1, Wv2):
    import ml_dtypes
    nc1, nc2 = _get_progs()
    cc = np.ascontiguousarray

    fa = np.asarray(features_a, np.float32).reshape(B, C, N)
    fb = np.asarray(features_b, np.float32).reshape(B, C, N)

    def feat_dr(f_core):  # [PB, C, N] -> [KT1*64, 2*BN] fp8 DR
        fT = f_core.transpose(1, 0, 2).reshape(C, BN)
        a = fT.reshape(KT1, 2, 64, BN).transpose(0, 2, 1, 3)
        return cc(a.reshape(KT1 * 64, 2 * BN).astype(ml_dtypes.float8_e4m3fn))

    def w1_dr(W):  # [C, C] -> [KT1*64, 2*C] fp8 DR
        a = np.asarray(W, np.float32).reshape(KT1, 2, 64, C).transpose(
            0, 2, 1, 3)
        return cc(a.reshape(KT1 * 64, 2 * C).astype(ml_dtypes.float8_e4m3fn))

    ws = {"wq1dr": w1_dr(Wq1), "wk1dr": w1_dr(Wk1), "wv1dr": w1_dr(Wv1)}
    ws.update({k: cc(np.asarray(v, np.float32).astype(ml_dtypes.bfloat16))
               for k, v in (("wq2", Wq2), ("wk2", Wk2), ("wv2", Wv2))})

    in1 = [dict(fa8dr=feat_dr(fa[PB * i:PB * (i + 1)]),
                fb8dr=feat_dr(fb[PB * i:PB * (i + 1)]), **ws)
           for i in range(CORES)]
    res1 = run_bass_kernel_spmd(nc1, in1, core_ids=list(range(CORES)))

    def gather(name):
        return np.concatenate([res1.results[i][name] for i in range(CORES)],
                              axis=1)

    qaT, kaT, vaT = gather("qaT8"), gather("kaT8"), gather("vaT8")
    qbT = [res1.results[i]["qbT8"] for i in range(CORES)]
    kbT = [res1.results[i]["kbT8"] for i in range(CORES)]
    vbT = [res1.results[i]["vbT8"] for i in range(CORES)]

    # a-side derived tensors (shared by all cores)
    vaT32 = vaT.astype(np.float32)
    va_nm = cc(vaT.T)                       # [B*N, INNER] fp16
    na = np.maximum(np.sqrt((vaT32 * vaT32).sum(0)), EPS)
    vhat_aT = vaT32 / na[None, :]
    vaL = np.zeros((N, (B // 2) * 128), np.float16)
    vaR = np.zeros((N, (B // 2) * 128), np.float16)
    for j in range(B // 2):
        vaL[:, 128 * j:128 * j + 64] = va_nm[N * 2 * j:N * (2 * j + 1)]
        vaR[:, 128 * j + 64:128 * (j + 1)] = va_nm[N * (2 * j + 1):
                                                   N * (2 * j + 2)]
    vhat_aT2 = np.zeros((128, B * N // 2), np.float32)
    for j2 in range(8):
        vhat_aT2[0:64, 400 * j2:400 * (j2 + 1)] = \
            vhat_aT[:, 800 * j2:800 * j2 + 400]
        vhat_aT2[64:128, 400 * j2:400 * (j2 + 1)] = \
            vhat_aT[:, 800 * j2 + 400:800 * (j2 + 1)]
    m1, m8 = _masters()

    kaTdr = _dr_pack_k_padded(kaT.astype(np.float32), B, N, MP)
    qaTdr = _dr_pack_k(qaT.astype(np.float32))
    in2 = []
    vhat_bTs = []
    for i in range(CORES):
        vbT32 = vbT[i].astype(np.float32)
        vb_nm = cc(vbT[i].T)                # [BN, INNER] fp16
        nb = np.maximum(np.sqrt((vbT32 * vbT32).sum(0)), EPS)
        vhat_bT = vbT32 / nb[None, :]
        vbL = np.zeros((N, PB * 128), np.float16)
        vbR = np.zeros((N, PB * 128), np.float16)
        for p in range(PB):
            vbL[:, 128 * p:128 * p + 64] = vb_nm[N * p:N * (p + 1)]
            vbR[:, 128 * p + 64:128 * (p + 1)] = vb_nm[N * p:N * (p + 1)]
        vhat_bTs.append(vhat_bT)
        in2.append(dict(
            kaTdr=kaTdr, qaTdr=qaTdr,
            qbTdr=_dr_pack_k(qbT[i].astype(np.float32)),
            kbTdr=_dr_pack_k_padded(kbT[i].astype(np.float32), PB, N, MP),
            vaL=vaL, vaR=vaR, vbL=vbL, vbR=vbR))
    res2 = run_bass_kernel_spmd(nc2, in2, core_ids=list(range(CORES)))

    sim = np.zeros((B, B), np.float32)
    for i in range(CORES):
        r = res2.results[i]
        # path1: As1 col-block 800j = pair j (rows 0:64 -> q=2j,
        # rows 64:128 -> q=2j+1, cols (p, n)); dot/ny2 on host
        as1 = np.asarray(r["as1o"], np.float32).reshape(128, 32, 800)
        vb_h = vhat_bTs[i]                              # [64 i, 800 (p n)]
        ny2_1 = np.empty((64, 800), np.float32)
        dot1 = np.empty((64, 800), np.float32)
        ny2_1[0::2] = (as1[0:64] ** 2).sum(0)
        ny2_1[1::2] = (as1[64:128] ** 2).sum(0)
        dot1[0::2] = np.einsum('ijc,ic->jc', as1[0:64], vb_h)
        dot1[1::2] = np.einsum('ijc,ic->jc', as1[64:128], vb_h)
        cos1 = dot1 / np.maximum(np.sqrt(ny2_1), EPS)
        sim1 = cos1.reshape(64, PB, N).sum(-1)          # [q, p]

        # path2: As2 cols 3200p + 800g + 400h + c; rows 0:64 ->
        # qn = 800*(2g+h)+c, rows 64:128 -> +400; vhat_a [64, (g,h,half,c)]
        as2 = np.asarray(r["as2o"], np.float32).reshape(128, PB, 4, 2, 400)
        va4 = vhat_aT.reshape(64, 4, 2, 2, 400)         # [i, g, h, half, c]
        ny_lo = (as2[0:64] ** 2).sum(0).reshape(PB, 8, 400)
        ny_hi = (as2[64:128] ** 2).sum(0).reshape(PB, 8, 400)
        ny2_2 = np.concatenate([ny_lo, ny_hi], axis=2).reshape(PB, B * N)
        d_lo = np.einsum('ipghc,ighc->pghc', as2[0:64], va4[:, :, :, 0])
        d_hi = np.einsum('ipghc,ighc->pghc', as2[64:128], va4[:, :, :, 1])
        dot2 = np.concatenate([d_lo.reshape(PB, 8, 400),
                               d_hi.reshape(PB, 8, 400)],
                              axis=2).reshape(PB, B * N)
        cos2 = dot2 / np.maximum(np.sqrt(ny2_2), EPS)
        sim2 = cos2.reshape(PB, B, N).sum(-1)           # [p, q]

        sim[PB * i:PB * (i + 1)] = (sim1.T + sim2) / N
    return sim



# revision 9
# speedup vs baseline: 1.3759x; 1.0372x over previous
"""Trainium2 Bass kernel for nn_AttentionSimilarity.

Contract: kernel(**inputs) takes the FULL unsharded inputs (numpy) and
returns the FULL [64, 64] similarity matrix, distributing work across 8
NeuronCores internally.

Structure:
  prog1 (projections, sharded by batch): each core projects its 8
    a-batches and 8 b-batches through the three two-layer MLPs,
    emitting qaT/kaT/vaT/qbT/kbT/vbT chunks in [inner, (batch, n)]
    layout. Host gathers the a-side to full tensors.
  prog2 (attention, sharded by p = b-side batch): each core computes
    both attention paths for its 8 p's against all 64 q's, the cosine
    numerators/denominators via selector matmuls on the PE, and the
    per-(p,q) sums over n. Host assembles the [64, 64] output.

Math notes:
  - softmax feeds only cosine similarity, which is scale-invariant in
    the aligned vector, so the softmax max-shift and denominator cancel:
    softmax reduces to exp(scores/8).
  - the x-side cosine norm is folded on the host (vhat = v / max(|v|, eps)).
  - 1/max(|y|, eps) and the dot with vhat are applied on the host from
    the streamed-out aligned values.

Performance notes (vs the first working version):
  - prog1 W1 layer and both programs' score matmuls run in fp8e4 with
    MatmulPerfMode.DoubleRow (2 contraction rows per PE partition, 0.5
    cycles/output column): weights/features/q/k are DR-packed on the
    host ([K/2, 2, M] with k = (K/2)*s + p; lhsT m-blocks padded to
    MP=112 so the DR pair-stride stays 16-byte aligned).
  - the entire cosine stage (dot, squared-norm, rsqrt, mean over n)
    is computed on the HOST: the aligned values (As, bf16) stream out
    over the otherwise-idle DMA engines, deleting the M/SQ multiplies,
    all selector-reduce matmuls, the P1/P2 PSUM accumulators (freeing
    banks for aligned double-buffering), and the device epilogues.
    The device does projections, scores, softmax-exp and the aligned
    matmuls -- all of the O(B^2 N^2) compute.
  - warmup/tail: weight DMAs are split/consolidated so the first matmul
    starts as early as possible; prog1's W2 PSUM/copy/DMA pipeline is
    chunked per bank so stores drain during compute; path2 score tiles
    are 1536 columns (3 PSUM banks) to amortize the fixed per-
    instruction ACT access latency on the softmax exp, which is the
    saturated engine (~98% busy) in the final balance.
  - measured rel err vs fp32 reference: ~1.7e-3.

Dead end (measured): packing score tiles to 128 partitions by mixing
(q, m) across rows would cut exp columns 100/128, but the follow-up
aligned matmuls need operand slices at arbitrary partition offsets and
the PE requires base partition 0/32/64 (bass matmul assert); since
100 is not a multiple of 32, per-q slices of a packed layout are
unaddressable. The [m<=100, cols] score layout is forced.
"""

import os
import sys

sys.path.insert(0, "/opt/trn_rl_repo")
os.environ.setdefault("NEURON_RT_RESET_CORES", "1")

import numpy as np
import ml_dtypes  # noqa: F401  (bf16 host arrays)

import bass_rust
import concourse.bass as bass
import concourse.mybir as mybir
import concourse.tile as tile
from concourse.bass_utils import run_bass_kernel_spmd

F32 = mybir.dt.float32
F32R = mybir.dt.float32r
BF16 = mybir.dt.bfloat16
F16 = mybir.dt.float16
F8E4 = mybir.dt.float8e4
AF = mybir.ActivationFunctionType
DR = mybir.MatmulPerfMode.DoubleRow

B = 64          # batches per side
C = 512         # channels
N = 100         # H*W tokens per batch
INNER = 64      # projected dim
CORES = 8
PB = B // CORES  # batches per core (8)
BN = PB * N      # 800: (batch, n) columns per core chunk
EPS = 1e-8
KT1 = C // 128   # prog1 contraction tiles (4)
MP = 112         # fp8-DR padded m stride (112 % 16 == 0, >= N)

E1_BUFS = int(os.environ.get("K_E1_BUFS", "5"))
SEL_LAG = int(os.environ.get("K_SEL_LAG", "4"))
POOL_MOD1 = int(os.environ.get("K_POOL_MOD1", os.environ.get("K_POOL_MOD", "3")))
POOL_MOD2 = int(os.environ.get("K_POOL_MOD2", os.environ.get("K_POOL_MOD", "2")))
SEL_LAG2 = int(os.environ.get("K_SEL_LAG2", "4"))
M2_BUFS = int(os.environ.get("K_M2_BUFS", "8"))
MPOOL_MOD = int(os.environ.get("K_MPOOL_MOD", "0"))  # 0=never, k=every kth M on pool
M_BUFS = int(os.environ.get("K_M_BUFS", "8"))
E2_BUFS = int(os.environ.get("K_E2_BUFS", "3"))
S1_BUFS = int(os.environ.get("K_S1_BUFS", "2"))
A1_BUFS = int(os.environ.get("K_A1_BUFS", "1"))

_waitsplit_ctr = [0]


def _split_multi_waits(nc, max_waits=1):
    """This container's walrus build accepts at most ONE sync wait per
    instruction; Tile attaches several. Move extras onto preceding
    same-engine NoOps (engines are in-order, so semantics hold)."""
    n_split = 0
    for f in nc.m.functions:
        for blk in f.blocks:
            insts = list(blk.instructions)
            new_list = []
            changed = False
            for inst in insts:
                si = inst.sync_info
                waits = list(si.on_wait) if (si is not None and si.on_wait) else []
                if len(waits) > max_waits:
                    for w in waits[:-max_waits]:
                        _waitsplit_ctr[0] += 1
                        nop = mybir.InstNoOp(
                            name=f"I-waitsplit-{_waitsplit_ctr[0]}",
                            engine=inst.engine,
                            ins=[],
                            outs=[],
                            sync_info=bass_rust.SyncInfo(on_wait=[w], on_update=[]),
                        )
                        nc.register_instruction(nop, overwrite=True)
                        new_list.append(nop)
                        n_split += 1
                    si.on_wait = waits[-max_waits:]
                    inst.sync_info = si
                    changed = True
                new_list.append(inst)
            if changed:
                blk.instructions = new_list
    return n_split


# ---------------------------------------------------------------- prog1

def build_prog1():
    """Projection program, K=256-per-pass DoubleRow everywhere.

    Per-core inputs (all fp8e4 DR-packed on the host):
      f8:    [128, 2*2*2*BN]   features; [p, (side, b, s, n)] holds
                               feat_side[cin = 256b + 128s + p, n]
      w1dr:  [128, 3*2*2*C]    [p, (proj, b, s, cout)] = W1[cin, cout]
      w2dr:  [128, 3*2*2*64]   [p, (proj, b2, s2, i)] = W2[cout, i]
                               (cout = 256*b2 + 128*s2 + p)
    Outputs (f16): qko_a/qko_b [128, BN] (q rows 0:64, k rows 64:128),
      vo_a/vo_b [64, BN].

    Hidden activations are stored fp8e4 so the W2 layer also runs
    DoubleRow (0.5 cyc/col); h layout [128, (b2, s2, n)] makes the DR
    rhs a plain strided view of the relu outputs.
    """
    nc = bass.Bass("TRN2", target_bir_lowering=False, debug=False,
                   num_devices=CORES)
    f8 = nc.dram_tensor("f8", [128, 8 * BN], F8E4, kind="ExternalInput").ap()
    w1d = nc.dram_tensor("w1dr", [128, 12 * C], F8E4,
                         kind="ExternalInput").ap()
    w2d = nc.dram_tensor("w2dr", [128, 12 * INNER], F8E4,
                         kind="ExternalInput").ap()
    outs = {"a": nc.dram_tensor("qko_a", [128, BN], F16,
                                kind="ExternalOutput").ap(),
            "b": nc.dram_tensor("qko_b", [128, BN], F16,
                                kind="ExternalOutput").ap()}
    vouts = {"a": nc.dram_tensor("vo_a", [INNER, BN], F16,
                                 kind="ExternalOutput").ap(),
             "b": nc.dram_tensor("vo_b", [INNER, BN], F16,
                                 kind="ExternalOutput").ap()}
    CH = [(0, 512), (512, BN)]  # psum-bank-aligned column chunks of BN

    with tile.TileContext(nc) as tc:
        with (
            tc.tile_pool(name="wpool", bufs=1) as wpool,
            tc.tile_pool(name="hpool", bufs=3) as hpool,
            tc.tile_pool(name="opool", bufs=4) as opool,
            tc.tile_pool(name="psH", bufs=2, space="PSUM") as psHp,
            tc.tile_pool(name="psO", bufs=2, space="PSUM") as psOp,
        ):
            # weights + features, hot-first: W1q pass-b0, fa pass-b0, the
            # rest.  w1sb view: [p, proj, b, s, cout]; f view: [p, side,
            # b, s, n]; w2sb view: [p, proj, b2, s2, i].
            w1sb = wpool.tile([128, 12 * C], F8E4, tag="w1", name="w1sb")
            w1v = w1sb[:].rearrange("p (pr b s c) -> p pr b s c", pr=3, b=2,
                                    s=2)
            w1dv = w1d.rearrange("p (pr b s c) -> p pr b s c", pr=3, b=2, s=2)
            fsb = wpool.tile([128, 8 * BN], F8E4, tag="f", name="fsb")
            fv = fsb[:].rearrange("p (sd b s n) -> p sd b s n", sd=2, b=2,
                                  s=2)
            fdv = f8.rearrange("p (sd b s n) -> p sd b s n", sd=2, b=2, s=2)
            nc.sync.dma_start(w1v[:, 0, 0], w1dv[:, 0, 0])
            nc.sync.dma_start(fv[:, 0, 0], fdv[:, 0, 0])
            nc.sync.dma_start(w1v[:, 0, 1], w1dv[:, 0, 1])
            nc.sync.dma_start(fv[:, 0, 1], fdv[:, 0, 1])
            w2sb = wpool.tile([128, 12 * INNER], F8E4, tag="w2", name="w2sb")
            nc.sync.dma_start(w2sb[:], w2d[:])
            nc.sync.dma_start(w1v[:, 1:3], w1dv[:, 1:3])
            nc.sync.dma_start(fv[:, 1], fdv[:, 1])
            w2v = w2sb[:].rearrange("p (pr b s i) -> p pr b s i", pr=3, b=2,
                                    s=2)

            relu_alt = [0]

            for si, s in enumerate(("a", "b")):
                hts = {}
                for pi, p in enumerate("qkv"):
                    # ---- W1: 4 cout-tiles, 2 DR passes of K=256 each ----
                    ht = hpool.tile([128, 4 * BN], F8E4, tag="h",
                                    name=f"h{s}{p}")
                    hv = ht[:].rearrange("p (b s n) -> p b s n", b=2, s=2)
                    for t in range(4):
                        psH = psHp.tile([128, 1024], F32, tag="psH",
                                        name="psH")
                        for b in range(2):
                            for lo, hi in CH:
                                nc.tensor.matmul(
                                    psH[:, lo:hi],
                                    w1v[:, pi, b, :, 128 * t:128 * (t + 1)],
                                    fv[:, si, b, :, lo:hi],
                                    start=(b == 0), stop=(b == 1),
                                    perf_mode=DR)
                        relu_alt[0] ^= 1
                        if relu_alt[0]:
                            nc.scalar.activation(hv[:, t // 2, t % 2],
                                                 psH[:, 0:BN], AF.Relu)
                        else:
                            nc.vector.tensor_scalar_max(hv[:, t // 2, t % 2],
                                                        psH[:, 0:BN], 0.0)
                    hts[p] = hv
                # ---- W2: q into rows 0:64 + k into 64:128 of one psO ----
                psO = psOp.tile([128, 1024], F32, tag="psO", name="psOqk")
                for r0, p, pi in ((0, "q", 0), (64, "k", 1)):
                    for b2 in range(2):
                        for lo, hi in CH:
                            nc.tensor.matmul(
                                psO[r0:r0 + 64, lo:hi], w2v[:, pi, b2],
                                hts[p][:, b2, :, lo:hi],
                                start=(b2 == 0), stop=(b2 == 1),
                                perf_mode=DR)
                ot = opool.tile([128, BN], F16, tag="out", name="qkout")
                nc.vector.tensor_copy(ot[:], psO[:, 0:BN])
                nc.sync.dma_start(outs[s][:], ot[:])
                psV = psOp.tile([128, 1024], F32, tag="psO", name="psOv")
                for b2 in range(2):
                    for lo, hi in CH:
                        nc.tensor.matmul(
                            psV[0:64, lo:hi], w2v[:, 2, b2],
                            hts["v"][:, b2, :, lo:hi],
                            start=(b2 == 0), stop=(b2 == 1), perf_mode=DR)
                vt = opool.tile([INNER, BN], F16, tag="vout", name="vout")
                nc.scalar.copy(vt[:], psV[0:64, 0:BN])
                nc.sync.dma_start(vouts[s][:], vt[:])

    _split_multi_waits(nc)
    return nc


# ---------------------------------------------------------------- prog2

def build_prog2():
    """Attention program, sharded over p (this core's 8 b-batches).

    Unified 64-stage software pipeline; every stage produces 1600 score
    columns in a [128, 2048] PSUM tile (4 banks, double-buffered = all 8
    banks), does ONE 1600-wide exp on ACT (the bottleneck engine), then
    reuses the exp-consumed banks of the same tile as the aligned-matmul
    accumulator (carve-after-read; subtile deps order the WAR hazard).
    Stage t+1's score matmuls are emitted before stage t's aligned
    matmuls so PE always has score work ready when ACT finishes an exp.

      path1 stage j (32): scores for q-pair (2j, 2j+1) over this core's
        800 (p, n) columns; q0 at S cols 0:800, q1 at 1024:1824; exp via
        a strided [100, 2, 800] AP; aligned A at cols 0:800.
      path2 stage (p, k) (32): scores for 1600 (q n) columns
        [1600k, 1600k+1600) against kb[p]; aligned A groups at cols
        0:400 and 512:912; strided copy out.

    Outputs (identical layout to the previous version; host unchanged):
      as1o [128, 32*800] bf16, as2o [128, 32*800] bf16
    """
    nc = bass.Bass("TRN2", target_bir_lowering=False, debug=False,
                   num_devices=CORES)
    din = {}
    for name, shape, dt in [
        ("kaTdr", [32, 2 * B * MP], F8E4), ("qaTdr", [32, 2 * B * N], F8E4),
        ("qbTdr", [32, 2 * BN], F8E4), ("kbTdr", [32, 2 * PB * MP], F8E4),
        ("vaL", [N, (B // 2) * 128], F16), ("vaR", [N, (B // 2) * 128], F16),
        ("vbL", [N, PB * 128], F16), ("vbR", [N, PB * 128], F16),
    ]:
        din[name] = nc.dram_tensor(name, shape, dt, kind="ExternalInput").ap()
    as1o = nc.dram_tensor("as1o", [128, 32 * BN], BF16,
                          kind="ExternalOutput").ap()
    as2o = nc.dram_tensor("as2o", [128, 32 * 800], BF16,
                          kind="ExternalOutput").ap()

    with tile.TileContext(nc) as tc:
        from contextlib import ExitStack
        with ExitStack() as ctx:
            inp = ctx.enter_context(tc.tile_pool(name="inp", bufs=1))
            sb = {}

            def load(name):
                ap = din[name]
                t = inp.tile(list(ap.shape), ap.dtype, tag=name,
                             name=f"sb_{name}")
                nc.sync.dma_start(t[:], ap[:])
                sb[name] = t

            # Input DMAs, hot-first. All on the SP (sync) queue, issued
            # before any output DMA so no wait ever blocks the SP SEQ.
            ka_t = inp.tile([32, 2 * B * MP], F8E4, tag="kaTdr",
                            name="sb_kaTdr")
            sb["kaTdr"] = ka_t
            ka3d = din["kaTdr"].rearrange("p (two q m) -> p two q m",
                                          two=2, q=B)
            ka3s = ka_t[:].rearrange("p (two q m) -> p two q m", two=2, q=B)
            load("qbTdr")
            nc.sync.dma_start(ka3s[:, :, 0:8, :], ka3d[:, :, 0:8, :])
            va_t = {}
            for nm in ("vaL", "vaR"):
                va_t[nm] = inp.tile([N, (B // 2) * 128], F16, tag=nm,
                                    name=f"sb_{nm}")
                nc.sync.dma_start(va_t[nm][:, 0:512], din[nm][:, 0:512])
            for nm in ("vaL", "vaR"):
                nc.sync.dma_start(va_t[nm][:, 512:2048], din[nm][:, 512:2048])
            nc.sync.dma_start(ka3s[:, :, 8:32, :], ka3d[:, :, 8:32, :])
            for nm in ("vaL", "vaR"):
                nc.sync.dma_start(va_t[nm][:, 2048:4096],
                                  din[nm][:, 2048:4096])
            nc.sync.dma_start(ka3s[:, :, 32:64, :], ka3d[:, :, 32:64, :])
            qa_t = inp.tile([32, 2 * B * N], F8E4, tag="qaTdr",
                            name="sb_qaTdr")
            sb["qaTdr"] = qa_t
            qa3d = din["qaTdr"].rearrange("p (two n) -> p two n", two=2)
            qa3s = qa_t[:].rearrange("p (two n) -> p two n", two=2)
            nc.sync.dma_start(qa3s[:, :, 0:3200], qa3d[:, :, 0:3200])
            nc.sync.dma_start(qa3s[:, :, 3200:6400], qa3d[:, :, 3200:6400])
            for name in ("kbTdr", "vbL", "vbR"):
                load(name)

            epool = ctx.enter_context(tc.tile_pool(name="epool", bufs=4))
            mpool = ctx.enter_context(tc.tile_pool(name="mpool", bufs=8))
            spool = ctx.enter_context(
                tc.tile_pool(name="spool", bufs=2, space="PSUM"))
            apool = ctx.enter_context(
                tc.tile_pool(name="apool", bufs=2, space="PSUM"))

            ka3 = sb["kaTdr"][:].rearrange("p (two q m) -> p two q m",
                                           two=2, q=B)
            qb3 = sb["qbTdr"][:].rearrange("p (two n) -> p two n", two=2)
            kb3 = sb["kbTdr"][:].rearrange("p (two b m) -> p two b m",
                                           two=2, b=PB)
            qa3 = sb["qaTdr"][:].rearrange("p (two n) -> p two n", two=2)

            # The whole attention is one score stream of 102,400 columns:
            #   cols [1600j + 800h, +800)          = path1 pair j, q = 2j+h
            #   cols [51200 + 6400p + o, ...)      = path2 batch p
            # chunked into CW-wide exp stages (3-bank PSUM S tiles).
            CW = 1536
            SL = 102400
            NT = (SL + CW - 1) // CW  # 67 chunks (last 1024)
            segs = []  # (base, length, lhsT, rhs3)
            for j in range(B // 2):
                for h in range(2):
                    segs.append((1600 * j + 800 * h, 800,
                                 ka3[:, :, 2 * j + h, 0:N], qb3))
            for p in range(PB):
                segs.append((51200 + 6400 * p, 6400, kb3[:, :, p, 0:N], qa3))

            etiles = {}  # chunk index -> E tile

            def eslices(a, b):
                """Stream range [a, b) as a list of E-tile slices."""
                out = []
                while a < b:
                    t = a // CW
                    e = min(b, (t + 1) * CW)
                    out.append(etiles[t][:][:, a - t * CW:e - t * CW])
                    a = e
                return out

            def emit_front(t):
                """Score matmuls + one exp for stream chunk t."""
                c0, c1 = CW * t, min(CW * (t + 1), SL)
                sa = spool.tile([100, CW], F32, tag="S", name=f"S{t % 2}")
                E = epool.tile([100, CW], F16, tag="E")
                for base, ln, lhsT, rhs3 in segs:
                    a, b = max(c0, base), min(c1, base + ln)
                    while a < b:  # split at this S tile's 512-col banks
                        e = min(b, c0 + ((a - c0) // 512 + 1) * 512)
                        nc.tensor.matmul(
                            sa[:][:, a - c0:e - c0], lhsT,
                            rhs3[:, :, a - base:e - base],
                            start=True, stop=True, perf_mode=DR)
                        a = e
                nc.scalar.activation(E[:][:, 0:c1 - c0], sa[:][:, 0:c1 - c0],
                                     AF.Exp, scale=0.125)
                etiles[t] = E

            def emit_aligned(At, dcols, pairs):
                """At[:, d] = sum_i lhsT_i.T @ E[stream a_i + d] for
                d in [0, dcols). Dest is split at every E-chunk boundary of
                either source range so each dest interval is a complete
                start/stop accumulation group."""
                cuts = {0, dcols}
                for _, a in pairs:
                    c = (a // CW + 1) * CW
                    while c < a + dcols:
                        cuts.add(c - a)
                        c += CW
                cs = sorted(cuts)
                for d0, d1 in zip(cs, cs[1:]):
                    for i, (lhsT, a) in enumerate(pairs):
                        (sl,) = eslices(a + d0, a + d1)
                        nc.tensor.matmul(At[:][:, d0:d1], lhsT, sl,
                                         start=(i == 0),
                                         stop=(i == len(pairs) - 1),
                                         skip_group_check=True)

            as2_live = {}

            def emit_back(g):
                """Aligned matmuls + copy (+DMA) for finished group g."""
                if g < B // 2:  # path1 pair j
                    j = g
                    vaLs = va_t["vaL"][:, 128 * j:128 * (j + 1)]
                    vaRs = va_t["vaR"][:, 128 * j:128 * (j + 1)]
                    As = mpool.tile([128, 800], BF16, tag="As")
                    for lo, w in ((0, 512), (512, 288)):
                        At = apool.tile([128, 512], F32, tag="A")
                        emit_aligned(At, w,
                                     [(vaLs, 1600 * j + lo),
                                      (vaRs, 1600 * j + 800 + lo)])
                        nc.vector.tensor_copy(As[:][:, lo:lo + w],
                                              At[:][:, 0:w])
                    nc.sync.dma_start(as1o[:, BN * j:BN * (j + 1)], As[:])
                else:  # path2 800-col group
                    gg = g - B // 2
                    p, o8 = gg // 8, (gg % 8) * 800
                    base = 51200 + 6400 * p + o8
                    vbLs = sb["vbL"][:, 128 * p:128 * (p + 1)]
                    vbRs = sb["vbR"][:, 128 * p:128 * (p + 1)]
                    At = apool.tile([128, 512], F32, tag="A")
                    emit_aligned(At, 400, [(vbLs, base), (vbRs, base + 400)])
                    u = (gg % 8) // 2
                    if gg % 2 == 0:
                        as2_live[p] = mpool.tile([128, 800], BF16, tag="As", name="As2")
                    As2 = as2_live[p]
                    nc.vector.tensor_copy(
                        As2[:][:, 400 * (gg % 2):400 * (gg % 2) + 400],
                        At[:][:, 0:400])
                    if gg % 2 == 1:
                        nc.sync.dma_start(
                            as2o[:, 3200 * p + 800 * u:
                                 3200 * p + 800 * (u + 1)], As2[:])

            # group g ready once its last stream column's chunk is emitted
            ends = [1600 * (j + 1) for j in range(B // 2)] + \
                   [51200 + 6400 * (gg // 8) + 800 * (gg % 8) + 800
                    for gg in range(64)]
            ready = [(e + CW - 1) // CW - 1 for e in ends]
            for t in range(NT + 1):
                if t < NT:
                    emit_front(t)
                for g in range(len(ends)):
                    if ready[g] == t - 1:
                        emit_back(g)

    _split_multi_waits(nc)
    return nc


# ---------------------------------------------------------------- host

_progs = {}


def _install_compile_cache():
    """Persist compiled NEFF-wrapped custom calls across processes: walrus
    compilation takes tens of seconds per program and bass2jax recompiles
    in every fresh process otherwise."""
    import hashlib
    import pathlib
    from concourse import bass2jax
    if getattr(bass2jax, "_ant_disk_cache", False):
        return
    bass2jax._ant_disk_cache = True
    orig = bass2jax.neuronx_cc_hook
    cdir = pathlib.Path(os.environ.get("BASS_NEFF_CACHE",
                                       "/tmp/bass_neff_cache"))
    try:
        cdir.mkdir(parents=True, exist_ok=True)
    except OSError:
        return

    def cached_hook(code, code_format, platform_version, file_prefix):
        try:
            key = hashlib.sha256(
                bytes(code) + b"|" + bytes(code_format)).hexdigest()
            path = cdir / f"{key}.neffcall"
            if path.exists():
                return 0, path.read_bytes()
        except Exception:
            return orig(code, code_format, platform_version, file_prefix)
        rc, blob = orig(code, code_format, platform_version, file_prefix)
        if rc == 0:
            try:
                tmp = path.with_suffix(f".tmp{os.getpid()}")
                tmp.write_bytes(blob)
                tmp.rename(path)
            except OSError:
                pass
        return rc, blob

    bass2jax.neuronx_cc_hook = cached_hook
    try:
        import libneuronxla
        if libneuronxla.neuronx_cc is orig:
            libneuronxla.neuronx_cc = cached_hook
    except ImportError:
        pass


def _get_progs():
    if "p1" not in _progs:
        _install_compile_cache()
        _progs["p1"] = build_prog1()
        _progs["p2"] = build_prog2()
    return _progs["p1"], _progs["p2"]


def _masters():
    import ml_dtypes
    m1 = np.zeros((128, 320), ml_dtypes.bfloat16)
    m1[0:64, 128] = 1.0   # up-plane (rows 0:64 of rhs) -> out row q
    m1[64:128, 129] = 1.0  # down-plane -> out row q+1
    m8 = np.zeros((128, 320), ml_dtypes.bfloat16)
    m8[0:64, 128] = 1.0
    m8[64:128, 136] = 1.0  # down-plane -> out row r0+8
    return m1, m8


def _dr_pack_k(x, pad_to=None):
    """Pack [K, M] (K contraction, even) into DoubleRow layout
    [K//2, 2*M] fp8e4 with k = (K//2)*s + p."""
    import ml_dtypes
    K = x.shape[0]
    h = K // 2
    arr = x.reshape(2, h, *x.shape[1:]).transpose(1, 0, *range(2, x.ndim + 1))
    return np.ascontiguousarray(arr.reshape(h, -1).astype(
        ml_dtypes.float8_e4m3fn))


def _dr_pack_k_padded(x, nblk, blk, pad):
    """[K, nblk*blk] -> DR fp8 [K//2, 2*nblk*pad] with each blk padded."""
    import ml_dtypes
    K = x.shape[0]
    h = K // 2
    a = x.reshape(2, h, nblk, blk).transpose(1, 0, 2, 3)
    z = np.zeros((h, 2, nblk, pad), np.float32)
    z[:, :, :, 0:blk] = a
    return np.ascontiguousarray(z.reshape(h, -1).astype(
        ml_dtypes.float8_e4m3fn))


def kernel(features_a, features_b, Wq1, Wq2, Wk1, Wk2, Wv1, Wv2):
    import ml_dtypes
    nc1, nc2 = _get_progs()
    cc = np.ascontiguousarray
    FP8 = ml_dtypes.float8_e4m3fn

    fa = np.asarray(features_a, np.float32).reshape(B, C, N)
    fb = np.asarray(features_b, np.float32).reshape(B, C, N)

    def feat8(fa_core, fb_core):  # 2x [PB, C, N] -> [128, 8*BN] fp8
        # [sd, b, s, p, n] with cin = 256b + 128s + p -> [p, sd, b, s, n]
        fT = np.stack([fc.transpose(1, 0, 2).reshape(C, BN)
                       for fc in (fa_core, fb_core)])
        a = fT.reshape(2, 2, 2, 128, BN).transpose(3, 0, 1, 2, 4)
        return cc(a.reshape(128, 8 * BN).astype(FP8))

    def wpack(Ws):  # list of [C, M] -> [128, 3*2*2*M] fp8
        a = np.stack([np.asarray(W, np.float32) for W in Ws])
        M = a.shape[-1]
        a = a.reshape(3, 2, 2, 128, M).transpose(3, 0, 1, 2, 4)
        return cc(a.reshape(128, 12 * M).astype(FP8))

    ws = {"w1dr": wpack([Wq1, Wk1, Wv1]), "w2dr": wpack([Wq2, Wk2, Wv2])}

    in1 = [dict(f8=feat8(fa[PB * i:PB * (i + 1)], fb[PB * i:PB * (i + 1)]),
                **ws)
           for i in range(CORES)]
    res1 = run_bass_kernel_spmd(nc1, in1, core_ids=list(range(CORES)))

    qaT = np.concatenate([res1.results[i]["qko_a"][0:64]
                          for i in range(CORES)], axis=1)
    kaT = np.concatenate([res1.results[i]["qko_a"][64:128]
                          for i in range(CORES)], axis=1)
    vaT = np.concatenate([res1.results[i]["vo_a"]
                          for i in range(CORES)], axis=1)
    qbT = [res1.results[i]["qko_b"][0:64] for i in range(CORES)]
    kbT = [res1.results[i]["qko_b"][64:128] for i in range(CORES)]
    vbT = [res1.results[i]["vo_b"] for i in range(CORES)]

    # a-side derived tensors (shared by all cores)
    vaT32 = vaT.astype(np.float32)
    va_nm = cc(vaT.T)                       # [B*N, INNER] fp16
    na = np.maximum(np.sqrt((vaT32 * vaT32).sum(0)), EPS)
    vhat_aT = vaT32 / na[None, :]
    vaL = np.zeros((N, (B // 2) * 128), np.float16)
    vaR = np.zeros((N, (B // 2) * 128), np.float16)
    for j in range(B // 2):
        vaL[:, 128 * j:128 * j + 64] = va_nm[N * 2 * j:N * (2 * j + 1)]
        vaR[:, 128 * j + 64:128 * (j + 1)] = va_nm[N * (2 * j + 1):
                                                   N * (2 * j + 2)]
    vhat_aT2 = np.zeros((128, B * N // 2), np.float32)
    for j2 in range(8):
        vhat_aT2[0:64, 400 * j2:400 * (j2 + 1)] = \
            vhat_aT[:, 800 * j2:800 * j2 + 400]
        vhat_aT2[64:128, 400 * j2:400 * (j2 + 1)] = \
            vhat_aT[:, 800 * j2 + 400:800 * (j2 + 1)]
    m1, m8 = _masters()

    kaTdr = _dr_pack_k_padded(kaT.astype(np.float32), B, N, MP)
    qaTdr = _dr_pack_k(qaT.astype(np.float32))
    in2 = []
    vhat_bTs = []
    for i in range(CORES):
        vbT32 = vbT[i].astype(np.float32)
        vb_nm = cc(vbT[i].T)                # [BN, INNER] fp16
        nb = np.maximum(np.sqrt((vbT32 * vbT32).sum(0)), EPS)
        vhat_bT = vbT32 / nb[None, :]
        vbL = np.zeros((N, PB * 128), np.float16)
        vbR = np.zeros((N, PB * 128), np.float16)
        for p in range(PB):
            vbL[:, 128 * p:128 * p + 64] = vb_nm[N * p:N * (p + 1)]
            vbR[:, 128 * p + 64:128 * (p + 1)] = vb_nm[N * p:N * (p + 1)]
        vhat_bTs.append(vhat_bT)
        in2.append(dict(
            kaTdr=kaTdr, qaTdr=qaTdr,
            qbTdr=_dr_pack_k(qbT[i].astype(np.float32)),
            kbTdr=_dr_pack_k_padded(kbT[i].astype(np.float32), PB, N, MP),
            vaL=vaL, vaR=vaR, vbL=vbL, vbR=vbR))
    res2 = run_bass_kernel_spmd(nc2, in2, core_ids=list(range(CORES)))

    sim = np.zeros((B, B), np.float32)
    for i in range(CORES):
        r = res2.results[i]
        # path1: As1 col-block 800j = pair j (rows 0:64 -> q=2j,
        # rows 64:128 -> q=2j+1, cols (p, n)); dot/ny2 on host
        as1 = np.asarray(r["as1o"], np.float32).reshape(128, 32, 800)
        vb_h = vhat_bTs[i]                              # [64 i, 800 (p n)]
        ny2_1 = np.empty((64, 800), np.float32)
        dot1 = np.empty((64, 800), np.float32)
        ny2_1[0::2] = (as1[0:64] ** 2).sum(0)
        ny2_1[1::2] = (as1[64:128] ** 2).sum(0)
        dot1[0::2] = np.einsum('ijc,ic->jc', as1[0:64], vb_h)
        dot1[1::2] = np.einsum('ijc,ic->jc', as1[64:128], vb_h)
        cos1 = dot1 / np.maximum(np.sqrt(ny2_1), EPS)
        sim1 = cos1.reshape(64, PB, N).sum(-1)          # [q, p]

        # path2: As2 cols 3200p + 800g + 400h + c; rows 0:64 ->
        # qn = 800*(2g+h)+c, rows 64:128 -> +400; vhat_a [64, (g,h,half,c)]
        as2 = np.asarray(r["as2o"], np.float32).reshape(128, PB, 4, 2, 400)
        va4 = vhat_aT.reshape(64, 4, 2, 2, 400)         # [i, g, h, half, c]
        ny_lo = (as2[0:64] ** 2).sum(0).reshape(PB, 8, 400)
        ny_hi = (as2[64:128] ** 2).sum(0).reshape(PB, 8, 400)
        ny2_2 = np.concatenate([ny_lo, ny_hi], axis=2).reshape(PB, B * N)
        d_lo = np.einsum('ipghc,ighc->pghc', as2[0:64], va4[:, :, :, 0])
        d_hi = np.einsum('ipghc,ighc->pghc', as2[64:128], va4[:, :, :, 1])
        dot2 = np.concatenate([d_lo.reshape(PB, 8, 400),
                               d_hi.reshape(PB, 8, 400)],
                              axis=2).reshape(PB, B * N)
        cos2 = dot2 / np.maximum(np.sqrt(ny2_2), EPS)
        sim2 = cos2.reshape(PB, B, N).sum(-1)           # [p, q]

        sim[PB * i:PB * (i + 1)] = (sim1.T + sim2) / N
    return sim



# revision 10
# speedup vs baseline: 1.3826x; 1.0048x over previous
"""Trainium2 Bass kernel for nn_AttentionSimilarity.

Contract: kernel(**inputs) takes the FULL unsharded inputs (numpy) and
returns the FULL [64, 64] similarity matrix, distributing work across 8
NeuronCores internally.

Structure:
  prog1 (projections, sharded by batch): each core projects its 8
    a-batches and 8 b-batches through the three two-layer MLPs,
    emitting qaT/kaT/vaT/qbT/kbT/vbT chunks in [inner, (batch, n)]
    layout. Host gathers the a-side to full tensors.
  prog2 (attention, sharded by p = b-side batch): each core computes
    both attention paths for its 8 p's against all 64 q's, the cosine
    numerators/denominators via selector matmuls on the PE, and the
    per-(p,q) sums over n. Host assembles the [64, 64] output.

Math notes:
  - softmax feeds only cosine similarity, which is scale-invariant in
    the aligned vector, so the softmax max-shift and denominator cancel:
    softmax reduces to exp(scores/8).
  - the x-side cosine norm is folded on the host (vhat = v / max(|v|, eps)).
  - 1/max(|y|, eps) and the dot with vhat are applied on the host from
    the streamed-out aligned values.

Performance notes (vs the first working version):
  - prog1 W1 layer and both programs' score matmuls run in fp8e4 with
    MatmulPerfMode.DoubleRow (2 contraction rows per PE partition, 0.5
    cycles/output column): weights/features/q/k are DR-packed on the
    host ([K/2, 2, M] with k = (K/2)*s + p; lhsT m-blocks padded to
    MP=112 so the DR pair-stride stays 16-byte aligned).
  - the entire cosine stage (dot, squared-norm, rsqrt, mean over n)
    is computed on the HOST: the aligned values (As, bf16) stream out
    over the otherwise-idle DMA engines, deleting the M/SQ multiplies,
    all selector-reduce matmuls, the P1/P2 PSUM accumulators (freeing
    banks for aligned double-buffering), and the device epilogues.
    The device does projections, scores, softmax-exp and the aligned
    matmuls -- all of the O(B^2 N^2) compute.
  - warmup/tail: weight DMAs are split/consolidated so the first matmul
    starts as early as possible; prog1's W2 PSUM/copy/DMA pipeline is
    chunked per bank so stores drain during compute; path2 score tiles
    are 1536 columns (3 PSUM banks) to amortize the fixed per-
    instruction ACT access latency on the softmax exp, which is the
    saturated engine (~98% busy) in the final balance.
  - measured rel err vs fp32 reference: ~1.7e-3.

Dead end (measured): packing score tiles to 128 partitions by mixing
(q, m) across rows would cut exp columns 100/128, but the follow-up
aligned matmuls need operand slices at arbitrary partition offsets and
the PE requires base partition 0/32/64 (bass matmul assert); since
100 is not a multiple of 32, per-q slices of a packed layout are
unaddressable. The [m<=100, cols] score layout is forced.
"""

import os
import sys

sys.path.insert(0, "/opt/trn_rl_repo")
os.environ.setdefault("NEURON_RT_RESET_CORES", "1")

import numpy as np
import ml_dtypes  # noqa: F401  (bf16 host arrays)

import bass_rust
import concourse.bass as bass
import concourse.mybir as mybir
import concourse.tile as tile
from concourse.bass_utils import run_bass_kernel_spmd

F32 = mybir.dt.float32
F32R = mybir.dt.float32r
BF16 = mybir.dt.bfloat16
F16 = mybir.dt.float16
F8E4 = mybir.dt.float8e4
AF = mybir.ActivationFunctionType
DR = mybir.MatmulPerfMode.DoubleRow

B = 64          # batches per side
C = 512         # channels
N = 100         # H*W tokens per batch
INNER = 64      # projected dim
CORES = 8
PB = B // CORES  # batches per core (8)
BN = PB * N      # 800: (batch, n) columns per core chunk
EPS = 1e-8
KT1 = C // 128   # prog1 contraction tiles (4)
MP = 112         # fp8-DR padded m stride (112 % 16 == 0, >= N)

E1_BUFS = int(os.environ.get("K_E1_BUFS", "5"))
SEL_LAG = int(os.environ.get("K_SEL_LAG", "4"))
POOL_MOD1 = int(os.environ.get("K_POOL_MOD1", os.environ.get("K_POOL_MOD", "3")))
POOL_MOD2 = int(os.environ.get("K_POOL_MOD2", os.environ.get("K_POOL_MOD", "2")))
SEL_LAG2 = int(os.environ.get("K_SEL_LAG2", "4"))
M2_BUFS = int(os.environ.get("K_M2_BUFS", "8"))
MPOOL_MOD = int(os.environ.get("K_MPOOL_MOD", "0"))  # 0=never, k=every kth M on pool
M_BUFS = int(os.environ.get("K_M_BUFS", "8"))
E2_BUFS = int(os.environ.get("K_E2_BUFS", "3"))
S1_BUFS = int(os.environ.get("K_S1_BUFS", "2"))
A1_BUFS = int(os.environ.get("K_A1_BUFS", "1"))

_waitsplit_ctr = [0]


def _split_multi_waits(nc, max_waits=1):
    """This container's walrus build accepts at most ONE sync wait per
    instruction; Tile attaches several. Move extras onto preceding
    same-engine NoOps (engines are in-order, so semantics hold)."""
    n_split = 0
    for f in nc.m.functions:
        for blk in f.blocks:
            insts = list(blk.instructions)
            new_list = []
            changed = False
            for inst in insts:
                si = inst.sync_info
                waits = list(si.on_wait) if (si is not None and si.on_wait) else []
                if len(waits) > max_waits:
                    for w in waits[:-max_waits]:
                        _waitsplit_ctr[0] += 1
                        nop = mybir.InstNoOp(
                            name=f"I-waitsplit-{_waitsplit_ctr[0]}",
                            engine=inst.engine,
                            ins=[],
                            outs=[],
                            sync_info=bass_rust.SyncInfo(on_wait=[w], on_update=[]),
                        )
                        nc.register_instruction(nop, overwrite=True)
                        new_list.append(nop)
                        n_split += 1
                    si.on_wait = waits[-max_waits:]
                    inst.sync_info = si
                    changed = True
                new_list.append(inst)
            if changed:
                blk.instructions = new_list
    return n_split


# ---------------------------------------------------------------- prog1

def build_prog1():
    """Projection program, K=256-per-pass DoubleRow everywhere.

    Per-core inputs (all fp8e4 DR-packed on the host):
      f8:    [128, 2*2*2*BN]   features; [p, (side, b, s, n)] holds
                               feat_side[cin = 256b + 128s + p, n]
      w1dr:  [128, 3*2*2*C]    [p, (proj, b, s, cout)] = W1[cin, cout]
      w2dr:  [128, 3*2*2*64]   [p, (proj, b2, s2, i)] = W2[cout, i]
                               (cout = 256*b2 + 128*s2 + p)
    Outputs (f16): qko_a/qko_b [128, BN] (q rows 0:64, k rows 64:128),
      vo_a/vo_b [64, BN].

    Hidden activations are stored fp8e4 so the W2 layer also runs
    DoubleRow (0.5 cyc/col); h layout [128, (b2, s2, n)] makes the DR
    rhs a plain strided view of the relu outputs.
    """
    nc = bass.Bass("TRN2", target_bir_lowering=False, debug=False,
                   num_devices=CORES)
    f8 = nc.dram_tensor("f8", [128, 8 * BN], F8E4, kind="ExternalInput").ap()
    w1d = nc.dram_tensor("w1dr", [128, 12 * C], F8E4,
                         kind="ExternalInput").ap()
    w2d = nc.dram_tensor("w2dr", [128, 12 * INNER], F8E4,
                         kind="ExternalInput").ap()
    outs = {"a": nc.dram_tensor("qko_a", [128, BN], F16,
                                kind="ExternalOutput").ap(),
            "b": nc.dram_tensor("qko_b", [128, BN], F16,
                                kind="ExternalOutput").ap()}
    vouts = {"a": nc.dram_tensor("vo_a", [INNER, BN], F16,
                                 kind="ExternalOutput").ap(),
             "b": nc.dram_tensor("vo_b", [INNER, BN], F16,
                                 kind="ExternalOutput").ap()}
    CH = [(0, 512), (512, BN)]  # psum-bank-aligned column chunks of BN

    with tile.TileContext(nc) as tc:
        with (
            tc.tile_pool(name="wpool", bufs=1) as wpool,
            tc.tile_pool(name="hpool", bufs=3) as hpool,
            tc.tile_pool(name="opool", bufs=4) as opool,
            tc.tile_pool(name="psH", bufs=2, space="PSUM") as psHp,
            tc.tile_pool(name="psO", bufs=2, space="PSUM") as psOp,
        ):
            # weights + features, hot-first: W1q pass-b0, fa pass-b0, the
            # rest.  w1sb view: [p, proj, b, s, cout]; f view: [p, side,
            # b, s, n]; w2sb view: [p, proj, b2, s2, i].
            w1sb = wpool.tile([128, 12 * C], F8E4, tag="w1", name="w1sb")
            w1v = w1sb[:].rearrange("p (pr b s c) -> p pr b s c", pr=3, b=2,
                                    s=2)
            w1dv = w1d.rearrange("p (pr b s c) -> p pr b s c", pr=3, b=2, s=2)
            fsb = wpool.tile([128, 8 * BN], F8E4, tag="f", name="fsb")
            fv = fsb[:].rearrange("p (sd b s n) -> p sd b s n", sd=2, b=2,
                                  s=2)
            fdv = f8.rearrange("p (sd b s n) -> p sd b s n", sd=2, b=2, s=2)
            nc.sync.dma_start(w1v[:, 0, 0], w1dv[:, 0, 0])
            nc.sync.dma_start(fv[:, 0, 0], fdv[:, 0, 0])
            nc.sync.dma_start(w1v[:, 0, 1], w1dv[:, 0, 1])
            nc.sync.dma_start(fv[:, 0, 1], fdv[:, 0, 1])
            w2sb = wpool.tile([128, 12 * INNER], F8E4, tag="w2", name="w2sb")
            nc.sync.dma_start(w2sb[:], w2d[:])
            nc.sync.dma_start(w1v[:, 1:3], w1dv[:, 1:3])
            nc.sync.dma_start(fv[:, 1], fdv[:, 1])
            w2v = w2sb[:].rearrange("p (pr b s i) -> p pr b s i", pr=3, b=2,
                                    s=2)

            relu_alt = [0]

            for si, s in enumerate(("a", "b")):
                hts = {}
                for pi, p in enumerate("qkv"):
                    # ---- W1: 4 cout-tiles, 2 DR passes of K=256 each ----
                    ht = hpool.tile([128, 4 * BN], F8E4, tag="h",
                                    name=f"h{s}{p}")
                    hv = ht[:].rearrange("p (b s n) -> p b s n", b=2, s=2)
                    for t in range(4):
                        psH = psHp.tile([128, 1024], F32, tag="psH",
                                        name="psH")
                        for b in range(2):
                            for lo, hi in CH:
                                nc.tensor.matmul(
                                    psH[:, lo:hi],
                                    w1v[:, pi, b, :, 128 * t:128 * (t + 1)],
                                    fv[:, si, b, :, lo:hi],
                                    start=(b == 0), stop=(b == 1),
                                    perf_mode=DR)
                        relu_alt[0] ^= 1
                        if relu_alt[0]:
                            nc.scalar.activation(hv[:, t // 2, t % 2],
                                                 psH[:, 0:BN], AF.Relu)
                        else:
                            nc.vector.tensor_scalar_max(hv[:, t // 2, t % 2],
                                                        psH[:, 0:BN], 0.0)
                    hts[p] = hv
                # ---- W2: q into rows 0:64 (DR) + k into 64:128 of one
                # psO.  DoubleRow requires dst partition base 0, so the
                # k half runs plain fp8 (4 K=128 passes, 1 cyc/col). ----
                psO = psOp.tile([128, 1024], F32, tag="psO", name="psOqk")
                for b2 in range(2):
                    for lo, hi in CH:
                        nc.tensor.matmul(
                            psO[0:64, lo:hi], w2v[:, 0, b2],
                            hts["q"][:, b2, :, lo:hi],
                            start=(b2 == 0), stop=(b2 == 1),
                            perf_mode=DR)
                for b2 in range(2):
                    for s2 in range(2):
                        for lo, hi in CH:
                            nc.tensor.matmul(
                                psO[64:128, lo:hi], w2v[:, 1, b2, s2],
                                hts["k"][:, b2, s2, lo:hi],
                                start=(b2 == 0 and s2 == 0),
                                stop=(b2 == 1 and s2 == 1))
                ot = opool.tile([128, BN], F16, tag="out", name="qkout")
                nc.vector.tensor_copy(ot[:], psO[:, 0:BN])
                nc.sync.dma_start(outs[s][:], ot[:])
                psV = psOp.tile([128, 1024], F32, tag="psO", name="psOv")
                for b2 in range(2):
                    for lo, hi in CH:
                        nc.tensor.matmul(
                            psV[0:64, lo:hi], w2v[:, 2, b2],
                            hts["v"][:, b2, :, lo:hi],
                            start=(b2 == 0), stop=(b2 == 1), perf_mode=DR)
                vt = opool.tile([INNER, BN], F16, tag="vout", name="vout")
                nc.scalar.copy(vt[:], psV[0:64, 0:BN])
                nc.sync.dma_start(vouts[s][:], vt[:])

    _split_multi_waits(nc)
    return nc


# ---------------------------------------------------------------- prog2

def build_prog2():
    """Attention program, sharded over p (this core's 8 b-batches).

    Unified 64-stage software pipeline; every stage produces 1600 score
    columns in a [128, 2048] PSUM tile (4 banks, double-buffered = all 8
    banks), does ONE 1600-wide exp on ACT (the bottleneck engine), then
    reuses the exp-consumed banks of the same tile as the aligned-matmul
    accumulator (carve-after-read; subtile deps order the WAR hazard).
    Stage t+1's score matmuls are emitted before stage t's aligned
    matmuls so PE always has score work ready when ACT finishes an exp.

      path1 stage j (32): scores for q-pair (2j, 2j+1) over this core's
        800 (p, n) columns; q0 at S cols 0:800, q1 at 1024:1824; exp via
        a strided [100, 2, 800] AP; aligned A at cols 0:800.
      path2 stage (p, k) (32): scores for 1600 (q n) columns
        [1600k, 1600k+1600) against kb[p]; aligned A groups at cols
        0:400 and 512:912; strided copy out.

    Outputs (identical layout to the previous version; host unchanged):
      as1o [128, 32*800] bf16, as2o [128, 32*800] bf16
    """
    nc = bass.Bass("TRN2", target_bir_lowering=False, debug=False,
                   num_devices=CORES)
    din = {}
    for name, shape, dt in [
        ("kaTdr", [32, 2 * B * MP], F8E4), ("qaTdr", [32, 2 * B * N], F8E4),
        ("qbTdr", [32, 2 * BN], F8E4), ("kbTdr", [32, 2 * PB * MP], F8E4),
        ("vaL", [N, (B // 2) * 128], F16), ("vaR", [N, (B // 2) * 128], F16),
        ("vbL", [N, PB * 128], F16), ("vbR", [N, PB * 128], F16),
    ]:
        din[name] = nc.dram_tensor(name, shape, dt, kind="ExternalInput").ap()
    as1o = nc.dram_tensor("as1o", [128, 32 * BN], BF16,
                          kind="ExternalOutput").ap()
    as2o = nc.dram_tensor("as2o", [128, 32 * 800], BF16,
                          kind="ExternalOutput").ap()

    with tile.TileContext(nc) as tc:
        from contextlib import ExitStack
        with ExitStack() as ctx:
            inp = ctx.enter_context(tc.tile_pool(name="inp", bufs=1))
            sb = {}

            def load(name):
                ap = din[name]
                t = inp.tile(list(ap.shape), ap.dtype, tag=name,
                             name=f"sb_{name}")
                nc.sync.dma_start(t[:], ap[:])
                sb[name] = t

            # Input DMAs, hot-first. All on the SP (sync) queue, issued
            # before any output DMA so no wait ever blocks the SP SEQ.
            ka_t = inp.tile([32, 2 * B * MP], F8E4, tag="kaTdr",
                            name="sb_kaTdr")
            sb["kaTdr"] = ka_t
            ka3d = din["kaTdr"].rearrange("p (two q m) -> p two q m",
                                          two=2, q=B)
            ka3s = ka_t[:].rearrange("p (two q m) -> p two q m", two=2, q=B)
            load("qbTdr")
            nc.sync.dma_start(ka3s[:, :, 0:8, :], ka3d[:, :, 0:8, :])
            va_t = {}
            for nm in ("vaL", "vaR"):
                va_t[nm] = inp.tile([N, (B // 2) * 128], F16, tag=nm,
                                    name=f"sb_{nm}")
                nc.sync.dma_start(va_t[nm][:, 0:512], din[nm][:, 0:512])
            for nm in ("vaL", "vaR"):
                nc.sync.dma_start(va_t[nm][:, 512:2048], din[nm][:, 512:2048])
            nc.sync.dma_start(ka3s[:, :, 8:32, :], ka3d[:, :, 8:32, :])
            for nm in ("vaL", "vaR"):
                nc.sync.dma_start(va_t[nm][:, 2048:4096],
                                  din[nm][:, 2048:4096])
            nc.sync.dma_start(ka3s[:, :, 32:64, :], ka3d[:, :, 32:64, :])
            qa_t = inp.tile([32, 2 * B * N], F8E4, tag="qaTdr",
                            name="sb_qaTdr")
            sb["qaTdr"] = qa_t
            qa3d = din["qaTdr"].rearrange("p (two n) -> p two n", two=2)
            qa3s = qa_t[:].rearrange("p (two n) -> p two n", two=2)
            nc.sync.dma_start(qa3s[:, :, 0:3200], qa3d[:, :, 0:3200])
            nc.sync.dma_start(qa3s[:, :, 3200:6400], qa3d[:, :, 3200:6400])
            for name in ("kbTdr", "vbL", "vbR"):
                load(name)

            epool = ctx.enter_context(tc.tile_pool(name="epool", bufs=4))
            mpool = ctx.enter_context(tc.tile_pool(name="mpool", bufs=8))
            spool = ctx.enter_context(
                tc.tile_pool(name="spool", bufs=2, space="PSUM"))
            apool = ctx.enter_context(
                tc.tile_pool(name="apool", bufs=2, space="PSUM"))

            ka3 = sb["kaTdr"][:].rearrange("p (two q m) -> p two q m",
                                           two=2, q=B)
            qb3 = sb["qbTdr"][:].rearrange("p (two n) -> p two n", two=2)
            kb3 = sb["kbTdr"][:].rearrange("p (two b m) -> p two b m",
                                           two=2, b=PB)
            qa3 = sb["qaTdr"][:].rearrange("p (two n) -> p two n", two=2)

            # The whole attention is one score stream of 102,400 columns:
            #   cols [1600j + 800h, +800)          = path1 pair j, q = 2j+h
            #   cols [51200 + 6400p + o, ...)      = path2 batch p
            # chunked into CW-wide exp stages (3-bank PSUM S tiles).
            CW = 1536
            SL = 102400
            NT = (SL + CW - 1) // CW  # 67 chunks (last 1024)
            segs = []  # (base, length, lhsT, rhs3)
            for j in range(B // 2):
                for h in range(2):
                    segs.append((1600 * j + 800 * h, 800,
                                 ka3[:, :, 2 * j + h, 0:N], qb3))
            for p in range(PB):
                segs.append((51200 + 6400 * p, 6400, kb3[:, :, p, 0:N], qa3))

            etiles = {}  # chunk index -> E tile

            def eslices(a, b):
                """Stream range [a, b) as a list of E-tile slices."""
                out = []
                while a < b:
                    t = a // CW
                    e = min(b, (t + 1) * CW)
                    out.append(etiles[t][:][:, a - t * CW:e - t * CW])
                    a = e
                return out

            def emit_front(t):
                """Score matmuls + one exp for stream chunk t."""
                c0, c1 = CW * t, min(CW * (t + 1), SL)
                sa = spool.tile([100, CW], F32, tag="S", name=f"S{t % 2}")
                E = epool.tile([100, CW], F16, tag="E")
                for base, ln, lhsT, rhs3 in segs:
                    a, b = max(c0, base), min(c1, base + ln)
                    while a < b:  # split at this S tile's 512-col banks
                        e = min(b, c0 + ((a - c0) // 512 + 1) * 512)
                        nc.tensor.matmul(
                            sa[:][:, a - c0:e - c0], lhsT,
                            rhs3[:, :, a - base:e - base],
                            start=True, stop=True, perf_mode=DR)
                        a = e
                nc.scalar.activation(E[:][:, 0:c1 - c0], sa[:][:, 0:c1 - c0],
                                     AF.Exp, scale=0.125)
                etiles[t] = E

            def emit_aligned(At, dcols, pairs):
                """At[:, d] = sum_i lhsT_i.T @ E[stream a_i + d] for
                d in [0, dcols). Dest is split at every E-chunk boundary of
                either source range so each dest interval is a complete
                start/stop accumulation group."""
                cuts = {0, dcols}
                for _, a in pairs:
                    c = (a // CW + 1) * CW
                    while c < a + dcols:
                        cuts.add(c - a)
                        c += CW
                cs = sorted(cuts)
                for d0, d1 in zip(cs, cs[1:]):
                    for i, (lhsT, a) in enumerate(pairs):
                        (sl,) = eslices(a + d0, a + d1)
                        nc.tensor.matmul(At[:][:, d0:d1], lhsT, sl,
                                         start=(i == 0),
                                         stop=(i == len(pairs) - 1),
                                         skip_group_check=True)

            as2_live = {}

            def emit_back(g):
                """Aligned matmuls + copy (+DMA) for finished group g."""
                if g < B // 2:  # path1 pair j
                    j = g
                    vaLs = va_t["vaL"][:, 128 * j:128 * (j + 1)]
                    vaRs = va_t["vaR"][:, 128 * j:128 * (j + 1)]
                    As = mpool.tile([128, 800], BF16, tag="As")
                    for lo, w in ((0, 512), (512, 288)):
                        At = apool.tile([128, 512], F32, tag="A")
                        emit_aligned(At, w,
                                     [(vaLs, 1600 * j + lo),
                                      (vaRs, 1600 * j + 800 + lo)])
                        nc.vector.tensor_copy(As[:][:, lo:lo + w],
                                              At[:][:, 0:w])
                    nc.sync.dma_start(as1o[:, BN * j:BN * (j + 1)], As[:])
                else:  # path2 800-col group
                    gg = g - B // 2
                    p, o8 = gg // 8, (gg % 8) * 800
                    base = 51200 + 6400 * p + o8
                    vbLs = sb["vbL"][:, 128 * p:128 * (p + 1)]
                    vbRs = sb["vbR"][:, 128 * p:128 * (p + 1)]
                    At = apool.tile([128, 512], F32, tag="A")
                    emit_aligned(At, 400, [(vbLs, base), (vbRs, base + 400)])
                    u = (gg % 8) // 2
                    if gg % 2 == 0:
                        as2_live[p] = mpool.tile([128, 800], BF16, tag="As", name="As2")
                    As2 = as2_live[p]
                    nc.vector.tensor_copy(
                        As2[:][:, 400 * (gg % 2):400 * (gg % 2) + 400],
                        At[:][:, 0:400])
                    if gg % 2 == 1:
                        nc.sync.dma_start(
                            as2o[:, 3200 * p + 800 * u:
                                 3200 * p + 800 * (u + 1)], As2[:])

            # group g ready once its last stream column's chunk is emitted
            ends = [1600 * (j + 1) for j in range(B // 2)] + \
                   [51200 + 6400 * (gg // 8) + 800 * (gg % 8) + 800
                    for gg in range(64)]
            ready = [(e + CW - 1) // CW - 1 for e in ends]
            for t in range(NT + 1):
                if t < NT:
                    emit_front(t)
                for g in range(len(ends)):
                    if ready[g] == t - 1:
                        emit_back(g)

    _split_multi_waits(nc)
    return nc


# ---------------------------------------------------------------- host

_progs = {}


def _install_compile_cache():
    """Persist compiled NEFF-wrapped custom calls across processes: walrus
    compilation takes tens of seconds per program and bass2jax recompiles
    in every fresh process otherwise."""
    import hashlib
    import pathlib
    from concourse import bass2jax
    if getattr(bass2jax, "_ant_disk_cache", False):
        return
    bass2jax._ant_disk_cache = True
    orig = bass2jax.neuronx_cc_hook
    cdir = pathlib.Path(os.environ.get("BASS_NEFF_CACHE",
                                       "/tmp/bass_neff_cache"))
    try:
        cdir.mkdir(parents=True, exist_ok=True)
    except OSError:
        return

    def cached_hook(code, code_format, platform_version, file_prefix):
        try:
            key = hashlib.sha256(
                bytes(code) + b"|" + bytes(code_format)).hexdigest()
            path = cdir / f"{key}.neffcall"
            if path.exists():
                return 0, path.read_bytes()
        except Exception:
            return orig(code, code_format, platform_version, file_prefix)
        rc, blob = orig(code, code_format, platform_version, file_prefix)
        if rc == 0:
            try:
                tmp = path.with_suffix(f".tmp{os.getpid()}")
                tmp.write_bytes(blob)
                tmp.rename(path)
            except OSError:
                pass
        return rc, blob

    bass2jax.neuronx_cc_hook = cached_hook
    try:
        import libneuronxla
        if libneuronxla.neuronx_cc is orig:
            libneuronxla.neuronx_cc = cached_hook
    except ImportError:
        pass


def _get_progs():
    if "p1" not in _progs:
        _install_compile_cache()
        _progs["p1"] = build_prog1()
        _progs["p2"] = build_prog2()
    return _progs["p1"], _progs["p2"]


def _masters():
    import ml_dtypes
    m1 = np.zeros((128, 320), ml_dtypes.bfloat16)
    m1[0:64, 128] = 1.0   # up-plane (rows 0:64 of rhs) -> out row q
    m1[64:128, 129] = 1.0  # down-plane -> out row q+1
    m8 = np.zeros((128, 320), ml_dtypes.bfloat16)
    m8[0:64, 128] = 1.0
    m8[64:128, 136] = 1.0  # down-plane -> out row r0+8
    return m1, m8


def _dr_pack_k(x, pad_to=None):
    """Pack [K, M] (K contraction, even) into DoubleRow layout
    [K//2, 2*M] fp8e4 with k = (K//2)*s + p."""
    import ml_dtypes
    K = x.shape[0]
    h = K // 2
    arr = x.reshape(2, h, *x.shape[1:]).transpose(1, 0, *range(2, x.ndim + 1))
    return np.ascontiguousarray(arr.reshape(h, -1).astype(
        ml_dtypes.float8_e4m3fn))


def _dr_pack_k_padded(x, nblk, blk, pad):
    """[K, nblk*blk] -> DR fp8 [K//2, 2*nblk*pad] with each blk padded."""
    import ml_dtypes
    K = x.shape[0]
    h = K // 2
    a = x.reshape(2, h, nblk, blk).transpose(1, 0, 2, 3)
    z = np.zeros((h, 2, nblk, pad), np.float32)
    z[:, :, :, 0:blk] = a
    return np.ascontiguousarray(z.reshape(h, -1).astype(
        ml_dtypes.float8_e4m3fn))


def kernel(features_a, features_b, Wq1, Wq2, Wk1, Wk2, Wv1, Wv2):
    import ml_dtypes
    nc1, nc2 = _get_progs()
    cc = np.ascontiguousarray
    FP8 = ml_dtypes.float8_e4m3fn

    fa = np.asarray(features_a, np.float32).reshape(B, C, N)
    fb = np.asarray(features_b, np.float32).reshape(B, C, N)

    def feat8(fa_core, fb_core):  # 2x [PB, C, N] -> [128, 8*BN] fp8
        # [sd, b, s, p, n] with cin = 256b + 128s + p -> [p, sd, b, s, n]
        fT = np.stack([fc.transpose(1, 0, 2).reshape(C, BN)
                       for fc in (fa_core, fb_core)])
        a = fT.reshape(2, 2, 2, 128, BN).transpose(3, 0, 1, 2, 4)
        return cc(a.reshape(128, 8 * BN).astype(FP8))

    def wpack(Ws):  # list of [C, M] -> [128, 3*2*2*M] fp8
        a = np.stack([np.asarray(W, np.float32) for W in Ws])
        M = a.shape[-1]
        a = a.reshape(3, 2, 2, 128, M).transpose(3, 0, 1, 2, 4)
        return cc(a.reshape(128, 12 * M).astype(FP8))

    ws = {"w1dr": wpack([Wq1, Wk1, Wv1]), "w2dr": wpack([Wq2, Wk2, Wv2])}

    in1 = [dict(f8=feat8(fa[PB * i:PB * (i + 1)], fb[PB * i:PB * (i + 1)]),
                **ws)
           for i in range(CORES)]
    res1 = run_bass_kernel_spmd(nc1, in1, core_ids=list(range(CORES)))

    qaT = np.concatenate([res1.results[i]["qko_a"][0:64]
                          for i in range(CORES)], axis=1)
    kaT = np.concatenate([res1.results[i]["qko_a"][64:128]
                          for i in range(CORES)], axis=1)
    vaT = np.concatenate([res1.results[i]["vo_a"]
                          for i in range(CORES)], axis=1)
    qbT = [res1.results[i]["qko_b"][0:64] for i in range(CORES)]
    kbT = [res1.results[i]["qko_b"][64:128] for i in range(CORES)]
    vbT = [res1.results[i]["vo_b"] for i in range(CORES)]

    # a-side derived tensors (shared by all cores)
    vaT32 = vaT.astype(np.float32)
    va_nm = cc(vaT.T)                       # [B*N, INNER] fp16
    na = np.maximum(np.sqrt((vaT32 * vaT32).sum(0)), EPS)
    vhat_aT = vaT32 / na[None, :]
    vaL = np.zeros((N, (B // 2) * 128), np.float16)
    vaR = np.zeros((N, (B // 2) * 128), np.float16)
    for j in range(B // 2):
        vaL[:, 128 * j:128 * j + 64] = va_nm[N * 2 * j:N * (2 * j + 1)]
        vaR[:, 128 * j + 64:128 * (j + 1)] = va_nm[N * (2 * j + 1):
                                                   N * (2 * j + 2)]
    vhat_aT2 = np.zeros((128, B * N // 2), np.float32)
    for j2 in range(8):
        vhat_aT2[0:64, 400 * j2:400 * (j2 + 1)] = \
            vhat_aT[:, 800 * j2:800 * j2 + 400]
        vhat_aT2[64:128, 400 * j2:400 * (j2 + 1)] = \
            vhat_aT[:, 800 * j2 + 400:800 * (j2 + 1)]
    m1, m8 = _masters()

    kaTdr = _dr_pack_k_padded(kaT.astype(np.float32), B, N, MP)
    qaTdr = _dr_pack_k(qaT.astype(np.float32))
    in2 = []
    vhat_bTs = []
    for i in range(CORES):
        vbT32 = vbT[i].astype(np.float32)
        vb_nm = cc(vbT[i].T)                # [BN, INNER] fp16
        nb = np.maximum(np.sqrt((vbT32 * vbT32).sum(0)), EPS)
        vhat_bT = vbT32 / nb[None, :]
        vbL = np.zeros((N, PB * 128), np.float16)
        vbR = np.zeros((N, PB * 128), np.float16)
        for p in range(PB):
            vbL[:, 128 * p:128 * p + 64] = vb_nm[N * p:N * (p + 1)]
            vbR[:, 128 * p + 64:128 * (p + 1)] = vb_nm[N * p:N * (p + 1)]
        vhat_bTs.append(vhat_bT)
        in2.append(dict(
            kaTdr=kaTdr, qaTdr=qaTdr,
            qbTdr=_dr_pack_k(qbT[i].astype(np.float32)),
            kbTdr=_dr_pack_k_padded(kbT[i].astype(np.float32), PB, N, MP),
            vaL=vaL, vaR=vaR, vbL=vbL, vbR=vbR))
    res2 = run_bass_kernel_spmd(nc2, in2, core_ids=list(range(CORES)))

    sim = np.zeros((B, B), np.float32)
    for i in range(CORES):
        r = res2.results[i]
        # path1: As1 col-block 800j = pair j (rows 0:64 -> q=2j,
        # rows 64:128 -> q=2j+1, cols (p, n)); dot/ny2 on host
        as1 = np.asarray(r["as1o"], np.float32).reshape(128, 32, 800)
        vb_h = vhat_bTs[i]                              # [64 i, 800 (p n)]
        ny2_1 = np.empty((64, 800), np.float32)
        dot1 = np.empty((64, 800), np.float32)
        ny2_1[0::2] = (as1[0:64] ** 2).sum(0)
        ny2_1[1::2] = (as1[64:128] ** 2).sum(0)
        dot1[0::2] = np.einsum('ijc,ic->jc', as1[0:64], vb_h)
        dot1[1::2] = np.einsum('ijc,ic->jc', as1[64:128], vb_h)
        cos1 = dot1 / np.maximum(np.sqrt(ny2_1), EPS)
        sim1 = cos1.reshape(64, PB, N).sum(-1)          # [q, p]

        # path2: As2 cols 3200p + 800g + 400h + c; rows 0:64 ->
        # qn = 800*(2g+h)+c, rows 64:128 -> +400; vhat_a [64, (g,h,half,c)]
        as2 = np.asarray(r["as2o"], np.float32).reshape(128, PB, 4, 2, 400)
        va4 = vhat_aT.reshape(64, 4, 2, 2, 400)         # [i, g, h, half, c]
        ny_lo = (as2[0:64] ** 2).sum(0).reshape(PB, 8, 400)
        ny_hi = (as2[64:128] ** 2).sum(0).reshape(PB, 8, 400)
        ny2_2 = np.concatenate([ny_lo, ny_hi], axis=2).reshape(PB, B * N)
        d_lo = np.einsum('ipghc,ighc->pghc', as2[0:64], va4[:, :, :, 0])
        d_hi = np.einsum('ipghc,ighc->pghc', as2[64:128], va4[:, :, :, 1])
        dot2 = np.concatenate([d_lo.reshape(PB, 8, 400),
                               d_hi.reshape(PB, 8, 400)],
                              axis=2).reshape(PB, B * N)
        cos2 = dot2 / np.maximum(np.sqrt(ny2_2), EPS)
        sim2 = cos2.reshape(PB, B, N).sum(-1)           # [p, q]

        sim[PB * i:PB * (i + 1)] = (sim1.T + sim2) / N
    return sim



# revision 11
# speedup vs baseline: 1.4114x; 1.0208x over previous
"""Trainium2 Bass kernel for nn_AttentionSimilarity.

Contract: kernel(**inputs) takes the FULL unsharded inputs (numpy) and
returns the FULL [64, 64] similarity matrix, distributing work across 8
NeuronCores internally.

Structure:
  prog1 (projections, sharded by batch): each core projects its 8
    a-batches and 8 b-batches through the three two-layer MLPs,
    emitting qaT/kaT/vaT/qbT/kbT/vbT chunks in [inner, (batch, n)]
    layout. Host gathers the a-side to full tensors.
  prog2 (attention, sharded by p = b-side batch): each core computes
    both attention paths for its 8 p's against all 64 q's, the cosine
    numerators/denominators via selector matmuls on the PE, and the
    per-(p,q) sums over n. Host assembles the [64, 64] output.

Math notes:
  - softmax feeds only cosine similarity, which is scale-invariant in
    the aligned vector, so the softmax max-shift and denominator cancel:
    softmax reduces to exp(scores/8).
  - the x-side cosine norm is folded on the host (vhat = v / max(|v|, eps)).
  - 1/max(|y|, eps) and the dot with vhat are applied on the host from
    the streamed-out aligned values.

Performance notes (vs the first working version):
  - prog1 W1 layer and both programs' score matmuls run in fp8e4 with
    MatmulPerfMode.DoubleRow (2 contraction rows per PE partition, 0.5
    cycles/output column): weights/features/q/k are DR-packed on the
    host ([K/2, 2, M] with k = (K/2)*s + p; lhsT m-blocks padded to
    MP=112 so the DR pair-stride stays 16-byte aligned).
  - the entire cosine stage (dot, squared-norm, rsqrt, mean over n)
    is computed on the HOST: the aligned values (As, bf16) stream out
    over the otherwise-idle DMA engines, deleting the M/SQ multiplies,
    all selector-reduce matmuls, the P1/P2 PSUM accumulators (freeing
    banks for aligned double-buffering), and the device epilogues.
    The device does projections, scores, softmax-exp and the aligned
    matmuls -- all of the O(B^2 N^2) compute.
  - warmup/tail: weight DMAs are split/consolidated so the first matmul
    starts as early as possible; prog1's W2 PSUM/copy/DMA pipeline is
    chunked per bank so stores drain during compute; path2 score tiles
    are 1536 columns (3 PSUM banks) to amortize the fixed per-
    instruction ACT access latency on the softmax exp, which is the
    saturated engine (~98% busy) in the final balance.
  - measured rel err vs fp32 reference: ~1.7e-3.

Dead end (measured): packing score tiles to 128 partitions by mixing
(q, m) across rows would cut exp columns 100/128, but the follow-up
aligned matmuls need operand slices at arbitrary partition offsets and
the PE requires base partition 0/32/64 (bass matmul assert); since
100 is not a multiple of 32, per-q slices of a packed layout are
unaddressable. The [m<=100, cols] score layout is forced.
"""

import os
import sys

sys.path.insert(0, "/opt/trn_rl_repo")
os.environ.setdefault("NEURON_RT_RESET_CORES", "1")

import numpy as np
import ml_dtypes  # noqa: F401  (bf16 host arrays)

import bass_rust
import concourse.bass as bass
import concourse.mybir as mybir
import concourse.tile as tile
from concourse.bass_utils import run_bass_kernel_spmd

F32 = mybir.dt.float32
F32R = mybir.dt.float32r
BF16 = mybir.dt.bfloat16
F16 = mybir.dt.float16
F8E4 = mybir.dt.float8e4
AF = mybir.ActivationFunctionType
DR = mybir.MatmulPerfMode.DoubleRow

B = 64          # batches per side
C = 512         # channels
N = 100         # H*W tokens per batch
INNER = 64      # projected dim
CORES = 8
PB = B // CORES  # batches per core (8)
BN = PB * N      # 800: (batch, n) columns per core chunk
EPS = 1e-8
KT1 = C // 128   # prog1 contraction tiles (4)
MP = 112         # fp8-DR padded m stride (112 % 16 == 0, >= N)

E1_BUFS = int(os.environ.get("K_E1_BUFS", "5"))
SEL_LAG = int(os.environ.get("K_SEL_LAG", "4"))
POOL_MOD1 = int(os.environ.get("K_POOL_MOD1", os.environ.get("K_POOL_MOD", "3")))
POOL_MOD2 = int(os.environ.get("K_POOL_MOD2", os.environ.get("K_POOL_MOD", "2")))
SEL_LAG2 = int(os.environ.get("K_SEL_LAG2", "4"))
M2_BUFS = int(os.environ.get("K_M2_BUFS", "8"))
MPOOL_MOD = int(os.environ.get("K_MPOOL_MOD", "0"))  # 0=never, k=every kth M on pool
M_BUFS = int(os.environ.get("K_M_BUFS", "8"))
E2_BUFS = int(os.environ.get("K_E2_BUFS", "3"))
S1_BUFS = int(os.environ.get("K_S1_BUFS", "2"))
A1_BUFS = int(os.environ.get("K_A1_BUFS", "1"))

_waitsplit_ctr = [0]


def _split_multi_waits(nc, max_waits=1):
    """This container's walrus build accepts at most ONE sync wait per
    instruction; Tile attaches several. Move extras onto preceding
    same-engine NoOps (engines are in-order, so semantics hold)."""
    n_split = 0
    for f in nc.m.functions:
        for blk in f.blocks:
            insts = list(blk.instructions)
            new_list = []
            changed = False
            for inst in insts:
                si = inst.sync_info
                waits = list(si.on_wait) if (si is not None and si.on_wait) else []
                if len(waits) > max_waits:
                    for w in waits[:-max_waits]:
                        _waitsplit_ctr[0] += 1
                        nop = mybir.InstNoOp(
                            name=f"I-waitsplit-{_waitsplit_ctr[0]}",
                            engine=inst.engine,
                            ins=[],
                            outs=[],
                            sync_info=bass_rust.SyncInfo(on_wait=[w], on_update=[]),
                        )
                        nc.register_instruction(nop, overwrite=True)
                        new_list.append(nop)
                        n_split += 1
                    si.on_wait = waits[-max_waits:]
                    inst.sync_info = si
                    changed = True
                new_list.append(inst)
            if changed:
                blk.instructions = new_list
    return n_split


# ---------------------------------------------------------------- prog1

def build_prog1():
    """Projection program, K=256-per-pass DoubleRow everywhere.

    Per-core inputs (all fp8e4 DR-packed on the host):
      f8:    [128, 2*2*2*BN]   features; [p, (side, b, s, n)] holds
                               feat_side[cin = 256b + 128s + p, n]
      w1dr:  [128, 3*2*2*C]    [p, (proj, b, s, cout)] = W1[cin, cout]
      w2dr:  [128, 3*2*2*64]   [p, (proj, b2, s2, i)] = W2[cout, i]
                               (cout = 256*b2 + 128*s2 + p)
    Outputs (f16): qko_a/qko_b [128, BN] (q rows 0:64, k rows 64:128),
      vo_a/vo_b [64, BN].

    Hidden activations are stored fp8e4 so the W2 layer also runs
    DoubleRow (0.5 cyc/col); h layout [128, (b2, s2, n)] makes the DR
    rhs a plain strided view of the relu outputs.
    """
    nc = bass.Bass("TRN2", target_bir_lowering=False, debug=False,
                   num_devices=CORES)
    f8 = nc.dram_tensor("f8", [128, 8 * BN], F8E4, kind="ExternalInput").ap()
    w1d = nc.dram_tensor("w1dr", [128, 12 * C], F8E4,
                         kind="ExternalInput").ap()
    w2d = nc.dram_tensor("w2dr", [128, 12 * INNER], F8E4,
                         kind="ExternalInput").ap()
    outs = {"a": nc.dram_tensor("qko_a", [128, BN], F16,
                                kind="ExternalOutput").ap(),
            "b": nc.dram_tensor("qko_b", [128, BN], F16,
                                kind="ExternalOutput").ap()}
    vouts = {"a": nc.dram_tensor("vo_a", [INNER, BN], F16,
                                 kind="ExternalOutput").ap(),
             "b": nc.dram_tensor("vo_b", [INNER, BN], F16,
                                 kind="ExternalOutput").ap()}
    CH = [(0, 512), (512, BN)]  # psum-bank-aligned column chunks of BN

    with tile.TileContext(nc) as tc:
        with (
            tc.tile_pool(name="wpool", bufs=1) as wpool,
            tc.tile_pool(name="hpool", bufs=3) as hpool,
            tc.tile_pool(name="opool", bufs=4) as opool,
            tc.tile_pool(name="psH", bufs=3, space="PSUM") as psHp,
            tc.tile_pool(name="psO", bufs=1, space="PSUM") as psOp,
        ):
            # weights + features, hot-first.  w1sb view: [p, proj, b, s,
            # cout]; f view: [p, side, b, s, n]; w2sb: [p, proj, b2, s2, i].
            w1sb = wpool.tile([128, 12 * C], F8E4, tag="w1", name="w1sb")
            w1v = w1sb[:].rearrange("p (pr b s c) -> p pr b s c", pr=3, b=2,
                                    s=2)
            w1dv = w1d.rearrange("p (pr b s c) -> p pr b s c", pr=3, b=2, s=2)
            fsb = wpool.tile([128, 8 * BN], F8E4, tag="f", name="fsb")
            fv = fsb[:].rearrange("p (sd b s n) -> p sd b s n", sd=2, b=2,
                                  s=2)
            fdv = f8.rearrange("p (sd b s n) -> p sd b s n", sd=2, b=2, s=2)
            w2sb = wpool.tile([128, 12 * INNER], F8E4, tag="w2", name="w2sb")
            nc.sync.dma_start(w1v[:, 0, 0], w1dv[:, 0, 0])
            nc.sync.dma_start(fv[:, 0, 0], fdv[:, 0, 0])
            nc.sync.dma_start(w1v[:, 0, 1], w1dv[:, 0, 1])
            nc.sync.dma_start(fv[:, 0, 1], fdv[:, 0, 1])
            nc.sync.dma_start(w1v[:, 1:3], w1dv[:, 1:3])
            nc.sync.dma_start(w2sb[:], w2d[:])
            nc.sync.dma_start(fv[:, 1], fdv[:, 1])
            w2v = w2sb[:].rearrange("p (pr b s i) -> p pr b s i", pr=3, b=2,
                                    s=2)

            # relu engines, weighted round-robin (ACT/DVE faster than Pool)
            relu_cyc = [0]

            def relu(dst, src):
                e = (nc.scalar, nc.vector, nc.scalar, nc.vector,
                     nc.gpsimd)[relu_cyc[0] % 5]
                relu_cyc[0] += 1
                if e is nc.scalar:
                    e.activation(dst, src, AF.Relu)
                else:
                    e.tensor_scalar_max(dst, src, 0.0)

            hts = {}

            def w1(si, pi):
                ht = hpool.tile([128, 4 * BN], F8E4, tag="h",
                                name=f"h{si}{pi}")
                hv = ht[:].rearrange("p (b s n) -> p b s n", b=2, s=2)
                for t in range(4):
                    psH = psHp.tile([128, 1024], F32, tag="psH", name="psH")
                    for b in range(2):
                        for lo, hi in CH:
                            nc.tensor.matmul(
                                psH[:, lo:hi],
                                w1v[:, pi, b, :, 128 * t:128 * (t + 1)],
                                fv[:, si, b, :, lo:hi],
                                start=(b == 0), stop=(b == 1), perf_mode=DR)
                    relu(hv[:, t // 2, t % 2], psH[:, 0:BN])
                hts[(si, pi)] = hv

            def w2qk(si, s):
                """q rows 0:64 (DR; DR needs dst partition base 0) + k rows
                64:128 (plain fp8) of one psO tile, chunk-wise copy+DMA."""
                psO = psOp.tile([128, 1024], F32, tag="psO", name="psOqk")
                ot = opool.tile([128, BN], F16, tag="out", name="qkout")
                for lo, hi in CH:
                    for b2 in range(2):
                        nc.tensor.matmul(
                            psO[0:64, lo:hi], w2v[:, 0, b2],
                            hts[(si, 0)][:, b2, :, lo:hi],
                            start=(b2 == 0), stop=(b2 == 1), perf_mode=DR)
                    for b2 in range(2):
                        for s2 in range(2):
                            nc.tensor.matmul(
                                psO[64:128, lo:hi], w2v[:, 1, b2, s2],
                                hts[(si, 1)][:, b2, s2, lo:hi],
                                start=(b2 == 0 and s2 == 0),
                                stop=(b2 == 1 and s2 == 1))
                    nc.gpsimd.tensor_copy(ot[:][:, lo:hi], psO[:, lo:hi])
                    nc.sync.dma_start(outs[s][:, lo:hi], ot[:][:, lo:hi])

            def w2v_(si, s):
                psV = psOp.tile([128, 1024], F32, tag="psO", name="psOv")
                vt = opool.tile([INNER, BN], F16, tag="vout", name="vout")
                for lo, hi in CH:
                    for b2 in range(2):
                        nc.tensor.matmul(
                            psV[0:64, lo:hi], w2v[:, 2, b2],
                            hts[(si, 2)][:, b2, :, lo:hi],
                            start=(b2 == 0), stop=(b2 == 1), perf_mode=DR)
                    nc.scalar.copy(vt[:, lo:hi], psV[0:64, lo:hi])
                    nc.sync.dma_start(vouts[s][:, lo:hi], vt[:, lo:hi])

            # PE stream interleaved so W2v(a)'s psO WAR-wait on the qk copy
            # hides under W1q(b), and the relu pipeline never starves PE.
            w1(0, 0); w1(0, 1); w1(0, 2)
            w2qk(0, "a")
            w1(1, 0)
            w2v_(0, "a")
            w1(1, 1); w1(1, 2)
            w2qk(1, "b")
            w2v_(1, "b")

    _split_multi_waits(nc)
    return nc


# ---------------------------------------------------------------- prog2

def build_prog2():
    """Attention program, sharded over p (this core's 8 b-batches).

    Unified 64-stage software pipeline; every stage produces 1600 score
    columns in a [128, 2048] PSUM tile (4 banks, double-buffered = all 8
    banks), does ONE 1600-wide exp on ACT (the bottleneck engine), then
    reuses the exp-consumed banks of the same tile as the aligned-matmul
    accumulator (carve-after-read; subtile deps order the WAR hazard).
    Stage t+1's score matmuls are emitted before stage t's aligned
    matmuls so PE always has score work ready when ACT finishes an exp.

      path1 stage j (32): scores for q-pair (2j, 2j+1) over this core's
        800 (p, n) columns; q0 at S cols 0:800, q1 at 1024:1824; exp via
        a strided [100, 2, 800] AP; aligned A at cols 0:800.
      path2 stage (p, k) (32): scores for 1600 (q n) columns
        [1600k, 1600k+1600) against kb[p]; aligned A groups at cols
        0:400 and 512:912; strided copy out.

    Outputs (identical layout to the previous version; host unchanged):
      as1o [128, 32*800] bf16, as2o [128, 32*800] bf16
    """
    nc = bass.Bass("TRN2", target_bir_lowering=False, debug=False,
                   num_devices=CORES)
    din = {}
    for name, shape, dt in [
        ("kaTdr", [32, 2 * B * MP], F8E4), ("qaTdr", [32, 2 * B * N], F8E4),
        ("qbTdr", [32, 2 * BN], F8E4), ("kbTdr", [32, 2 * PB * MP], F8E4),
        ("vaL", [N, (B // 2) * 128], F16), ("vaR", [N, (B // 2) * 128], F16),
        ("vbL", [N, PB * 128], F16), ("vbR", [N, PB * 128], F16),
    ]:
        din[name] = nc.dram_tensor(name, shape, dt, kind="ExternalInput").ap()
    as1o = nc.dram_tensor("as1o", [128, 32 * BN], BF16,
                          kind="ExternalOutput").ap()
    as2o = nc.dram_tensor("as2o", [128, 32 * 800], BF16,
                          kind="ExternalOutput").ap()

    with tile.TileContext(nc) as tc:
        from contextlib import ExitStack
        with ExitStack() as ctx:
            inp = ctx.enter_context(tc.tile_pool(name="inp", bufs=1))
            sb = {}

            def load(name):
                ap = din[name]
                t = inp.tile(list(ap.shape), ap.dtype, tag=name,
                             name=f"sb_{name}")
                nc.sync.dma_start(t[:], ap[:])
                sb[name] = t

            # Input DMAs, hot-first. All on the SP (sync) queue, issued
            # before any output DMA so no wait ever blocks the SP SEQ.
            ka_t = inp.tile([32, 2 * B * MP], F8E4, tag="kaTdr",
                            name="sb_kaTdr")
            sb["kaTdr"] = ka_t
            ka3d = din["kaTdr"].rearrange("p (two q m) -> p two q m",
                                          two=2, q=B)
            ka3s = ka_t[:].rearrange("p (two q m) -> p two q m", two=2, q=B)
            load("qbTdr")
            nc.sync.dma_start(ka3s[:, :, 0:8, :], ka3d[:, :, 0:8, :])
            va_t = {}
            for nm in ("vaL", "vaR"):
                va_t[nm] = inp.tile([N, (B // 2) * 128], F16, tag=nm,
                                    name=f"sb_{nm}")
                nc.sync.dma_start(va_t[nm][:, 0:512], din[nm][:, 0:512])
            for nm in ("vaL", "vaR"):
                nc.sync.dma_start(va_t[nm][:, 512:2048], din[nm][:, 512:2048])
            nc.sync.dma_start(ka3s[:, :, 8:32, :], ka3d[:, :, 8:32, :])
            for nm in ("vaL", "vaR"):
                nc.sync.dma_start(va_t[nm][:, 2048:4096],
                                  din[nm][:, 2048:4096])
            nc.sync.dma_start(ka3s[:, :, 32:64, :], ka3d[:, :, 32:64, :])
            qa_t = inp.tile([32, 2 * B * N], F8E4, tag="qaTdr",
                            name="sb_qaTdr")
            sb["qaTdr"] = qa_t
            qa3d = din["qaTdr"].rearrange("p (two n) -> p two n", two=2)
            qa3s = qa_t[:].rearrange("p (two n) -> p two n", two=2)
            nc.sync.dma_start(qa3s[:, :, 0:3200], qa3d[:, :, 0:3200])
            nc.sync.dma_start(qa3s[:, :, 3200:6400], qa3d[:, :, 3200:6400])
            for name in ("kbTdr", "vbL", "vbR"):
                load(name)

            epool = ctx.enter_context(tc.tile_pool(name="epool", bufs=4))
            mpool = ctx.enter_context(tc.tile_pool(name="mpool", bufs=8))
            spool = ctx.enter_context(
                tc.tile_pool(name="spool", bufs=2, space="PSUM"))
            apool = ctx.enter_context(
                tc.tile_pool(name="apool", bufs=2, space="PSUM"))

            ka3 = sb["kaTdr"][:].rearrange("p (two q m) -> p two q m",
                                           two=2, q=B)
            qb3 = sb["qbTdr"][:].rearrange("p (two n) -> p two n", two=2)
            kb3 = sb["kbTdr"][:].rearrange("p (two b m) -> p two b m",
                                           two=2, b=PB)
            qa3 = sb["qaTdr"][:].rearrange("p (two n) -> p two n", two=2)

            # The whole attention is one score stream of 102,400 columns:
            #   cols [1600j + 800h, +800)          = path1 pair j, q = 2j+h
            #   cols [51200 + 6400p + o, ...)      = path2 batch p
            # chunked into CW-wide exp stages (3-bank PSUM S tiles).
            CW = 1536
            SL = 102400
            NT = (SL + CW - 1) // CW  # 67 chunks (last 1024)
            segs = []  # (base, length, lhsT, rhs3)
            for j in range(B // 2):
                for h in range(2):
                    segs.append((1600 * j + 800 * h, 800,
                                 ka3[:, :, 2 * j + h, 0:N], qb3))
            for p in range(PB):
                segs.append((51200 + 6400 * p, 6400, kb3[:, :, p, 0:N], qa3))

            etiles = {}  # chunk index -> E tile

            def eslices(a, b):
                """Stream range [a, b) as a list of E-tile slices."""
                out = []
                while a < b:
                    t = a // CW
                    e = min(b, (t + 1) * CW)
                    out.append(etiles[t][:][:, a - t * CW:e - t * CW])
                    a = e
                return out

            def emit_front(t):
                """Score matmuls + one exp for stream chunk t."""
                c0, c1 = CW * t, min(CW * (t + 1), SL)
                sa = spool.tile([100, CW], F32, tag="S", name=f"S{t % 2}")
                E = epool.tile([100, CW], F16, tag="E")
                for base, ln, lhsT, rhs3 in segs:
                    a, b = max(c0, base), min(c1, base + ln)
                    while a < b:  # split at this S tile's 512-col banks
                        e = min(b, c0 + ((a - c0) // 512 + 1) * 512)
                        nc.tensor.matmul(
                            sa[:][:, a - c0:e - c0], lhsT,
                            rhs3[:, :, a - base:e - base],
                            start=True, stop=True, perf_mode=DR)
                        a = e
                nc.scalar.activation(E[:][:, 0:c1 - c0], sa[:][:, 0:c1 - c0],
                                     AF.Exp, scale=0.125)
                etiles[t] = E

            def emit_aligned(At, dcols, pairs):
                """At[:, d] = sum_i lhsT_i.T @ E[stream a_i + d] for
                d in [0, dcols). Dest is split at every E-chunk boundary of
                either source range so each dest interval is a complete
                start/stop accumulation group."""
                cuts = {0, dcols}
                for _, a in pairs:
                    c = (a // CW + 1) * CW
                    while c < a + dcols:
                        cuts.add(c - a)
                        c += CW
                cs = sorted(cuts)
                for d0, d1 in zip(cs, cs[1:]):
                    for i, (lhsT, a) in enumerate(pairs):
                        (sl,) = eslices(a + d0, a + d1)
                        nc.tensor.matmul(At[:][:, d0:d1], lhsT, sl,
                                         start=(i == 0),
                                         stop=(i == len(pairs) - 1),
                                         skip_group_check=True)

            as2_live = {}

            def emit_back(g):
                """Aligned matmuls + copy (+DMA) for finished group g."""
                if g < B // 2:  # path1 pair j
                    j = g
                    vaLs = va_t["vaL"][:, 128 * j:128 * (j + 1)]
                    vaRs = va_t["vaR"][:, 128 * j:128 * (j + 1)]
                    As = mpool.tile([128, 800], BF16, tag="As")
                    for lo, w in ((0, 512), (512, 288)):
                        At = apool.tile([128, 512], F32, tag="A")
                        emit_aligned(At, w,
                                     [(vaLs, 1600 * j + lo),
                                      (vaRs, 1600 * j + 800 + lo)])
                        nc.vector.tensor_copy(As[:][:, lo:lo + w],
                                              At[:][:, 0:w])
                    nc.sync.dma_start(as1o[:, BN * j:BN * (j + 1)], As[:])
                else:  # path2 800-col group
                    gg = g - B // 2
                    p, o8 = gg // 8, (gg % 8) * 800
                    base = 51200 + 6400 * p + o8
                    vbLs = sb["vbL"][:, 128 * p:128 * (p + 1)]
                    vbRs = sb["vbR"][:, 128 * p:128 * (p + 1)]
                    At = apool.tile([128, 512], F32, tag="A")
                    emit_aligned(At, 400, [(vbLs, base), (vbRs, base + 400)])
                    u = (gg % 8) // 2
                    if gg % 2 == 0:
                        as2_live[p] = mpool.tile([128, 800], BF16, tag="As", name="As2")
                    As2 = as2_live[p]
                    nc.vector.tensor_copy(
                        As2[:][:, 400 * (gg % 2):400 * (gg % 2) + 400],
                        At[:][:, 0:400])
                    if gg % 2 == 1:
                        nc.sync.dma_start(
                            as2o[:, 3200 * p + 800 * u:
                                 3200 * p + 800 * (u + 1)], As2[:])

            # group g ready once its last stream column's chunk is emitted
            ends = [1600 * (j + 1) for j in range(B // 2)] + \
                   [51200 + 6400 * (gg // 8) + 800 * (gg % 8) + 800
                    for gg in range(64)]
            ready = [(e + CW - 1) // CW - 1 for e in ends]
            for t in range(NT + 1):
                if t < NT:
                    emit_front(t)
                for g in range(len(ends)):
                    if ready[g] == t - 1:
                        emit_back(g)

    _split_multi_waits(nc)
    return nc


# ---------------------------------------------------------------- host

_progs = {}


def _install_compile_cache():
    """Persist compiled NEFF-wrapped custom calls across processes: walrus
    compilation takes tens of seconds per program and bass2jax recompiles
    in every fresh process otherwise."""
    import hashlib
    import pathlib
    from concourse import bass2jax
    if getattr(bass2jax, "_ant_disk_cache", False):
        return
    bass2jax._ant_disk_cache = True
    orig = bass2jax.neuronx_cc_hook
    cdir = pathlib.Path(os.environ.get("BASS_NEFF_CACHE",
                                       "/tmp/bass_neff_cache"))
    try:
        cdir.mkdir(parents=True, exist_ok=True)
    except OSError:
        return

    def cached_hook(code, code_format, platform_version, file_prefix):
        try:
            key = hashlib.sha256(
                bytes(code) + b"|" + bytes(code_format)).hexdigest()
            path = cdir / f"{key}.neffcall"
            if path.exists():
                return 0, path.read_bytes()
        except Exception:
            return orig(code, code_format, platform_version, file_prefix)
        rc, blob = orig(code, code_format, platform_version, file_prefix)
        if rc == 0:
            try:
                tmp = path.with_suffix(f".tmp{os.getpid()}")
                tmp.write_bytes(blob)
                tmp.rename(path)
            except OSError:
                pass
        return rc, blob

    bass2jax.neuronx_cc_hook = cached_hook
    try:
        import libneuronxla
        if libneuronxla.neuronx_cc is orig:
            libneuronxla.neuronx_cc = cached_hook
    except ImportError:
        pass


def _get_progs():
    if "p1" not in _progs:
        _install_compile_cache()
        _progs["p1"] = build_prog1()
        _progs["p2"] = build_prog2()
    return _progs["p1"], _progs["p2"]


def _masters():
    import ml_dtypes
    m1 = np.zeros((128, 320), ml_dtypes.bfloat16)
    m1[0:64, 128] = 1.0   # up-plane (rows 0:64 of rhs) -> out row q
    m1[64:128, 129] = 1.0  # down-plane -> out row q+1
    m8 = np.zeros((128, 320), ml_dtypes.bfloat16)
    m8[0:64, 128] = 1.0
    m8[64:128, 136] = 1.0  # down-plane -> out row r0+8
    return m1, m8


def _dr_pack_k(x, pad_to=None):
    """Pack [K, M] (K contraction, even) into DoubleRow layout
    [K//2, 2*M] fp8e4 with k = (K//2)*s + p."""
    import ml_dtypes
    K = x.shape[0]
    h = K // 2
    arr = x.reshape(2, h, *x.shape[1:]).transpose(1, 0, *range(2, x.ndim + 1))
    return np.ascontiguousarray(arr.reshape(h, -1).astype(
        ml_dtypes.float8_e4m3fn))


def _dr_pack_k_padded(x, nblk, blk, pad):
    """[K, nblk*blk] -> DR fp8 [K//2, 2*nblk*pad] with each blk padded."""
    import ml_dtypes
    K = x.shape[0]
    h = K // 2
    a = x.reshape(2, h, nblk, blk).transpose(1, 0, 2, 3)
    z = np.zeros((h, 2, nblk, pad), np.float32)
    z[:, :, :, 0:blk] = a
    return np.ascontiguousarray(z.reshape(h, -1).astype(
        ml_dtypes.float8_e4m3fn))


def kernel(features_a, features_b, Wq1, Wq2, Wk1, Wk2, Wv1, Wv2):
    import ml_dtypes
    nc1, nc2 = _get_progs()
    cc = np.ascontiguousarray
    FP8 = ml_dtypes.float8_e4m3fn

    fa = np.asarray(features_a, np.float32).reshape(B, C, N)
    fb = np.asarray(features_b, np.float32).reshape(B, C, N)

    def feat8(fa_core, fb_core):  # 2x [PB, C, N] -> [128, 8*BN] fp8
        # [sd, b, s, p, n] with cin = 256b + 128s + p -> [p, sd, b, s, n]
        fT = np.stack([fc.transpose(1, 0, 2).reshape(C, BN)
                       for fc in (fa_core, fb_core)])
        a = fT.reshape(2, 2, 2, 128, BN).transpose(3, 0, 1, 2, 4)
        return cc(a.reshape(128, 8 * BN).astype(FP8))

    def wpack(Ws):  # list of [C, M] -> [128, 3*2*2*M] fp8
        a = np.stack([np.asarray(W, np.float32) for W in Ws])
        M = a.shape[-1]
        a = a.reshape(3, 2, 2, 128, M).transpose(3, 0, 1, 2, 4)
        return cc(a.reshape(128, 12 * M).astype(FP8))

    ws = {"w1dr": wpack([Wq1, Wk1, Wv1]), "w2dr": wpack([Wq2, Wk2, Wv2])}

    in1 = [dict(f8=feat8(fa[PB * i:PB * (i + 1)], fb[PB * i:PB * (i + 1)]),
                **ws)
           for i in range(CORES)]
    res1 = run_bass_kernel_spmd(nc1, in1, core_ids=list(range(CORES)))

    qaT = np.concatenate([res1.results[i]["qko_a"][0:64]
                          for i in range(CORES)], axis=1)
    kaT = np.concatenate([res1.results[i]["qko_a"][64:128]
                          for i in range(CORES)], axis=1)
    vaT = np.concatenate([res1.results[i]["vo_a"]
                          for i in range(CORES)], axis=1)
    qbT = [res1.results[i]["qko_b"][0:64] for i in range(CORES)]
    kbT = [res1.results[i]["qko_b"][64:128] for i in range(CORES)]
    vbT = [res1.results[i]["vo_b"] for i in range(CORES)]

    # a-side derived tensors (shared by all cores)
    vaT32 = vaT.astype(np.float32)
    va_nm = cc(vaT.T)                       # [B*N, INNER] fp16
    na = np.maximum(np.sqrt((vaT32 * vaT32).sum(0)), EPS)
    vhat_aT = vaT32 / na[None, :]
    vaL = np.zeros((N, (B // 2) * 128), np.float16)
    vaR = np.zeros((N, (B // 2) * 128), np.float16)
    for j in range(B // 2):
        vaL[:, 128 * j:128 * j + 64] = va_nm[N * 2 * j:N * (2 * j + 1)]
        vaR[:, 128 * j + 64:128 * (j + 1)] = va_nm[N * (2 * j + 1):
                                                   N * (2 * j + 2)]
    vhat_aT2 = np.zeros((128, B * N // 2), np.float32)
    for j2 in range(8):
        vhat_aT2[0:64, 400 * j2:400 * (j2 + 1)] = \
            vhat_aT[:, 800 * j2:800 * j2 + 400]
        vhat_aT2[64:128, 400 * j2:400 * (j2 + 1)] = \
            vhat_aT[:, 800 * j2 + 400:800 * (j2 + 1)]
    m1, m8 = _masters()

    kaTdr = _dr_pack_k_padded(kaT.astype(np.float32), B, N, MP)
    qaTdr = _dr_pack_k(qaT.astype(np.float32))
    in2 = []
    vhat_bTs = []
    for i in range(CORES):
        vbT32 = vbT[i].astype(np.float32)
        vb_nm = cc(vbT[i].T)                # [BN, INNER] fp16
        nb = np.maximum(np.sqrt((vbT32 * vbT32).sum(0)), EPS)
        vhat_bT = vbT32 / nb[None, :]
        vbL = np.zeros((N, PB * 128), np.float16)
        vbR = np.zeros((N, PB * 128), np.float16)
        for p in range(PB):
            vbL[:, 128 * p:128 * p + 64] = vb_nm[N * p:N * (p + 1)]
            vbR[:, 128 * p + 64:128 * (p + 1)] = vb_nm[N * p:N * (p + 1)]
        vhat_bTs.append(vhat_bT)
        in2.append(dict(
            kaTdr=kaTdr, qaTdr=qaTdr,
            qbTdr=_dr_pack_k(qbT[i].astype(np.float32)),
            kbTdr=_dr_pack_k_padded(kbT[i].astype(np.float32), PB, N, MP),
            vaL=vaL, vaR=vaR, vbL=vbL, vbR=vbR))
    res2 = run_bass_kernel_spmd(nc2, in2, core_ids=list(range(CORES)))

    sim = np.zeros((B, B), np.float32)
    for i in range(CORES):
        r = res2.results[i]
        # path1: As1 col-block 800j = pair j (rows 0:64 -> q=2j,
        # rows 64:128 -> q=2j+1, cols (p, n)); dot/ny2 on host
        as1 = np.asarray(r["as1o"], np.float32).reshape(128, 32, 800)
        vb_h = vhat_bTs[i]                              # [64 i, 800 (p n)]
        ny2_1 = np.empty((64, 800), np.float32)
        dot1 = np.empty((64, 800), np.float32)
        ny2_1[0::2] = (as1[0:64] ** 2).sum(0)
        ny2_1[1::2] = (as1[64:128] ** 2).sum(0)
        dot1[0::2] = np.einsum('ijc,ic->jc', as1[0:64], vb_h)
        dot1[1::2] = np.einsum('ijc,ic->jc', as1[64:128], vb_h)
        cos1 = dot1 / np.maximum(np.sqrt(ny2_1), EPS)
        sim1 = cos1.reshape(64, PB, N).sum(-1)          # [q, p]

        # path2: As2 cols 3200p + 800g + 400h + c; rows 0:64 ->
        # qn = 800*(2g+h)+c, rows 64:128 -> +400; vhat_a [64, (g,h,half,c)]
        as2 = np.asarray(r["as2o"], np.float32).reshape(128, PB, 4, 2, 400)
        va4 = vhat_aT.reshape(64, 4, 2, 2, 400)         # [i, g, h, half, c]
        ny_lo = (as2[0:64] ** 2).sum(0).reshape(PB, 8, 400)
        ny_hi = (as2[64:128] ** 2).sum(0).reshape(PB, 8, 400)
        ny2_2 = np.concatenate([ny_lo, ny_hi], axis=2).reshape(PB, B * N)
        d_lo = np.einsum('ipghc,ighc->pghc', as2[0:64], va4[:, :, :, 0])
        d_hi = np.einsum('ipghc,ighc->pghc', as2[64:128], va4[:, :, :, 1])
        dot2 = np.concatenate([d_lo.reshape(PB, 8, 400),
                               d_hi.reshape(PB, 8, 400)],
                              axis=2).reshape(PB, B * N)
        cos2 = dot2 / np.maximum(np.sqrt(ny2_2), EPS)
        sim2 = cos2.reshape(PB, B, N).sum(-1)           # [p, q]

        sim[PB * i:PB * (i + 1)] = (sim1.T + sim2) / N
    return sim



# revision 12
# speedup vs baseline: 1.4356x; 1.0172x over previous
"""Trainium2 Bass kernel for nn_AttentionSimilarity.

Contract: kernel(**inputs) takes the FULL unsharded inputs (numpy) and
returns the FULL [64, 64] similarity matrix, distributing work across 8
NeuronCores internally.

Structure:
  prog1 (projections, sharded by batch): each core projects its 8
    a-batches and 8 b-batches through the three two-layer MLPs,
    emitting qaT/kaT/vaT/qbT/kbT/vbT chunks in [inner, (batch, n)]
    layout. Host gathers the a-side to full tensors.
  prog2 (attention, sharded by p = b-side batch): each core computes
    both attention paths for its 8 p's against all 64 q's, the cosine
    numerators/denominators via selector matmuls on the PE, and the
    per-(p,q) sums over n. Host assembles the [64, 64] output.

Math notes:
  - softmax feeds only cosine similarity, which is scale-invariant in
    the aligned vector, so the softmax max-shift and denominator cancel:
    softmax reduces to exp(scores/8).
  - the x-side cosine norm is folded on the host (vhat = v / max(|v|, eps)).
  - 1/max(|y|, eps) and the dot with vhat are applied on the host from
    the streamed-out aligned values.

Performance notes (vs the first working version):
  - prog1 W1 layer and both programs' score matmuls run in fp8e4 with
    MatmulPerfMode.DoubleRow (2 contraction rows per PE partition, 0.5
    cycles/output column): weights/features/q/k are DR-packed on the
    host ([K/2, 2, M] with k = (K/2)*s + p; lhsT m-blocks padded to
    MP=112 so the DR pair-stride stays 16-byte aligned).
  - the entire cosine stage (dot, squared-norm, rsqrt, mean over n)
    is computed on the HOST: the aligned values (As, bf16) stream out
    over the otherwise-idle DMA engines, deleting the M/SQ multiplies,
    all selector-reduce matmuls, the P1/P2 PSUM accumulators (freeing
    banks for aligned double-buffering), and the device epilogues.
    The device does projections, scores, softmax-exp and the aligned
    matmuls -- all of the O(B^2 N^2) compute.
  - warmup/tail: weight DMAs are split/consolidated so the first matmul
    starts as early as possible; prog1's W2 PSUM/copy/DMA pipeline is
    chunked per bank so stores drain during compute; path2 score tiles
    are 1536 columns (3 PSUM banks) to amortize the fixed per-
    instruction ACT access latency on the softmax exp, which is the
    saturated engine (~98% busy) in the final balance.
  - measured rel err vs fp32 reference: ~1.7e-3.

Dead end (measured): packing score tiles to 128 partitions by mixing
(q, m) across rows would cut exp columns 100/128, but the follow-up
aligned matmuls need operand slices at arbitrary partition offsets and
the PE requires base partition 0/32/64 (bass matmul assert); since
100 is not a multiple of 32, per-q slices of a packed layout are
unaddressable. The [m<=100, cols] score layout is forced.
"""

import os
import sys

sys.path.insert(0, "/opt/trn_rl_repo")
os.environ.setdefault("NEURON_RT_RESET_CORES", "1")

import numpy as np
import ml_dtypes  # noqa: F401  (bf16 host arrays)

import bass_rust
import concourse.bass as bass
import concourse.mybir as mybir
import concourse.tile as tile
from concourse.bass_utils import run_bass_kernel_spmd

F32 = mybir.dt.float32
F32R = mybir.dt.float32r
BF16 = mybir.dt.bfloat16
F16 = mybir.dt.float16
F8E4 = mybir.dt.float8e4
AF = mybir.ActivationFunctionType
DR = mybir.MatmulPerfMode.DoubleRow

B = 64          # batches per side
C = 512         # channels
N = 100         # H*W tokens per batch
INNER = 64      # projected dim
CORES = 8
PB = B // CORES  # batches per core (8)
BN = PB * N      # 800: (batch, n) columns per core chunk
EPS = 1e-8
KT1 = C // 128   # prog1 contraction tiles (4)
MP = 112         # fp8-DR padded m stride (112 % 16 == 0, >= N)

E1_BUFS = int(os.environ.get("K_E1_BUFS", "5"))
SEL_LAG = int(os.environ.get("K_SEL_LAG", "4"))
POOL_MOD1 = int(os.environ.get("K_POOL_MOD1", os.environ.get("K_POOL_MOD", "3")))
POOL_MOD2 = int(os.environ.get("K_POOL_MOD2", os.environ.get("K_POOL_MOD", "2")))
SEL_LAG2 = int(os.environ.get("K_SEL_LAG2", "4"))
M2_BUFS = int(os.environ.get("K_M2_BUFS", "8"))
MPOOL_MOD = int(os.environ.get("K_MPOOL_MOD", "0"))  # 0=never, k=every kth M on pool
M_BUFS = int(os.environ.get("K_M_BUFS", "8"))
E2_BUFS = int(os.environ.get("K_E2_BUFS", "3"))
S1_BUFS = int(os.environ.get("K_S1_BUFS", "2"))
A1_BUFS = int(os.environ.get("K_A1_BUFS", "1"))

_waitsplit_ctr = [0]


def _split_multi_waits(nc, max_waits=1):
    """This container's walrus build accepts at most ONE sync wait per
    instruction; Tile attaches several. Move extras onto preceding
    same-engine NoOps (engines are in-order, so semantics hold)."""
    n_split = 0
    for f in nc.m.functions:
        for blk in f.blocks:
            insts = list(blk.instructions)
            new_list = []
            changed = False
            for inst in insts:
                si = inst.sync_info
                waits = list(si.on_wait) if (si is not None and si.on_wait) else []
                if len(waits) > max_waits:
                    for w in waits[:-max_waits]:
                        _waitsplit_ctr[0] += 1
                        nop = mybir.InstNoOp(
                            name=f"I-waitsplit-{_waitsplit_ctr[0]}",
                            engine=inst.engine,
                            ins=[],
                            outs=[],
                            sync_info=bass_rust.SyncInfo(on_wait=[w], on_update=[]),
                        )
                        nc.register_instruction(nop, overwrite=True)
                        new_list.append(nop)
                        n_split += 1
                    si.on_wait = waits[-max_waits:]
                    inst.sync_info = si
                    changed = True
                new_list.append(inst)
            if changed:
                blk.instructions = new_list
    return n_split


# ---------------------------------------------------------------- prog1

def build_prog1():
    """Projection program, K=256-per-pass DoubleRow everywhere.

    Per-core inputs (all fp8e4 DR-packed on the host):
      f8:    [128, 2*2*2*BN]   features; [p, (side, b, s, n)] holds
                               feat_side[cin = 256b + 128s + p, n]
      w1dr:  [128, 3*2*2*C]    [p, (proj, b, s, cout)] = W1[cin, cout]
      w2dr:  [128, 3*2*2*64]   [p, (proj, b2, s2, i)] = W2[cout, i]
                               (cout = 256*b2 + 128*s2 + p)
    Outputs (f16): qko_a/qko_b [128, BN] (q rows 0:64, k rows 64:128),
      vo_a/vo_b [64, BN].

    Hidden activations are stored fp8e4 so the W2 layer also runs
    DoubleRow (0.5 cyc/col); h layout [128, (b2, s2, n)] makes the DR
    rhs a plain strided view of the relu outputs.
    """
    nc = bass.Bass("TRN2", target_bir_lowering=False, debug=False,
                   num_devices=CORES)
    f8 = nc.dram_tensor("f8", [128, 8 * BN], F8E4, kind="ExternalInput").ap()
    w1d = nc.dram_tensor("w1dr", [128, 12 * C], F8E4,
                         kind="ExternalInput").ap()
    w2d = nc.dram_tensor("w2dr", [128, 12 * INNER], F8E4,
                         kind="ExternalInput").ap()
    outs = {"a": nc.dram_tensor("qko_a", [128, BN], F16,
                                kind="ExternalOutput").ap(),
            "b": nc.dram_tensor("qko_b", [128, BN], F16,
                                kind="ExternalOutput").ap()}
    vouts = {"a": nc.dram_tensor("vo_a", [INNER, BN], F16,
                                 kind="ExternalOutput").ap(),
             "b": nc.dram_tensor("vo_b", [INNER, BN], F16,
                                 kind="ExternalOutput").ap()}
    CH = [(0, 512), (512, BN)]  # psum-bank-aligned column chunks of BN

    with tile.TileContext(nc) as tc:
        with (
            tc.tile_pool(name="wpool", bufs=1) as wpool,
            tc.tile_pool(name="hpool", bufs=3) as hpool,
            tc.tile_pool(name="opool", bufs=4) as opool,
            tc.tile_pool(name="psH", bufs=4, space="PSUM") as psHp,
        ):
            # weights + features, hot-first.  w1sb view: [p, proj, b, s,
            # cout]; f view: [p, side, b, s, n]; w2sb: [p, proj, b2, s2, i].
            w1sb = wpool.tile([128, 12 * C], F8E4, tag="w1", name="w1sb")
            w1v = w1sb[:].rearrange("p (pr b s c) -> p pr b s c", pr=3, b=2,
                                    s=2)
            w1dv = w1d.rearrange("p (pr b s c) -> p pr b s c", pr=3, b=2, s=2)
            fsb = wpool.tile([128, 8 * BN], F8E4, tag="f", name="fsb")
            fv = fsb[:].rearrange("p (sd b s n) -> p sd b s n", sd=2, b=2,
                                  s=2)
            fdv = f8.rearrange("p (sd b s n) -> p sd b s n", sd=2, b=2, s=2)
            w2sb = wpool.tile([128, 12 * INNER], F8E4, tag="w2", name="w2sb")
            nc.sync.dma_start(w1v[:, 0, 0], w1dv[:, 0, 0])
            nc.sync.dma_start(fv[:, 0, 0], fdv[:, 0, 0])
            nc.sync.dma_start(w1v[:, 0, 1], w1dv[:, 0, 1])
            nc.sync.dma_start(fv[:, 0, 1], fdv[:, 0, 1])
            nc.sync.dma_start(w1v[:, 1:3], w1dv[:, 1:3])
            nc.sync.dma_start(w2sb[:], w2d[:])
            nc.sync.dma_start(fv[:, 1], fdv[:, 1])
            w2v = w2sb[:].rearrange("p (pr b s i) -> p pr b s i", pr=3, b=2,
                                    s=2)

            # relu engines, weighted round-robin (ACT/DVE faster than Pool)
            relu_cyc = [0]

            def relu(dst, src):
                e = (nc.scalar, nc.vector, nc.scalar, nc.vector,
                     nc.gpsimd)[relu_cyc[0] % 5]
                relu_cyc[0] += 1
                if e is nc.scalar:
                    e.activation(dst, src, AF.Relu)
                else:
                    e.tensor_scalar_max(dst, src, 0.0)

            hts = {}

            def w1(si, pi):
                ht = hpool.tile([128, 4 * BN], F8E4, tag="h",
                                name=f"h{si}{pi}")
                hv = ht[:].rearrange("p (b s n) -> p b s n", b=2, s=2)
                for t in range(4):
                    psH = psHp.tile([128, 1024], F32, tag="psH", name="psH")
                    for b in range(2):
                        for lo, hi in CH:
                            nc.tensor.matmul(
                                psH[:, lo:hi],
                                w1v[:, pi, b, :, 128 * t:128 * (t + 1)],
                                fv[:, si, b, :, lo:hi],
                                start=(b == 0), stop=(b == 1), perf_mode=DR)
                    relu(hv[:, t // 2, t % 2], psH[:, 0:BN])
                hts[(si, pi)] = hv

            def w2qk(si, s):
                """q rows 0:64 (DR; DR needs dst partition base 0) + k rows
                64:128 (plain fp8) of one psO tile, chunk-wise copy+DMA."""
                psO = psHp.tile([128, 1024], F32, tag="psH", name="psOqk")
                ot = opool.tile([128, BN], F16, tag="out", name="qkout")
                for lo, hi in CH:
                    for b2 in range(2):
                        nc.tensor.matmul(
                            psO[0:64, lo:hi], w2v[:, 0, b2],
                            hts[(si, 0)][:, b2, :, lo:hi],
                            start=(b2 == 0), stop=(b2 == 1), perf_mode=DR)
                    for b2 in range(2):
                        for s2 in range(2):
                            nc.tensor.matmul(
                                psO[64:128, lo:hi], w2v[:, 1, b2, s2],
                                hts[(si, 1)][:, b2, s2, lo:hi],
                                start=(b2 == 0 and s2 == 0),
                                stop=(b2 == 1 and s2 == 1))
                    nc.gpsimd.tensor_copy(ot[:][:, lo:hi], psO[:, lo:hi])
                    nc.sync.dma_start(outs[s][:, lo:hi], ot[:][:, lo:hi])

            def w2v_(si, s):
                psV = psHp.tile([128, 1024], F32, tag="psH", name="psOv")
                vt = opool.tile([INNER, BN], F16, tag="vout", name="vout")
                for lo, hi in CH:
                    for b2 in range(2):
                        nc.tensor.matmul(
                            psV[0:64, lo:hi], w2v[:, 2, b2],
                            hts[(si, 2)][:, b2, :, lo:hi],
                            start=(b2 == 0), stop=(b2 == 1), perf_mode=DR)
                    nc.scalar.copy(vt[:, lo:hi], psV[0:64, lo:hi])
                    nc.sync.dma_start(vouts[s][:, lo:hi], vt[:, lo:hi])

            # PE stream interleaved so W2v(a)'s psO WAR-wait on the qk copy
            # hides under W1q(b), and the relu pipeline never starves PE.
            w1(0, 0); w1(0, 1); w1(0, 2)
            w2qk(0, "a")
            w1(1, 0)
            w2v_(0, "a")
            w1(1, 1); w1(1, 2)
            w2qk(1, "b")
            w2v_(1, "b")

    _split_multi_waits(nc)
    return nc


# ---------------------------------------------------------------- prog2

def build_prog2():
    """Attention program, sharded over p (this core's 8 b-batches).

    Unified 64-stage software pipeline; every stage produces 1600 score
    columns in a [128, 2048] PSUM tile (4 banks, double-buffered = all 8
    banks), does ONE 1600-wide exp on ACT (the bottleneck engine), then
    reuses the exp-consumed banks of the same tile as the aligned-matmul
    accumulator (carve-after-read; subtile deps order the WAR hazard).
    Stage t+1's score matmuls are emitted before stage t's aligned
    matmuls so PE always has score work ready when ACT finishes an exp.

      path1 stage j (32): scores for q-pair (2j, 2j+1) over this core's
        800 (p, n) columns; q0 at S cols 0:800, q1 at 1024:1824; exp via
        a strided [100, 2, 800] AP; aligned A at cols 0:800.
      path2 stage (p, k) (32): scores for 1600 (q n) columns
        [1600k, 1600k+1600) against kb[p]; aligned A groups at cols
        0:400 and 512:912; strided copy out.

    Outputs (identical layout to the previous version; host unchanged):
      as1o [128, 32*800] bf16, as2o [128, 32*800] bf16
    """
    nc = bass.Bass("TRN2", target_bir_lowering=False, debug=False,
                   num_devices=CORES)
    din = {}
    for name, shape, dt in [
        ("kaTdr", [32, 2 * B * MP], F8E4), ("qaTdr", [32, 2 * B * N], F8E4),
        ("qbTdr", [32, 2 * BN], F8E4), ("kbTdr", [32, 2 * PB * MP], F8E4),
        ("vaL", [N, (B // 2) * 128], F16), ("vaR", [N, (B // 2) * 128], F16),
        ("vbL", [N, PB * 128], F16), ("vbR", [N, PB * 128], F16),
    ]:
        din[name] = nc.dram_tensor(name, shape, dt, kind="ExternalInput").ap()
    as1o = nc.dram_tensor("as1o", [128, 32 * BN], BF16,
                          kind="ExternalOutput").ap()
    as2o = nc.dram_tensor("as2o", [128, 32 * 800], BF16,
                          kind="ExternalOutput").ap()

    with tile.TileContext(nc) as tc:
        from contextlib import ExitStack
        with ExitStack() as ctx:
            inp = ctx.enter_context(tc.tile_pool(name="inp", bufs=1))
            sb = {}

            def load(name):
                ap = din[name]
                t = inp.tile(list(ap.shape), ap.dtype, tag=name,
                             name=f"sb_{name}")
                nc.sync.dma_start(t[:], ap[:])
                sb[name] = t

            # Input DMAs, hot-first. All on the SP (sync) queue, issued
            # before any output DMA so no wait ever blocks the SP SEQ.
            ka_t = inp.tile([32, 2 * B * MP], F8E4, tag="kaTdr",
                            name="sb_kaTdr")
            sb["kaTdr"] = ka_t
            ka3d = din["kaTdr"].rearrange("p (two q m) -> p two q m",
                                          two=2, q=B)
            ka3s = ka_t[:].rearrange("p (two q m) -> p two q m", two=2, q=B)
            load("qbTdr")
            nc.sync.dma_start(ka3s[:, :, 0:8, :], ka3d[:, :, 0:8, :])
            va_t = {}
            for nm in ("vaL", "vaR"):
                va_t[nm] = inp.tile([N, (B // 2) * 128], F16, tag=nm,
                                    name=f"sb_{nm}")
                nc.sync.dma_start(va_t[nm][:, 0:512], din[nm][:, 0:512])
            for nm in ("vaL", "vaR"):
                nc.sync.dma_start(va_t[nm][:, 512:2048], din[nm][:, 512:2048])
            nc.sync.dma_start(ka3s[:, :, 8:32, :], ka3d[:, :, 8:32, :])
            for nm in ("vaL", "vaR"):
                nc.sync.dma_start(va_t[nm][:, 2048:4096],
                                  din[nm][:, 2048:4096])
            nc.sync.dma_start(ka3s[:, :, 32:64, :], ka3d[:, :, 32:64, :])
            qa_t = inp.tile([32, 2 * B * N], F8E4, tag="qaTdr",
                            name="sb_qaTdr")
            sb["qaTdr"] = qa_t
            qa3d = din["qaTdr"].rearrange("p (two n) -> p two n", two=2)
            qa3s = qa_t[:].rearrange("p (two n) -> p two n", two=2)
            nc.sync.dma_start(qa3s[:, :, 0:3200], qa3d[:, :, 0:3200])
            nc.sync.dma_start(qa3s[:, :, 3200:6400], qa3d[:, :, 3200:6400])
            for name in ("kbTdr", "vbL", "vbR"):
                load(name)

            epool = ctx.enter_context(tc.tile_pool(name="epool", bufs=4))
            mpool = ctx.enter_context(tc.tile_pool(name="mpool", bufs=8))
            spool = ctx.enter_context(
                tc.tile_pool(name="spool", bufs=2, space="PSUM"))
            apool = ctx.enter_context(
                tc.tile_pool(name="apool", bufs=2, space="PSUM"))

            ka3 = sb["kaTdr"][:].rearrange("p (two q m) -> p two q m",
                                           two=2, q=B)
            qb3 = sb["qbTdr"][:].rearrange("p (two n) -> p two n", two=2)
            kb3 = sb["kbTdr"][:].rearrange("p (two b m) -> p two b m",
                                           two=2, b=PB)
            qa3 = sb["qaTdr"][:].rearrange("p (two n) -> p two n", two=2)

            # The whole attention is one score stream of 102,400 columns:
            #   cols [1600j + 800h, +800)          = path1 pair j, q = 2j+h
            #   cols [51200 + 6400p + o, ...)      = path2 batch p
            # chunked into CW-wide exp stages (3-bank PSUM S tiles).
            CW = 1536
            SL = 102400
            NT = (SL + CW - 1) // CW  # 67 chunks (last 1024)
            segs = []  # (base, length, lhsT, rhs3)
            for j in range(B // 2):
                for h in range(2):
                    segs.append((1600 * j + 800 * h, 800,
                                 ka3[:, :, 2 * j + h, 0:N], qb3))
            for p in range(PB):
                segs.append((51200 + 6400 * p, 6400, kb3[:, :, p, 0:N], qa3))

            etiles = {}  # chunk index -> E tile

            def eslices(a, b):
                """Stream range [a, b) as a list of E-tile slices."""
                out = []
                while a < b:
                    t = a // CW
                    e = min(b, (t + 1) * CW)
                    out.append(etiles[t][:][:, a - t * CW:e - t * CW])
                    a = e
                return out

            def emit_front(t):
                """Score matmuls + one exp for stream chunk t."""
                c0, c1 = CW * t, min(CW * (t + 1), SL)
                sa = spool.tile([100, CW], F32, tag="S", name=f"S{t % 2}")
                E = epool.tile([100, CW], F16, tag="E")
                for base, ln, lhsT, rhs3 in segs:
                    a, b = max(c0, base), min(c1, base + ln)
                    while a < b:  # split at this S tile's 512-col banks
                        e = min(b, c0 + ((a - c0) // 512 + 1) * 512)
                        nc.tensor.matmul(
                            sa[:][:, a - c0:e - c0], lhsT,
                            rhs3[:, :, a - base:e - base],
                            start=True, stop=True, perf_mode=DR)
                        a = e
                nc.scalar.activation(E[:][:, 0:c1 - c0], sa[:][:, 0:c1 - c0],
                                     AF.Exp, scale=0.125)
                etiles[t] = E

            def emit_aligned(At, dcols, pairs):
                """At[:, d] = sum_i lhsT_i.T @ E[stream a_i + d] for
                d in [0, dcols). Dest is split at every E-chunk boundary of
                either source range so each dest interval is a complete
                start/stop accumulation group."""
                cuts = {0, dcols}
                for _, a in pairs:
                    c = (a // CW + 1) * CW
                    while c < a + dcols:
                        cuts.add(c - a)
                        c += CW
                cs = sorted(cuts)
                for d0, d1 in zip(cs, cs[1:]):
                    for i, (lhsT, a) in enumerate(pairs):
                        (sl,) = eslices(a + d0, a + d1)
                        nc.tensor.matmul(At[:][:, d0:d1], lhsT, sl,
                                         start=(i == 0),
                                         stop=(i == len(pairs) - 1),
                                         skip_group_check=True)

            as2_live = {}

            def emit_back(g):
                """Aligned matmuls + copy (+DMA) for finished group g."""
                if g < B // 2:  # path1 pair j
                    j = g
                    vaLs = va_t["vaL"][:, 128 * j:128 * (j + 1)]
                    vaRs = va_t["vaR"][:, 128 * j:128 * (j + 1)]
                    As = mpool.tile([128, 800], BF16, tag="As")
                    for lo, w in ((0, 512), (512, 288)):
                        At = apool.tile([128, 512], F32, tag="A")
                        emit_aligned(At, w,
                                     [(vaLs, 1600 * j + lo),
                                      (vaRs, 1600 * j + 800 + lo)])
                        nc.vector.tensor_copy(As[:][:, lo:lo + w],
                                              At[:][:, 0:w])
                    nc.sync.dma_start(as1o[:, BN * j:BN * (j + 1)], As[:])
                else:  # path2 800-col group
                    gg = g - B // 2
                    p, o8 = gg // 8, (gg % 8) * 800
                    base = 51200 + 6400 * p + o8
                    vbLs = sb["vbL"][:, 128 * p:128 * (p + 1)]
                    vbRs = sb["vbR"][:, 128 * p:128 * (p + 1)]
                    At = apool.tile([128, 512], F32, tag="A")
                    emit_aligned(At, 400, [(vbLs, base), (vbRs, base + 400)])
                    u = (gg % 8) // 2
                    if gg % 2 == 0:
                        as2_live[p] = mpool.tile([128, 800], BF16, tag="As", name="As2")
                    As2 = as2_live[p]
                    nc.vector.tensor_copy(
                        As2[:][:, 400 * (gg % 2):400 * (gg % 2) + 400],
                        At[:][:, 0:400])
                    if gg % 2 == 1:
                        nc.sync.dma_start(
                            as2o[:, 3200 * p + 800 * u:
                                 3200 * p + 800 * (u + 1)], As2[:])

            # group g ready once its last stream column's chunk is emitted
            ends = [1600 * (j + 1) for j in range(B // 2)] + \
                   [51200 + 6400 * (gg // 8) + 800 * (gg % 8) + 800
                    for gg in range(64)]
            ready = [(e + CW - 1) // CW - 1 for e in ends]
            for t in range(NT + 1):
                if t < NT:
                    emit_front(t)
                for g in range(len(ends)):
                    if ready[g] == t - 1:
                        emit_back(g)

    _split_multi_waits(nc)
    return nc


# ---------------------------------------------------------------- host

_progs = {}


def _install_compile_cache():
    """Persist compiled NEFF-wrapped custom calls across processes: walrus
    compilation takes tens of seconds per program and bass2jax recompiles
    in every fresh process otherwise."""
    import hashlib
    import pathlib
    from concourse import bass2jax
    if getattr(bass2jax, "_ant_disk_cache", False):
        return
    bass2jax._ant_disk_cache = True
    orig = bass2jax.neuronx_cc_hook
    cdir = pathlib.Path(os.environ.get("BASS_NEFF_CACHE",
                                       "/tmp/bass_neff_cache"))
    try:
        cdir.mkdir(parents=True, exist_ok=True)
    except OSError:
        return

    def cached_hook(code, code_format, platform_version, file_prefix):
        try:
            key = hashlib.sha256(
                bytes(code) + b"|" + bytes(code_format)).hexdigest()
            path = cdir / f"{key}.neffcall"
            if path.exists():
                return 0, path.read_bytes()
        except Exception:
            return orig(code, code_format, platform_version, file_prefix)
        rc, blob = orig(code, code_format, platform_version, file_prefix)
        if rc == 0:
            try:
                tmp = path.with_suffix(f".tmp{os.getpid()}")
                tmp.write_bytes(blob)
                tmp.rename(path)
            except OSError:
                pass
        return rc, blob

    bass2jax.neuronx_cc_hook = cached_hook
    try:
        import libneuronxla
        if libneuronxla.neuronx_cc is orig:
            libneuronxla.neuronx_cc = cached_hook
    except ImportError:
        pass


def _get_progs():
    if "p1" not in _progs:
        _install_compile_cache()
        _progs["p1"] = build_prog1()
        _progs["p2"] = build_prog2()
    return _progs["p1"], _progs["p2"]


def _masters():
    import ml_dtypes
    m1 = np.zeros((128, 320), ml_dtypes.bfloat16)
    m1[0:64, 128] = 1.0   # up-plane (rows 0:64 of rhs) -> out row q
    m1[64:128, 129] = 1.0  # down-plane -> out row q+1
    m8 = np.zeros((128, 320), ml_dtypes.bfloat16)
    m8[0:64, 128] = 1.0
    m8[64:128, 136] = 1.0  # down-plane -> out row r0+8
    return m1, m8


def _dr_pack_k(x, pad_to=None):
    """Pack [K, M] (K contraction, even) into DoubleRow layout
    [K//2, 2*M] fp8e4 with k = (K//2)*s + p."""
    import ml_dtypes
    K = x.shape[0]
    h = K // 2
    arr = x.reshape(2, h, *x.shape[1:]).transpose(1, 0, *range(2, x.ndim + 1))
    return np.ascontiguousarray(arr.reshape(h, -1).astype(
        ml_dtypes.float8_e4m3fn))


def _dr_pack_k_padded(x, nblk, blk, pad):
    """[K, nblk*blk] -> DR fp8 [K//2, 2*nblk*pad] with each blk padded."""
    import ml_dtypes
    K = x.shape[0]
    h = K // 2
    a = x.reshape(2, h, nblk, blk).transpose(1, 0, 2, 3)
    z = np.zeros((h, 2, nblk, pad), np.float32)
    z[:, :, :, 0:blk] = a
    return np.ascontiguousarray(z.reshape(h, -1).astype(
        ml_dtypes.float8_e4m3fn))


def kernel(features_a, features_b, Wq1, Wq2, Wk1, Wk2, Wv1, Wv2):
    import ml_dtypes
    nc1, nc2 = _get_progs()
    cc = np.ascontiguousarray
    FP8 = ml_dtypes.float8_e4m3fn

    fa = np.asarray(features_a, np.float32).reshape(B, C, N)
    fb = np.asarray(features_b, np.float32).reshape(B, C, N)

    def feat8(fa_core, fb_core):  # 2x [PB, C, N] -> [128, 8*BN] fp8
        # [sd, b, s, p, n] with cin = 256b + 128s + p -> [p, sd, b, s, n]
        fT = np.stack([fc.transpose(1, 0, 2).reshape(C, BN)
                       for fc in (fa_core, fb_core)])
        a = fT.reshape(2, 2, 2, 128, BN).transpose(3, 0, 1, 2, 4)
        return cc(a.reshape(128, 8 * BN).astype(FP8))

    def wpack(Ws):  # list of [C, M] -> [128, 3*2*2*M] fp8
        a = np.stack([np.asarray(W, np.float32) for W in Ws])
        M = a.shape[-1]
        a = a.reshape(3, 2, 2, 128, M).transpose(3, 0, 1, 2, 4)
        return cc(a.reshape(128, 12 * M).astype(FP8))

    ws = {"w1dr": wpack([Wq1, Wk1, Wv1]), "w2dr": wpack([Wq2, Wk2, Wv2])}

    in1 = [dict(f8=feat8(fa[PB * i:PB * (i + 1)], fb[PB * i:PB * (i + 1)]),
                **ws)
           for i in range(CORES)]
    res1 = run_bass_kernel_spmd(nc1, in1, core_ids=list(range(CORES)))

    qaT = np.concatenate([res1.results[i]["qko_a"][0:64]
                          for i in range(CORES)], axis=1)
    kaT = np.concatenate([res1.results[i]["qko_a"][64:128]
                          for i in range(CORES)], axis=1)
    vaT = np.concatenate([res1.results[i]["vo_a"]
                          for i in range(CORES)], axis=1)
    qbT = [res1.results[i]["qko_b"][0:64] for i in range(CORES)]
    kbT = [res1.results[i]["qko_b"][64:128] for i in range(CORES)]
    vbT = [res1.results[i]["vo_b"] for i in range(CORES)]

    # a-side derived tensors (shared by all cores)
    vaT32 = vaT.astype(np.float32)
    va_nm = cc(vaT.T)                       # [B*N, INNER] fp16
    na = np.maximum(np.sqrt((vaT32 * vaT32).sum(0)), EPS)
    vhat_aT = vaT32 / na[None, :]
    vaL = np.zeros((N, (B // 2) * 128), np.float16)
    vaR = np.zeros((N, (B // 2) * 128), np.float16)
    for j in range(B // 2):
        vaL[:, 128 * j:128 * j + 64] = va_nm[N * 2 * j:N * (2 * j + 1)]
        vaR[:, 128 * j + 64:128 * (j + 1)] = va_nm[N * (2 * j + 1):
                                                   N * (2 * j + 2)]
    vhat_aT2 = np.zeros((128, B * N // 2), np.float32)
    for j2 in range(8):
        vhat_aT2[0:64, 400 * j2:400 * (j2 + 1)] = \
            vhat_aT[:, 800 * j2:800 * j2 + 400]
        vhat_aT2[64:128, 400 * j2:400 * (j2 + 1)] = \
            vhat_aT[:, 800 * j2 + 400:800 * (j2 + 1)]
    m1, m8 = _masters()

    kaTdr = _dr_pack_k_padded(kaT.astype(np.float32), B, N, MP)
    qaTdr = _dr_pack_k(qaT.astype(np.float32))
    in2 = []
    vhat_bTs = []
    for i in range(CORES):
        vbT32 = vbT[i].astype(np.float32)
        vb_nm = cc(vbT[i].T)                # [BN, INNER] fp16
        nb = np.maximum(np.sqrt((vbT32 * vbT32).sum(0)), EPS)
        vhat_bT = vbT32 / nb[None, :]
        vbL = np.zeros((N, PB * 128), np.float16)
        vbR = np.zeros((N, PB * 128), np.float16)
        for p in range(PB):
            vbL[:, 128 * p:128 * p + 64] = vb_nm[N * p:N * (p + 1)]
            vbR[:, 128 * p + 64:128 * (p + 1)] = vb_nm[N * p:N * (p + 1)]
        vhat_bTs.append(vhat_bT)
        in2.append(dict(
            kaTdr=kaTdr, qaTdr=qaTdr,
            qbTdr=_dr_pack_k(qbT[i].astype(np.float32)),
            kbTdr=_dr_pack_k_padded(kbT[i].astype(np.float32), PB, N, MP),
            vaL=vaL, vaR=vaR, vbL=vbL, vbR=vbR))
    res2 = run_bass_kernel_spmd(nc2, in2, core_ids=list(range(CORES)))

    sim = np.zeros((B, B), np.float32)
    for i in range(CORES):
        r = res2.results[i]
        # path1: As1 col-block 800j = pair j (rows 0:64 -> q=2j,
        # rows 64:128 -> q=2j+1, cols (p, n)); dot/ny2 on host
        as1 = np.asarray(r["as1o"], np.float32).reshape(128, 32, 800)
        vb_h = vhat_bTs[i]                              # [64 i, 800 (p n)]
        ny2_1 = np.empty((64, 800), np.float32)
        dot1 = np.empty((64, 800), np.float32)
        ny2_1[0::2] = (as1[0:64] ** 2).sum(0)
        ny2_1[1::2] = (as1[64:128] ** 2).sum(0)
        dot1[0::2] = np.einsum('ijc,ic->jc', as1[0:64], vb_h)
        dot1[1::2] = np.einsum('ijc,ic->jc', as1[64:128], vb_h)
        cos1 = dot1 / np.maximum(np.sqrt(ny2_1), EPS)
        sim1 = cos1.reshape(64, PB, N).sum(-1)          # [q, p]

        # path2: As2 cols 3200p + 800g + 400h + c; rows 0:64 ->
        # qn = 800*(2g+h)+c, rows 64:128 -> +400; vhat_a [64, (g,h,half,c)]
        as2 = np.asarray(r["as2o"], np.float32).reshape(128, PB, 4, 2, 400)
        va4 = vhat_aT.reshape(64, 4, 2, 2, 400)         # [i, g, h, half, c]
        ny_lo = (as2[0:64] ** 2).sum(0).reshape(PB, 8, 400)
        ny_hi = (as2[64:128] ** 2).sum(0).reshape(PB, 8, 400)
        ny2_2 = np.concatenate([ny_lo, ny_hi], axis=2).reshape(PB, B * N)
        d_lo = np.einsum('ipghc,ighc->pghc', as2[0:64], va4[:, :, :, 0])
        d_hi = np.einsum('ipghc,ighc->pghc', as2[64:128], va4[:, :, :, 1])
        dot2 = np.concatenate([d_lo.reshape(PB, 8, 400),
                               d_hi.reshape(PB, 8, 400)],
                              axis=2).reshape(PB, B * N)
        cos2 = dot2 / np.maximum(np.sqrt(ny2_2), EPS)
        sim2 = cos2.reshape(PB, B, N).sum(-1)           # [p, q]

        sim[PB * i:PB * (i + 1)] = (sim1.T + sim2) / N
    return sim



# revision 14
# speedup vs baseline: 1.4447x; 1.0063x over previous
"""Trainium2 Bass kernel for nn_AttentionSimilarity.

Contract: kernel(**inputs) takes the FULL unsharded inputs (numpy) and
returns the FULL [64, 64] similarity matrix, distributing work across 8
NeuronCores internally.

Structure:
  prog1 (projections, sharded by batch): each core projects its 8
    a-batches and 8 b-batches through the three two-layer MLPs,
    emitting qaT/kaT/vaT/qbT/kbT/vbT chunks in [inner, (batch, n)]
    layout. Host gathers the a-side to full tensors.
  prog2 (attention, sharded by p = b-side batch): each core computes
    both attention paths for its 8 p's against all 64 q's, the cosine
    numerators/denominators via selector matmuls on the PE, and the
    per-(p,q) sums over n. Host assembles the [64, 64] output.

Math notes:
  - softmax feeds only cosine similarity, which is scale-invariant in
    the aligned vector, so the softmax max-shift and denominator cancel:
    softmax reduces to exp(scores/8).
  - the x-side cosine norm is folded on the host (vhat = v / max(|v|, eps)).
  - 1/max(|y|, eps) and the dot with vhat are applied on the host from
    the streamed-out aligned values.

Performance notes (vs the first working version):
  - prog1 W1 layer and both programs' score matmuls run in fp8e4 with
    MatmulPerfMode.DoubleRow (2 contraction rows per PE partition, 0.5
    cycles/output column): weights/features/q/k are DR-packed on the
    host ([K/2, 2, M] with k = (K/2)*s + p; lhsT m-blocks padded to
    MP=112 so the DR pair-stride stays 16-byte aligned).
  - the entire cosine stage (dot, squared-norm, rsqrt, mean over n)
    is computed on the HOST: the aligned values (As, bf16) stream out
    over the otherwise-idle DMA engines, deleting the M/SQ multiplies,
    all selector-reduce matmuls, the P1/P2 PSUM accumulators (freeing
    banks for aligned double-buffering), and the device epilogues.
    The device does projections, scores, softmax-exp and the aligned
    matmuls -- all of the O(B^2 N^2) compute.
  - warmup/tail: weight DMAs are split/consolidated so the first matmul
    starts as early as possible; prog1's W2 PSUM/copy/DMA pipeline is
    chunked per bank so stores drain during compute; path2 score tiles
    are 1536 columns (3 PSUM banks) to amortize the fixed per-
    instruction ACT access latency on the softmax exp, which is the
    saturated engine (~98% busy) in the final balance.
  - measured rel err vs fp32 reference: ~1.7e-3.

Dead end (measured): packing score tiles to 128 partitions by mixing
(q, m) across rows would cut exp columns 100/128, but the follow-up
aligned matmuls need operand slices at arbitrary partition offsets and
the PE requires base partition 0/32/64 (bass matmul assert); since
100 is not a multiple of 32, per-q slices of a packed layout are
unaddressable. The [m<=100, cols] score layout is forced.
"""

import os
import sys

sys.path.insert(0, "/opt/trn_rl_repo")
os.environ.setdefault("NEURON_RT_RESET_CORES", "1")

import numpy as np
import ml_dtypes  # noqa: F401  (bf16 host arrays)

import bass_rust
import concourse.bass as bass
import concourse.mybir as mybir
import concourse.tile as tile
from concourse.bass_utils import run_bass_kernel_spmd

F32 = mybir.dt.float32
F32R = mybir.dt.float32r
BF16 = mybir.dt.bfloat16
F16 = mybir.dt.float16
F8E4 = mybir.dt.float8e4
AF = mybir.ActivationFunctionType
DR = mybir.MatmulPerfMode.DoubleRow

B = 64          # batches per side
C = 512         # channels
N = 100         # H*W tokens per batch
INNER = 64      # projected dim
CORES = 8
PB = B // CORES  # batches per core (8)
BN = PB * N      # 800: (batch, n) columns per core chunk
EPS = 1e-8
KT1 = C // 128   # prog1 contraction tiles (4)
MP = 112         # fp8-DR padded m stride (112 % 16 == 0, >= N)

E1_BUFS = int(os.environ.get("K_E1_BUFS", "5"))
SEL_LAG = int(os.environ.get("K_SEL_LAG", "4"))
POOL_MOD1 = int(os.environ.get("K_POOL_MOD1", os.environ.get("K_POOL_MOD", "3")))
POOL_MOD2 = int(os.environ.get("K_POOL_MOD2", os.environ.get("K_POOL_MOD", "2")))
SEL_LAG2 = int(os.environ.get("K_SEL_LAG2", "4"))
M2_BUFS = int(os.environ.get("K_M2_BUFS", "8"))
MPOOL_MOD = int(os.environ.get("K_MPOOL_MOD", "0"))  # 0=never, k=every kth M on pool
M_BUFS = int(os.environ.get("K_M_BUFS", "8"))
E2_BUFS = int(os.environ.get("K_E2_BUFS", "3"))
S1_BUFS = int(os.environ.get("K_S1_BUFS", "2"))
A1_BUFS = int(os.environ.get("K_A1_BUFS", "1"))

_waitsplit_ctr = [0]


def _split_multi_waits(nc, max_waits=1):
    """This container's walrus build accepts at most ONE sync wait per
    instruction; Tile attaches several. Move extras onto preceding
    same-engine NoOps (engines are in-order, so semantics hold)."""
    n_split = 0
    for f in nc.m.functions:
        for blk in f.blocks:
            insts = list(blk.instructions)
            new_list = []
            changed = False
            for inst in insts:
                si = inst.sync_info
                waits = list(si.on_wait) if (si is not None and si.on_wait) else []
                if len(waits) > max_waits:
                    for w in waits[:-max_waits]:
                        _waitsplit_ctr[0] += 1
                        nop = mybir.InstNoOp(
                            name=f"I-waitsplit-{_waitsplit_ctr[0]}",
                            engine=inst.engine,
                            ins=[],
                            outs=[],
                            sync_info=bass_rust.SyncInfo(on_wait=[w], on_update=[]),
                        )
                        nc.register_instruction(nop, overwrite=True)
                        new_list.append(nop)
                        n_split += 1
                    si.on_wait = waits[-max_waits:]
                    inst.sync_info = si
                    changed = True
                new_list.append(inst)
            if changed:
                blk.instructions = new_list
    return n_split


# ---------------------------------------------------------------- prog1

def build_prog1():
    """Projection program, K=256-per-pass DoubleRow everywhere.

    Per-core inputs (all fp8e4 DR-packed on the host):
      f8:    [128, 2*2*2*BN]   features; [p, (side, b, s, n)] holds
                               feat_side[cin = 256b + 128s + p, n]
      w1dr:  [128, 3*2*2*C]    [p, (proj, b, s, cout)] = W1[cin, cout]
      w2dr:  [128, 3*2*2*64]   [p, (proj, b2, s2, i)] = W2[cout, i]
                               (cout = 256*b2 + 128*s2 + p)
    Outputs (f16): qko_a/qko_b [128, BN] (q rows 0:64, k rows 64:128),
      vo_a/vo_b [64, BN].

    Hidden activations are stored fp8e4 so the W2 layer also runs
    DoubleRow (0.5 cyc/col); h layout [128, (b2, s2, n)] makes the DR
    rhs a plain strided view of the relu outputs.
    """
    nc = bass.Bass("TRN2", target_bir_lowering=False, debug=False,
                   num_devices=CORES)
    f8 = nc.dram_tensor("f8", [128, 8 * BN], F8E4, kind="ExternalInput").ap()
    hot1 = nc.dram_tensor("hot1", [128, 2 * (1024 + 1600)], F8E4,
                          kind="ExternalInput").ap()
    w1d = nc.dram_tensor("w1dr", [128, 12 * C], F8E4,
                         kind="ExternalInput").ap()
    w2d = nc.dram_tensor("w2dr", [128, 12 * INNER], F8E4,
                         kind="ExternalInput").ap()
    outs = {"a": nc.dram_tensor("qko_a", [128, BN], F16,
                                kind="ExternalOutput").ap(),
            "b": nc.dram_tensor("qko_b", [128, BN], F16,
                                kind="ExternalOutput").ap()}
    vouts = {"a": nc.dram_tensor("vo_a", [INNER, BN], F16,
                                 kind="ExternalOutput").ap(),
             "b": nc.dram_tensor("vo_b", [INNER, BN], F16,
                                 kind="ExternalOutput").ap()}
    CH = [(0, 512), (512, BN)]  # psum-bank-aligned column chunks of BN

    with tile.TileContext(nc) as tc:
        with (
            tc.tile_pool(name="wpool", bufs=1) as wpool,
            tc.tile_pool(name="hpool", bufs=3) as hpool,
            tc.tile_pool(name="opool", bufs=4) as opool,
            tc.tile_pool(name="psH", bufs=4, space="PSUM") as psHp,
        ):
            # weights + features, hot-first.  w1sb view: [p, proj, b, s,
            # cout]; f view: [p, side, b, s, n]; w2sb: [p, proj, b2, s2, i].
            w1sb = wpool.tile([128, 12 * C], F8E4, tag="w1", name="w1sb")
            w1v = w1sb[:].rearrange("p (pr b s c) -> p pr b s c", pr=3, b=2,
                                    s=2)
            w1dv = w1d.rearrange("p (pr b s c) -> p pr b s c", pr=3, b=2, s=2)
            fsb = wpool.tile([128, 8 * BN], F8E4, tag="f", name="fsb")
            fv = fsb[:].rearrange("p (sd b s n) -> p sd b s n", sd=2, b=2,
                                  s=2)
            fdv = f8.rearrange("p (sd b s n) -> p sd b s n", sd=2, b=2, s=2)
            w2sb = wpool.tile([128, 12 * INNER], F8E4, tag="w2", name="w2sb")
            hotsb = wpool.tile([128, 5248], F8E4, tag="hot", name="hotsb")
            hotv = hotsb[:].rearrange("p (b x) -> p b x", b=2)
            hotd = hot1.rearrange("p (b x) -> p b x", b=2)
            nc.sync.dma_start(hotv[:, 0], hotd[:, 0])
            nc.sync.dma_start(hotv[:, 1], hotd[:, 1])
            nc.sync.dma_start(w1v[:, 1:3], w1dv[:, 1:3])
            nc.sync.dma_start(w2sb[:], w2d[:])
            nc.sync.dma_start(fv[:, 1], fdv[:, 1])
            w1qv = hotv[:, :, 0:1024].rearrange("p b (s c) -> p b s c", s=2)
            fav = hotv[:, :, 1024:2624].rearrange("p b (s n) -> p b s n", s=2)
            w2v = w2sb[:].rearrange("p (pr b s i) -> p pr b s i", pr=3, b=2,
                                    s=2)

            # relu engines, weighted round-robin (ACT/DVE faster than Pool)
            relu_cyc = [0]

            def relu(dst, src):
                e = (nc.scalar, nc.vector, nc.scalar, nc.vector,
                     nc.gpsimd)[relu_cyc[0] % 5]
                relu_cyc[0] += 1
                if e is nc.scalar:
                    e.activation(dst, src, AF.Relu)
                else:
                    e.tensor_scalar_max(dst, src, 0.0)

            hts = {}

            def w1(si, pi):
                ht = hpool.tile([128, 4 * BN], F8E4, tag="h",
                                name=f"h{si}{pi}")
                hv = ht[:].rearrange("p (b s n) -> p b s n", b=2, s=2)
                for t in range(4):
                    psH = psHp.tile([128, 1024], F32, tag="psH", name="psH")
                    for b in range(2):
                        lhsT = (w1qv[:, b, :, 128 * t:128 * (t + 1)]
                                if pi == 0 else
                                w1v[:, pi, b, :, 128 * t:128 * (t + 1)])
                        for lo, hi in CH:
                            nc.tensor.matmul(
                                psH[:, lo:hi], lhsT,
                                (fav[:, b, :, lo:hi] if si == 0 else
                                 fv[:, 1, b, :, lo:hi]),
                                start=(b == 0), stop=(b == 1), perf_mode=DR)
                    relu(hv[:, t // 2, t % 2], psH[:, 0:BN])
                hts[(si, pi)] = hv

            def w2qk(si, s):
                """q rows 0:64 (DR; DR needs dst partition base 0) + k rows
                64:128 (plain fp8) of one psO tile, chunk-wise copy+DMA."""
                psO = psHp.tile([128, 1024], F32, tag="psH", name="psOqk")
                ot = opool.tile([128, BN], F16, tag="out", name="qkout")
                for lo, hi in CH:
                    for b2 in range(2):
                        nc.tensor.matmul(
                            psO[0:64, lo:hi], w2v[:, 0, b2],
                            hts[(si, 0)][:, b2, :, lo:hi],
                            start=(b2 == 0), stop=(b2 == 1), perf_mode=DR)
                    for b2 in range(2):
                        for s2 in range(2):
                            nc.tensor.matmul(
                                psO[64:128, lo:hi], w2v[:, 1, b2, s2],
                                hts[(si, 1)][:, b2, s2, lo:hi],
                                start=(b2 == 0 and s2 == 0),
                                stop=(b2 == 1 and s2 == 1))
                    nc.gpsimd.tensor_copy(ot[:][:, lo:hi], psO[:, lo:hi])
                    nc.sync.dma_start(outs[s][:, lo:hi], ot[:][:, lo:hi])

            def w2v_(si, s):
                psV = psHp.tile([128, 1024], F32, tag="psH", name="psOv")
                vt = opool.tile([INNER, BN], F16, tag="vout", name="vout")
                for lo, hi in CH:
                    for b2 in range(2):
                        nc.tensor.matmul(
                            psV[0:64, lo:hi], w2v[:, 2, b2],
                            hts[(si, 2)][:, b2, :, lo:hi],
                            start=(b2 == 0), stop=(b2 == 1), perf_mode=DR)
                    nc.scalar.copy(vt[:, lo:hi], psV[0:64, lo:hi])
                    nc.sync.dma_start(vouts[s][:, lo:hi], vt[:, lo:hi])

            # PE stream interleaved so W2v(a)'s psO WAR-wait on the qk copy
            # hides under W1q(b), and the relu pipeline never starves PE.
            w1(0, 0); w1(0, 1); w1(0, 2)
            w2qk(0, "a")
            w1(1, 0)
            w2v_(0, "a")
            w1(1, 1); w1(1, 2)
            w2qk(1, "b")
            w2v_(1, "b")

    _split_multi_waits(nc)
    return nc


# ---------------------------------------------------------------- prog2

def build_prog2():
    """Attention program, sharded over p (this core's 8 b-batches).

    Unified 64-stage software pipeline; every stage produces 1600 score
    columns in a [128, 2048] PSUM tile (4 banks, double-buffered = all 8
    banks), does ONE 1600-wide exp on ACT (the bottleneck engine), then
    reuses the exp-consumed banks of the same tile as the aligned-matmul
    accumulator (carve-after-read; subtile deps order the WAR hazard).
    Stage t+1's score matmuls are emitted before stage t's aligned
    matmuls so PE always has score work ready when ACT finishes an exp.

      path1 stage j (32): scores for q-pair (2j, 2j+1) over this core's
        800 (p, n) columns; q0 at S cols 0:800, q1 at 1024:1824; exp via
        a strided [100, 2, 800] AP; aligned A at cols 0:800.
      path2 stage (p, k) (32): scores for 1600 (q n) columns
        [1600k, 1600k+1600) against kb[p]; aligned A groups at cols
        0:400 and 512:912; strided copy out.

    Outputs (identical layout to the previous version; host unchanged):
      as1o [128, 32*800] bf16, as2o [128, 32*800] bf16
    """
    nc = bass.Bass("TRN2", target_bir_lowering=False, debug=False,
                   num_devices=CORES)
    din = {}
    for name, shape, dt in [
        ("kaTdr", [32, 2 * B * MP], F8E4), ("qaTdr", [32, 2 * B * N], F8E4),
        ("qbTdr", [32, 2 * BN], F8E4), ("kbTdr", [32, 2 * PB * MP], F8E4),
        ("hot2", [32, 4992], F8E4),
        ("vaL", [N, (B // 2) * 128], F16), ("vaR", [N, (B // 2) * 128], F16),
        ("vbL", [N, PB * 128], F16), ("vbR", [N, PB * 128], F16),
    ]:
        din[name] = nc.dram_tensor(name, shape, dt, kind="ExternalInput").ap()
    as1o = nc.dram_tensor("as1o", [128, 32 * BN], BF16,
                          kind="ExternalOutput").ap()
    as2o = nc.dram_tensor("as2o", [128, 32 * 800], BF16,
                          kind="ExternalOutput").ap()

    with tile.TileContext(nc) as tc:
        from contextlib import ExitStack
        with ExitStack() as ctx:
            inp = ctx.enter_context(tc.tile_pool(name="inp", bufs=1))
            sb = {}

            def load(name):
                ap = din[name]
                t = inp.tile(list(ap.shape), ap.dtype, tag=name,
                             name=f"sb_{name}")
                nc.sync.dma_start(t[:], ap[:])
                sb[name] = t

            # Input DMAs, hot-first. All on the SP (sync) queue, issued
            # before any output DMA so no wait ever blocks the SP SEQ.
            ka_t = inp.tile([32, 2 * B * MP], F8E4, tag="kaTdr",
                            name="sb_kaTdr")
            sb["kaTdr"] = ka_t
            ka3d = din["kaTdr"].rearrange("p (two q m) -> p two q m",
                                          two=2, q=B)
            ka3s = ka_t[:].rearrange("p (two q m) -> p two q m", two=2, q=B)
            hot2 = inp.tile([32, 4992], F8E4, tag="hot2", name="sb_hot2")
            nc.sync.dma_start(hot2[:], din["hot2"][:])
            va_t = {}
            for nm in ("vaL", "vaR"):
                va_t[nm] = inp.tile([N, (B // 2) * 128], F16, tag=nm,
                                    name=f"sb_{nm}")
                nc.sync.dma_start(va_t[nm][:, 0:512], din[nm][:, 0:512])
            for nm in ("vaL", "vaR"):
                nc.sync.dma_start(va_t[nm][:, 512:2048], din[nm][:, 512:2048])
            nc.sync.dma_start(ka3s[:, :, 8:32, :], ka3d[:, :, 8:32, :])
            for nm in ("vaL", "vaR"):
                nc.sync.dma_start(va_t[nm][:, 2048:4096],
                                  din[nm][:, 2048:4096])
            nc.sync.dma_start(ka3s[:, :, 32:64, :], ka3d[:, :, 32:64, :])
            qa_t = inp.tile([32, 2 * B * N], F8E4, tag="qaTdr",
                            name="sb_qaTdr")
            sb["qaTdr"] = qa_t
            qa3d = din["qaTdr"].rearrange("p (two n) -> p two n", two=2)
            qa3s = qa_t[:].rearrange("p (two n) -> p two n", two=2)
            nc.sync.dma_start(qa3s[:, :, 0:3200], qa3d[:, :, 0:3200])
            nc.sync.dma_start(qa3s[:, :, 3200:6400], qa3d[:, :, 3200:6400])
            for name in ("kbTdr", "vbL", "vbR"):
                load(name)

            epool = ctx.enter_context(tc.tile_pool(name="epool", bufs=4))
            mpool = ctx.enter_context(tc.tile_pool(name="mpool", bufs=8))
            spool = ctx.enter_context(
                tc.tile_pool(name="spool", bufs=2, space="PSUM"))
            apool = ctx.enter_context(
                tc.tile_pool(name="apool", bufs=2, space="PSUM"))

            ka3 = sb["kaTdr"][:].rearrange("p (two q m) -> p two q m",
                                           two=2, q=B)
            qb3 = hot2[:][:, 0:3200].rearrange("p (two n) -> p two n",
                                               two=2)
            ka_hot = hot2[:][:, 3200:4992].rearrange(
                "p (two q m) -> p two q m", two=2, q=8)
            kb3 = sb["kbTdr"][:].rearrange("p (two b m) -> p two b m",
                                           two=2, b=PB)
            qa3 = sb["qaTdr"][:].rearrange("p (two n) -> p two n", two=2)

            # The whole attention is one score stream of 102,400 columns:
            #   cols [1600j + 800h, +800)          = path1 pair j, q = 2j+h
            #   cols [51200 + 6400p + o, ...)      = path2 batch p
            # chunked into CW-wide exp stages (3-bank PSUM S tiles).
            SL = 102400
            # chunk grid: uniform 1536 except a custom tail so only ONE
            # 800-col aligned group drains after the final exp
            BND = [1536 * i for i in range(66)] + [101376, 101600, SL]
            NT = len(BND) - 1
            import bisect as _bi

            def chunk_of(pos):
                return _bi.bisect_right(BND, pos) - 1
            segs = []  # (base, length, lhsT, rhs3)
            for j in range(B // 2):
                for h in range(2):
                    q = 2 * j + h
                    lhsT = (ka_hot[:, :, q, 0:N] if q < 8 else
                            ka3[:, :, q, 0:N])
                    segs.append((1600 * j + 800 * h, 800, lhsT, qb3))
            for p in range(PB):
                segs.append((51200 + 6400 * p, 6400, kb3[:, :, p, 0:N], qa3))

            etiles = {}  # chunk index -> E tile

            def eslices(a, b):
                """Stream range [a, b) as a list of E-tile slices."""
                out = []
                while a < b:
                    t = chunk_of(a)
                    e = min(b, BND[t + 1])
                    out.append(etiles[t][:][:, a - BND[t]:e - BND[t]])
                    a = e
                return out

            def emit_front(t):
                """Score matmuls + one exp for stream chunk t."""
                c0, c1 = BND[t], BND[t + 1]
                sa = spool.tile([100, 1536], F32, tag="S", name=f"S{t % 2}")
                E = epool.tile([100, 1536], F16, tag="E")
                for base, ln, lhsT, rhs3 in segs:
                    a, b = max(c0, base), min(c1, base + ln)
                    while a < b:  # split at this S tile's 512-col banks
                        e = min(b, c0 + ((a - c0) // 512 + 1) * 512)
                        nc.tensor.matmul(
                            sa[:][:, a - c0:e - c0], lhsT,
                            rhs3[:, :, a - base:e - base],
                            start=True, stop=True, perf_mode=DR)
                        a = e
                nc.scalar.activation(E[:][:, 0:c1 - c0], sa[:][:, 0:c1 - c0],
                                     AF.Exp, scale=0.125)
                etiles[t] = E

            def emit_aligned(At, dcols, pairs):
                """At[:, d] = sum_i lhsT_i.T @ E[stream a_i + d] for
                d in [0, dcols). Dest is split at every E-chunk boundary of
                either source range so each dest interval is a complete
                start/stop accumulation group."""
                cuts = {0, dcols}
                for _, a in pairs:
                    for t in range(chunk_of(a) + 1, chunk_of(a + dcols - 1) + 1):
                        cuts.add(BND[t] - a)
                cs = sorted(cuts)
                for d0, d1 in zip(cs, cs[1:]):
                    for i, (lhsT, a) in enumerate(pairs):
                        (sl,) = eslices(a + d0, a + d1)
                        nc.tensor.matmul(At[:][:, d0:d1], lhsT, sl,
                                         start=(i == 0),
                                         stop=(i == len(pairs) - 1),
                                         skip_group_check=True)

            as2_live = {}

            def emit_back(g):
                """Aligned matmuls + copy (+DMA) for finished group g."""
                if g < B // 2:  # path1 pair j
                    j = g
                    vaLs = va_t["vaL"][:, 128 * j:128 * (j + 1)]
                    vaRs = va_t["vaR"][:, 128 * j:128 * (j + 1)]
                    As = mpool.tile([128, 800], BF16, tag="As")
                    for lo, w in ((0, 512), (512, 288)):
                        At = apool.tile([128, 512], F32, tag="A")
                        emit_aligned(At, w,
                                     [(vaLs, 1600 * j + lo),
                                      (vaRs, 1600 * j + 800 + lo)])
                        nc.vector.tensor_copy(As[:][:, lo:lo + w],
                                              At[:][:, 0:w])
                    nc.sync.dma_start(as1o[:, BN * j:BN * (j + 1)], As[:])
                else:  # path2 800-col group
                    gg = g - B // 2
                    p, o8 = gg // 8, (gg % 8) * 800
                    base = 51200 + 6400 * p + o8
                    vbLs = sb["vbL"][:, 128 * p:128 * (p + 1)]
                    vbRs = sb["vbR"][:, 128 * p:128 * (p + 1)]
                    At = apool.tile([128, 512], F32, tag="A")
                    emit_aligned(At, 400, [(vbLs, base), (vbRs, base + 400)])
                    u = (gg % 8) // 2
                    if gg % 2 == 0:
                        as2_live[p] = mpool.tile([128, 800], BF16, tag="As", name="As2")
                    As2 = as2_live[p]
                    nc.vector.tensor_copy(
                        As2[:][:, 400 * (gg % 2):400 * (gg % 2) + 400],
                        At[:][:, 0:400])
                    if gg % 2 == 1:
                        nc.sync.dma_start(
                            as2o[:, 3200 * p + 800 * u:
                                 3200 * p + 800 * (u + 1)], As2[:])

            # group g ready once its last stream column's chunk is emitted
            ends = [1600 * (j + 1) for j in range(B // 2)] + \
                   [51200 + 6400 * (gg // 8) + 800 * (gg % 8) + 800
                    for gg in range(64)]
            ready = [chunk_of(e - 1) for e in ends]
            for t in range(NT + 1):
                if t < NT:
                    emit_front(t)
                for g in range(len(ends)):
                    if ready[g] == t - 1:
                        emit_back(g)

    _split_multi_waits(nc)
    return nc


# ---------------------------------------------------------------- host

_progs = {}


def _install_compile_cache():
    """Persist compiled NEFF-wrapped custom calls across processes: walrus
    compilation takes tens of seconds per program and bass2jax recompiles
    in every fresh process otherwise."""
    import hashlib
    import pathlib
    from concourse import bass2jax
    if getattr(bass2jax, "_ant_disk_cache", False):
        return
    bass2jax._ant_disk_cache = True
    orig = bass2jax.neuronx_cc_hook
    cdir = pathlib.Path(os.environ.get("BASS_NEFF_CACHE",
                                       "/tmp/bass_neff_cache"))
    try:
        cdir.mkdir(parents=True, exist_ok=True)
    except OSError:
        return

    def cached_hook(code, code_format, platform_version, file_prefix):
        try:
            key = hashlib.sha256(
                bytes(code) + b"|" + bytes(code_format)).hexdigest()
            path = cdir / f"{key}.neffcall"
            if path.exists():
                return 0, path.read_bytes()
        except Exception:
            return orig(code, code_format, platform_version, file_prefix)
        rc, blob = orig(code, code_format, platform_version, file_prefix)
        if rc == 0:
            try:
                tmp = path.with_suffix(f".tmp{os.getpid()}")
                tmp.write_bytes(blob)
                tmp.rename(path)
            except OSError:
                pass
        return rc, blob

    bass2jax.neuronx_cc_hook = cached_hook
    try:
        import libneuronxla
        if libneuronxla.neuronx_cc is orig:
            libneuronxla.neuronx_cc = cached_hook
    except ImportError:
        pass


def _get_progs():
    if "p1" not in _progs:
        _install_compile_cache()
        _progs["p1"] = build_prog1()
        _progs["p2"] = build_prog2()
    return _progs["p1"], _progs["p2"]


def _masters():
    import ml_dtypes
    m1 = np.zeros((128, 320), ml_dtypes.bfloat16)
    m1[0:64, 128] = 1.0   # up-plane (rows 0:64 of rhs) -> out row q
    m1[64:128, 129] = 1.0  # down-plane -> out row q+1
    m8 = np.zeros((128, 320), ml_dtypes.bfloat16)
    m8[0:64, 128] = 1.0
    m8[64:128, 136] = 1.0  # down-plane -> out row r0+8
    return m1, m8


def _dr_pack_k(x, pad_to=None):
    """Pack [K, M] (K contraction, even) into DoubleRow layout
    [K//2, 2*M] fp8e4 with k = (K//2)*s + p."""
    import ml_dtypes
    K = x.shape[0]
    h = K // 2
    arr = x.reshape(2, h, *x.shape[1:]).transpose(1, 0, *range(2, x.ndim + 1))
    return np.ascontiguousarray(arr.reshape(h, -1).astype(
        ml_dtypes.float8_e4m3fn))


def _dr_pack_k_padded(x, nblk, blk, pad):
    """[K, nblk*blk] -> DR fp8 [K//2, 2*nblk*pad] with each blk padded."""
    import ml_dtypes
    K = x.shape[0]
    h = K // 2
    a = x.reshape(2, h, nblk, blk).transpose(1, 0, 2, 3)
    z = np.zeros((h, 2, nblk, pad), np.float32)
    z[:, :, :, 0:blk] = a
    return np.ascontiguousarray(z.reshape(h, -1).astype(
        ml_dtypes.float8_e4m3fn))


def kernel(features_a, features_b, Wq1, Wq2, Wk1, Wk2, Wv1, Wv2):
    import ml_dtypes
    nc1, nc2 = _get_progs()
    cc = np.ascontiguousarray
    FP8 = ml_dtypes.float8_e4m3fn

    fa = np.asarray(features_a, np.float32).reshape(B, C, N)
    fb = np.asarray(features_b, np.float32).reshape(B, C, N)

    def feat8(fa_core, fb_core):  # 2x [PB, C, N] -> [128, 8*BN] fp8
        # [sd, b, s, p, n] with cin = 256b + 128s + p -> [p, sd, b, s, n]
        fT = np.stack([fc.transpose(1, 0, 2).reshape(C, BN)
                       for fc in (fa_core, fb_core)])
        a = fT.reshape(2, 2, 2, 128, BN).transpose(3, 0, 1, 2, 4)
        return cc(a.reshape(128, 8 * BN).astype(FP8))

    def wpack(Ws):  # list of [C, M] -> [128, 3*2*2*M] fp8
        a = np.stack([np.asarray(W, np.float32) for W in Ws])
        M = a.shape[-1]
        a = a.reshape(3, 2, 2, 128, M).transpose(3, 0, 1, 2, 4)
        return cc(a.reshape(128, 12 * M).astype(FP8))

    ws = {"w1dr": wpack([Wq1, Wk1, Wv1]), "w2dr": wpack([Wq2, Wk2, Wv2])}
    w1q_b = np.asarray(Wq1, np.float32).reshape(2, 2, 128, C).transpose(
        2, 0, 1, 3).reshape(128, 2, 1024).astype(FP8)  # [p, b, (s c)]

    def hot1(f8c):  # f8c [128, 8*BN]: fuse [w1q-b | fa-b] per DR pass b
        fa4 = f8c.reshape(128, 2, 2, 2, BN)[:, 0].reshape(128, 2, 1600)
        return cc(np.concatenate([w1q_b, fa4], axis=2).reshape(128, 5248))

    in1 = []
    for i in range(CORES):
        f8c = feat8(fa[PB * i:PB * (i + 1)], fb[PB * i:PB * (i + 1)])
        in1.append(dict(f8=f8c, hot1=hot1(f8c), **ws))
    res1 = run_bass_kernel_spmd(nc1, in1, core_ids=list(range(CORES)))

    qaT = np.concatenate([res1.results[i]["qko_a"][0:64]
                          for i in range(CORES)], axis=1)
    kaT = np.concatenate([res1.results[i]["qko_a"][64:128]
                          for i in range(CORES)], axis=1)
    vaT = np.concatenate([res1.results[i]["vo_a"]
                          for i in range(CORES)], axis=1)
    qbT = [res1.results[i]["qko_b"][0:64] for i in range(CORES)]
    kbT = [res1.results[i]["qko_b"][64:128] for i in range(CORES)]
    vbT = [res1.results[i]["vo_b"] for i in range(CORES)]

    # a-side derived tensors (shared by all cores)
    vaT32 = vaT.astype(np.float32)
    va_nm = cc(vaT.T)                       # [B*N, INNER] fp16
    na = np.maximum(np.sqrt((vaT32 * vaT32).sum(0)), EPS)
    vhat_aT = vaT32 / na[None, :]
    vaL = np.zeros((N, (B // 2) * 128), np.float16)
    vaR = np.zeros((N, (B // 2) * 128), np.float16)
    for j in range(B // 2):
        vaL[:, 128 * j:128 * j + 64] = va_nm[N * 2 * j:N * (2 * j + 1)]
        vaR[:, 128 * j + 64:128 * (j + 1)] = va_nm[N * (2 * j + 1):
                                                   N * (2 * j + 2)]
    vhat_aT2 = np.zeros((128, B * N // 2), np.float32)
    for j2 in range(8):
        vhat_aT2[0:64, 400 * j2:400 * (j2 + 1)] = \
            vhat_aT[:, 800 * j2:800 * j2 + 400]
        vhat_aT2[64:128, 400 * j2:400 * (j2 + 1)] = \
            vhat_aT[:, 800 * j2 + 400:800 * (j2 + 1)]
    m1, m8 = _masters()

    kaTdr = _dr_pack_k_padded(kaT.astype(np.float32), B, N, MP)
    qaTdr = _dr_pack_k(qaT.astype(np.float32))
    in2 = []
    vhat_bTs = []
    for i in range(CORES):
        vbT32 = vbT[i].astype(np.float32)
        vb_nm = cc(vbT[i].T)                # [BN, INNER] fp16
        nb = np.maximum(np.sqrt((vbT32 * vbT32).sum(0)), EPS)
        vhat_bT = vbT32 / nb[None, :]
        vbL = np.zeros((N, PB * 128), np.float16)
        vbR = np.zeros((N, PB * 128), np.float16)
        for p in range(PB):
            vbL[:, 128 * p:128 * p + 64] = vb_nm[N * p:N * (p + 1)]
            vbR[:, 128 * p + 64:128 * (p + 1)] = vb_nm[N * p:N * (p + 1)]
        vhat_bTs.append(vhat_bT)
        qbdr_i = _dr_pack_k(qbT[i].astype(np.float32))
        hot2_i = cc(np.concatenate(
            [qbdr_i, kaTdr.reshape(32, 2, B, MP)[:, :, 0:8].reshape(32, 1792)],
            axis=1))
        in2.append(dict(
            kaTdr=kaTdr, qaTdr=qaTdr, hot2=hot2_i,
            qbTdr=qbdr_i,
            kbTdr=_dr_pack_k_padded(kbT[i].astype(np.float32), PB, N, MP),
            vaL=vaL, vaR=vaR, vbL=vbL, vbR=vbR))
    res2 = run_bass_kernel_spmd(nc2, in2, core_ids=list(range(CORES)))

    sim = np.zeros((B, B), np.float32)
    for i in range(CORES):
        r = res2.results[i]
        # path1: As1 col-block 800j = pair j (rows 0:64 -> q=2j,
        # rows 64:128 -> q=2j+1, cols (p, n)); dot/ny2 on host
        as1 = np.asarray(r["as1o"], np.float32).reshape(128, 32, 800)
        vb_h = vhat_bTs[i]                              # [64 i, 800 (p n)]
        ny2_1 = np.empty((64, 800), np.float32)
        dot1 = np.empty((64, 800), np.float32)
        ny2_1[0::2] = (as1[0:64] ** 2).sum(0)
        ny2_1[1::2] = (as1[64:128] ** 2).sum(0)
        dot1[0::2] = np.einsum('ijc,ic->jc', as1[0:64], vb_h)
        dot1[1::2] = np.einsum('ijc,ic->jc', as1[64:128], vb_h)
        cos1 = dot1 / np.maximum(np.sqrt(ny2_1), EPS)
        sim1 = cos1.reshape(64, PB, N).sum(-1)          # [q, p]

        # path2: As2 cols 3200p + 800g + 400h + c; rows 0:64 ->
        # qn = 800*(2g+h)+c, rows 64:128 -> +400; vhat_a [64, (g,h,half,c)]
        as2 = np.asarray(r["as2o"], np.float32).reshape(128, PB, 4, 2, 400)
        va4 = vhat_aT.reshape(64, 4, 2, 2, 400)         # [i, g, h, half, c]
        ny_lo = (as2[0:64] ** 2).sum(0).reshape(PB, 8, 400)
        ny_hi = (as2[64:128] ** 2).sum(0).reshape(PB, 8, 400)
        ny2_2 = np.concatenate([ny_lo, ny_hi], axis=2).reshape(PB, B * N)
        d_lo = np.einsum('ipghc,ighc->pghc', as2[0:64], va4[:, :, :, 0])
        d_hi = np.einsum('ipghc,ighc->pghc', as2[64:128], va4[:, :, :, 1])
        dot2 = np.concatenate([d_lo.reshape(PB, 8, 400),
                               d_hi.reshape(PB, 8, 400)],
                              axis=2).reshape(PB, B * N)
        cos2 = dot2 / np.maximum(np.sqrt(ny2_2), EPS)
        sim2 = cos2.reshape(PB, B, N).sum(-1)           # [p, q]

        sim[PB * i:PB * (i + 1)] = (sim1.T + sim2) / N
    return sim



# revision 15
# speedup vs baseline: 1.4580x; 1.0092x over previous
"""Trainium2 Bass kernel for nn_AttentionSimilarity.

Contract: kernel(**inputs) takes the FULL unsharded inputs (numpy) and
returns the FULL [64, 64] similarity matrix, distributing work across 8
NeuronCores internally.

Structure:
  prog1 (projections, sharded by batch): each core projects its 8
    a-batches and 8 b-batches through the three two-layer MLPs,
    emitting qaT/kaT/vaT/qbT/kbT/vbT chunks in [inner, (batch, n)]
    layout. Host gathers the a-side to full tensors.
  prog2 (attention, sharded by p = b-side batch): each core computes
    both attention paths for its 8 p's against all 64 q's, the cosine
    numerators/denominators via selector matmuls on the PE, and the
    per-(p,q) sums over n. Host assembles the [64, 64] output.

Math notes:
  - softmax feeds only cosine similarity, which is scale-invariant in
    the aligned vector, so the softmax max-shift and denominator cancel:
    softmax reduces to exp(scores/8).
  - the x-side cosine norm is folded on the host (vhat = v / max(|v|, eps)).
  - 1/max(|y|, eps) and the dot with vhat are applied on the host from
    the streamed-out aligned values.

Performance notes (vs the first working version):
  - prog1 W1 layer and both programs' score matmuls run in fp8e4 with
    MatmulPerfMode.DoubleRow (2 contraction rows per PE partition, 0.5
    cycles/output column): weights/features/q/k are DR-packed on the
    host ([K/2, 2, M] with k = (K/2)*s + p; lhsT m-blocks padded to
    MP=112 so the DR pair-stride stays 16-byte aligned).
  - the entire cosine stage (dot, squared-norm, rsqrt, mean over n)
    is computed on the HOST: the aligned values (As, bf16) stream out
    over the otherwise-idle DMA engines, deleting the M/SQ multiplies,
    all selector-reduce matmuls, the P1/P2 PSUM accumulators (freeing
    banks for aligned double-buffering), and the device epilogues.
    The device does projections, scores, softmax-exp and the aligned
    matmuls -- all of the O(B^2 N^2) compute.
  - warmup/tail: weight DMAs are split/consolidated so the first matmul
    starts as early as possible; prog1's W2 PSUM/copy/DMA pipeline is
    chunked per bank so stores drain during compute; path2 score tiles
    are 1536 columns (3 PSUM banks) to amortize the fixed per-
    instruction ACT access latency on the softmax exp, which is the
    saturated engine (~98% busy) in the final balance.
  - measured rel err vs fp32 reference: ~1.7e-3.

Dead end (measured): packing score tiles to 128 partitions by mixing
(q, m) across rows would cut exp columns 100/128, but the follow-up
aligned matmuls need operand slices at arbitrary partition offsets and
the PE requires base partition 0/32/64 (bass matmul assert); since
100 is not a multiple of 32, per-q slices of a packed layout are
unaddressable. The [m<=100, cols] score layout is forced.
"""

import os
import sys

sys.path.insert(0, "/opt/trn_rl_repo")
os.environ.setdefault("NEURON_RT_RESET_CORES", "1")

import numpy as np
import ml_dtypes  # noqa: F401  (bf16 host arrays)

import bass_rust
import concourse.bass as bass
import concourse.mybir as mybir
import concourse.tile as tile
from concourse.bass_utils import run_bass_kernel_spmd

F32 = mybir.dt.float32
F32R = mybir.dt.float32r
BF16 = mybir.dt.bfloat16
F16 = mybir.dt.float16
F8E4 = mybir.dt.float8e4
AF = mybir.ActivationFunctionType
DR = mybir.MatmulPerfMode.DoubleRow

B = 64          # batches per side
C = 512         # channels
N = 100         # H*W tokens per batch
INNER = 64      # projected dim
CORES = 8
PB = B // CORES  # batches per core (8)
BN = PB * N      # 800: (batch, n) columns per core chunk
EPS = 1e-8
KT1 = C // 128   # prog1 contraction tiles (4)
MP = 112         # fp8-DR padded m stride (112 % 16 == 0, >= N)

E1_BUFS = int(os.environ.get("K_E1_BUFS", "5"))
SEL_LAG = int(os.environ.get("K_SEL_LAG", "4"))
POOL_MOD1 = int(os.environ.get("K_POOL_MOD1", os.environ.get("K_POOL_MOD", "3")))
POOL_MOD2 = int(os.environ.get("K_POOL_MOD2", os.environ.get("K_POOL_MOD", "2")))
SEL_LAG2 = int(os.environ.get("K_SEL_LAG2", "4"))
M2_BUFS = int(os.environ.get("K_M2_BUFS", "8"))
MPOOL_MOD = int(os.environ.get("K_MPOOL_MOD", "0"))  # 0=never, k=every kth M on pool
M_BUFS = int(os.environ.get("K_M_BUFS", "8"))
E2_BUFS = int(os.environ.get("K_E2_BUFS", "3"))
S1_BUFS = int(os.environ.get("K_S1_BUFS", "2"))
A1_BUFS = int(os.environ.get("K_A1_BUFS", "1"))

_waitsplit_ctr = [0]


def _split_multi_waits(nc, max_waits=1):
    """This container's walrus build accepts at most ONE sync wait per
    instruction; Tile attaches several. Move extras onto preceding
    same-engine NoOps (engines are in-order, so semantics hold)."""
    n_split = 0
    for f in nc.m.functions:
        for blk in f.blocks:
            insts = list(blk.instructions)
            new_list = []
            changed = False
            for inst in insts:
                si = inst.sync_info
                waits = list(si.on_wait) if (si is not None and si.on_wait) else []
                if len(waits) > max_waits:
                    for w in waits[:-max_waits]:
                        _waitsplit_ctr[0] += 1
                        nop = mybir.InstNoOp(
                            name=f"I-waitsplit-{_waitsplit_ctr[0]}",
                            engine=inst.engine,
                            ins=[],
                            outs=[],
                            sync_info=bass_rust.SyncInfo(on_wait=[w], on_update=[]),
                        )
                        nc.register_instruction(nop, overwrite=True)
                        new_list.append(nop)
                        n_split += 1
                    si.on_wait = waits[-max_waits:]
                    inst.sync_info = si
                    changed = True
                new_list.append(inst)
            if changed:
                blk.instructions = new_list
    return n_split


# ---------------------------------------------------------------- prog1

def build_prog1():
    """Projection program, K=256-per-pass DoubleRow everywhere.

    Per-core inputs (all fp8e4 DR-packed on the host):
      f8:    [128, 2*2*2*BN]   features; [p, (side, b, s, n)] holds
                               feat_side[cin = 256b + 128s + p, n]
      w1dr:  [128, 3*2*2*C]    [p, (proj, b, s, cout)] = W1[cin, cout]
      w2dr:  [128, 3*2*2*64]   [p, (proj, b2, s2, i)] = W2[cout, i]
                               (cout = 256*b2 + 128*s2 + p)
    Outputs (f16): qko_a/qko_b [128, BN] (q rows 0:64, k rows 64:128),
      vo_a/vo_b [64, BN].

    Hidden activations are stored fp8e4 so the W2 layer also runs
    DoubleRow (0.5 cyc/col); h layout [128, (b2, s2, n)] makes the DR
    rhs a plain strided view of the relu outputs.
    """
    nc = bass.Bass("TRN2", target_bir_lowering=False, debug=False,
                   num_devices=CORES)
    f8 = nc.dram_tensor("f8", [128, 8 * BN], F8E4, kind="ExternalInput").ap()
    hot1 = nc.dram_tensor("hot1", [128, 2 * (1024 + 1600)], F8E4,
                          kind="ExternalInput").ap()
    w1d = nc.dram_tensor("w1dr", [128, 12 * C], F8E4,
                         kind="ExternalInput").ap()
    w2d = nc.dram_tensor("w2dr", [128, 12 * INNER], F8E4,
                         kind="ExternalInput").ap()
    outs = {"a": nc.dram_tensor("qko_a", [128, BN], F16,
                                kind="ExternalOutput").ap(),
            "b": nc.dram_tensor("qko_b", [128, BN], F16,
                                kind="ExternalOutput").ap()}
    vouts = {"a": nc.dram_tensor("vo_a", [INNER, BN], F16,
                                 kind="ExternalOutput").ap(),
             "b": nc.dram_tensor("vo_b", [INNER, BN], F16,
                                 kind="ExternalOutput").ap()}
    CH = [(0, 512), (512, BN)]  # psum-bank-aligned column chunks of BN

    with tile.TileContext(nc) as tc:
        with (
            tc.tile_pool(name="wpool", bufs=1) as wpool,
            tc.tile_pool(name="hpool", bufs=3) as hpool,
            tc.tile_pool(name="opool", bufs=4) as opool,
            tc.tile_pool(name="psH", bufs=4, space="PSUM") as psHp,
        ):
            # weights + features, hot-first.  w1sb view: [p, proj, b, s,
            # cout]; f view: [p, side, b, s, n]; w2sb: [p, proj, b2, s2, i].
            w1sb = wpool.tile([128, 12 * C], F8E4, tag="w1", name="w1sb")
            w1v = w1sb[:].rearrange("p (pr b s c) -> p pr b s c", pr=3, b=2,
                                    s=2)
            w1dv = w1d.rearrange("p (pr b s c) -> p pr b s c", pr=3, b=2, s=2)
            fsb = wpool.tile([128, 8 * BN], F8E4, tag="f", name="fsb")
            fv = fsb[:].rearrange("p (sd b s n) -> p sd b s n", sd=2, b=2,
                                  s=2)
            fdv = f8.rearrange("p (sd b s n) -> p sd b s n", sd=2, b=2, s=2)
            w2sb = wpool.tile([128, 12 * INNER], F8E4, tag="w2", name="w2sb")
            hotsb = wpool.tile([128, 5248], F8E4, tag="hot", name="hotsb")
            hotv = hotsb[:].rearrange("p (b x) -> p b x", b=2)
            hotd = hot1.rearrange("p (b x) -> p b x", b=2)
            nc.sync.dma_start(hotv[:, 0, 0:1024], hotd[:, 0, 0:1024])
            nc.sync.dma_start(hotv[:, 0, 1024:2624], hotd[:, 0, 1024:2624])
            nc.sync.dma_start(hotv[:, 1, 0:1024], hotd[:, 1, 0:1024])
            nc.sync.dma_start(hotv[:, 1, 1024:2624], hotd[:, 1, 1024:2624])
            nc.sync.dma_start(w1v[:, 1:3], w1dv[:, 1:3])
            nc.sync.dma_start(w2sb[:], w2d[:])
            nc.sync.dma_start(fv[:, 1], fdv[:, 1])
            w1qv = hotv[:, :, 0:1024].rearrange("p b (s c) -> p b s c", s=2)
            fav = hotv[:, :, 1024:2624].rearrange("p b (s n) -> p b s n", s=2)
            w2v = w2sb[:].rearrange("p (pr b s i) -> p pr b s i", pr=3, b=2,
                                    s=2)

            # relu engines, weighted round-robin (ACT/DVE faster than Pool)
            relu_cyc = [0]

            def relu(dst, src):
                e = (nc.scalar, nc.vector, nc.scalar, nc.vector,
                     nc.gpsimd)[relu_cyc[0] % 5]
                relu_cyc[0] += 1
                if e is nc.scalar:
                    e.activation(dst, src, AF.Relu)
                else:
                    e.tensor_scalar_max(dst, src, 0.0)

            hts = {}

            def w1(si, pi):
                ht = hpool.tile([128, 4 * BN], F8E4, tag="h",
                                name=f"h{si}{pi}")
                hv = ht[:].rearrange("p (b s n) -> p b s n", b=2, s=2)
                for t in range(4):
                    psH = psHp.tile([128, 1024], F32, tag="psH", name="psH")
                    for b in range(2):
                        lhsT = (w1qv[:, b, :, 128 * t:128 * (t + 1)]
                                if pi == 0 else
                                w1v[:, pi, b, :, 128 * t:128 * (t + 1)])
                        for lo, hi in CH:
                            nc.tensor.matmul(
                                psH[:, lo:hi], lhsT,
                                (fav[:, b, :, lo:hi] if si == 0 else
                                 fv[:, 1, b, :, lo:hi]),
                                start=(b == 0), stop=(b == 1), perf_mode=DR)
                    relu(hv[:, t // 2, t % 2], psH[:, 0:BN])
                hts[(si, pi)] = hv

            def w2qk(si, s):
                """q rows 0:64 (DR; DR needs dst partition base 0) + k rows
                64:128 (plain fp8) of one psO tile, chunk-wise copy+DMA."""
                psO = psHp.tile([128, 1024], F32, tag="psH", name="psOqk")
                ot = opool.tile([128, BN], F16, tag="out", name="qkout")
                for lo, hi in CH:
                    for b2 in range(2):
                        nc.tensor.matmul(
                            psO[0:64, lo:hi], w2v[:, 0, b2],
                            hts[(si, 0)][:, b2, :, lo:hi],
                            start=(b2 == 0), stop=(b2 == 1), perf_mode=DR)
                    for b2 in range(2):
                        for s2 in range(2):
                            nc.tensor.matmul(
                                psO[64:128, lo:hi], w2v[:, 1, b2, s2],
                                hts[(si, 1)][:, b2, s2, lo:hi],
                                start=(b2 == 0 and s2 == 0),
                                stop=(b2 == 1 and s2 == 1))
                    nc.gpsimd.tensor_copy(ot[:][:, lo:hi], psO[:, lo:hi])
                    nc.sync.dma_start(outs[s][:, lo:hi], ot[:][:, lo:hi])

            def w2v_(si, s):
                psV = psHp.tile([128, 1024], F32, tag="psH", name="psOv")
                vt = opool.tile([INNER, BN], F16, tag="vout", name="vout")
                for lo, hi in CH:
                    for b2 in range(2):
                        nc.tensor.matmul(
                            psV[0:64, lo:hi], w2v[:, 2, b2],
                            hts[(si, 2)][:, b2, :, lo:hi],
                            start=(b2 == 0), stop=(b2 == 1), perf_mode=DR)
                    nc.scalar.copy(vt[:, lo:hi], psV[0:64, lo:hi])
                    nc.sync.dma_start(vouts[s][:, lo:hi], vt[:, lo:hi])

            # PE stream interleaved so W2v(a)'s psO WAR-wait on the qk copy
            # hides under W1q(b), and the relu pipeline never starves PE.
            w1(0, 0); w1(0, 1); w1(0, 2)
            w2qk(0, "a")
            w1(1, 0)
            w2v_(0, "a")
            w1(1, 1); w1(1, 2)
            w2qk(1, "b")
            w2v_(1, "b")

    _split_multi_waits(nc)
    return nc


# ---------------------------------------------------------------- prog2

def build_prog2():
    """Attention program, sharded over p (this core's 8 b-batches).

    Unified 64-stage software pipeline; every stage produces 1600 score
    columns in a [128, 2048] PSUM tile (4 banks, double-buffered = all 8
    banks), does ONE 1600-wide exp on ACT (the bottleneck engine), then
    reuses the exp-consumed banks of the same tile as the aligned-matmul
    accumulator (carve-after-read; subtile deps order the WAR hazard).
    Stage t+1's score matmuls are emitted before stage t's aligned
    matmuls so PE always has score work ready when ACT finishes an exp.

      path1 stage j (32): scores for q-pair (2j, 2j+1) over this core's
        800 (p, n) columns; q0 at S cols 0:800, q1 at 1024:1824; exp via
        a strided [100, 2, 800] AP; aligned A at cols 0:800.
      path2 stage (p, k) (32): scores for 1600 (q n) columns
        [1600k, 1600k+1600) against kb[p]; aligned A groups at cols
        0:400 and 512:912; strided copy out.

    Outputs (identical layout to the previous version; host unchanged):
      as1o [128, 32*800] bf16, as2o [128, 32*800] bf16
    """
    nc = bass.Bass("TRN2", target_bir_lowering=False, debug=False,
                   num_devices=CORES)
    din = {}
    for name, shape, dt in [
        ("kaTdr", [32, 2 * B * MP], F8E4), ("qaTdr", [32, 2 * B * N], F8E4),
        ("qbTdr", [32, 2 * BN], F8E4), ("kbTdr", [32, 2 * PB * MP], F8E4),
        ("hot2", [32, 4992], F8E4),
        ("vaL", [N, (B // 2) * 128], F16), ("vaR", [N, (B // 2) * 128], F16),
        ("vbL", [N, PB * 128], F16), ("vbR", [N, PB * 128], F16),
    ]:
        din[name] = nc.dram_tensor(name, shape, dt, kind="ExternalInput").ap()
    as1o = nc.dram_tensor("as1o", [128, 32 * BN], BF16,
                          kind="ExternalOutput").ap()
    as2o = nc.dram_tensor("as2o", [128, 32 * 800], BF16,
                          kind="ExternalOutput").ap()

    with tile.TileContext(nc) as tc:
        from contextlib import ExitStack
        with ExitStack() as ctx:
            inp = ctx.enter_context(tc.tile_pool(name="inp", bufs=1))
            sb = {}

            def load(name):
                ap = din[name]
                t = inp.tile(list(ap.shape), ap.dtype, tag=name,
                             name=f"sb_{name}")
                nc.sync.dma_start(t[:], ap[:])
                sb[name] = t

            # Input DMAs, hot-first. All on the SP (sync) queue, issued
            # before any output DMA so no wait ever blocks the SP SEQ.
            ka_t = inp.tile([32, 2 * B * MP], F8E4, tag="kaTdr",
                            name="sb_kaTdr")
            sb["kaTdr"] = ka_t
            ka3d = din["kaTdr"].rearrange("p (two q m) -> p two q m",
                                          two=2, q=B)
            ka3s = ka_t[:].rearrange("p (two q m) -> p two q m", two=2, q=B)
            hot2 = inp.tile([32, 4992], F8E4, tag="hot2", name="sb_hot2")
            nc.sync.dma_start(hot2[:], din["hot2"][:])
            va_t = {}
            for nm in ("vaL", "vaR"):
                va_t[nm] = inp.tile([N, (B // 2) * 128], F16, tag=nm,
                                    name=f"sb_{nm}")
                nc.sync.dma_start(va_t[nm][:, 0:512], din[nm][:, 0:512])
            for nm in ("vaL", "vaR"):
                nc.sync.dma_start(va_t[nm][:, 512:2048], din[nm][:, 512:2048])
            nc.sync.dma_start(ka3s[:, :, 8:32, :], ka3d[:, :, 8:32, :])
            for nm in ("vaL", "vaR"):
                nc.sync.dma_start(va_t[nm][:, 2048:4096],
                                  din[nm][:, 2048:4096])
            nc.sync.dma_start(ka3s[:, :, 32:64, :], ka3d[:, :, 32:64, :])
            qa_t = inp.tile([32, 2 * B * N], F8E4, tag="qaTdr",
                            name="sb_qaTdr")
            sb["qaTdr"] = qa_t
            qa3d = din["qaTdr"].rearrange("p (two n) -> p two n", two=2)
            qa3s = qa_t[:].rearrange("p (two n) -> p two n", two=2)
            nc.sync.dma_start(qa3s[:, :, 0:3200], qa3d[:, :, 0:3200])
            nc.sync.dma_start(qa3s[:, :, 3200:6400], qa3d[:, :, 3200:6400])
            for name in ("kbTdr", "vbL", "vbR"):
                load(name)

            epool = ctx.enter_context(tc.tile_pool(name="epool", bufs=4))
            mpool = ctx.enter_context(tc.tile_pool(name="mpool", bufs=8))
            spool = ctx.enter_context(
                tc.tile_pool(name="spool", bufs=2, space="PSUM"))
            apool = ctx.enter_context(
                tc.tile_pool(name="apool", bufs=2, space="PSUM"))

            ka3 = sb["kaTdr"][:].rearrange("p (two q m) -> p two q m",
                                           two=2, q=B)
            qb3 = hot2[:][:, 0:3200].rearrange("p (two n) -> p two n",
                                               two=2)
            ka_hot = hot2[:][:, 3200:4992].rearrange(
                "p (two q m) -> p two q m", two=2, q=8)
            kb3 = sb["kbTdr"][:].rearrange("p (two b m) -> p two b m",
                                           two=2, b=PB)
            qa3 = sb["qaTdr"][:].rearrange("p (two n) -> p two n", two=2)

            # The whole attention is one score stream of 102,400 columns:
            #   cols [1600j + 800h, +800)          = path1 pair j, q = 2j+h
            #   cols [51200 + 6400p + o, ...)      = path2 batch p
            # chunked into CW-wide exp stages (3-bank PSUM S tiles).
            SL = 102400
            # chunk grid: uniform 1536 except a custom tail so only ONE
            # 800-col aligned group drains after the final exp
            BND = [1536 * i for i in range(66)] + [101376, 101600, SL]
            NT = len(BND) - 1
            import bisect as _bi

            def chunk_of(pos):
                return _bi.bisect_right(BND, pos) - 1
            segs = []  # (base, length, lhsT, rhs3)
            for j in range(B // 2):
                for h in range(2):
                    q = 2 * j + h
                    lhsT = (ka_hot[:, :, q, 0:N] if q < 8 else
                            ka3[:, :, q, 0:N])
                    segs.append((1600 * j + 800 * h, 800, lhsT, qb3))
            for p in range(PB):
                segs.append((51200 + 6400 * p, 6400, kb3[:, :, p, 0:N], qa3))

            etiles = {}  # chunk index -> E tile

            def eslices(a, b):
                """Stream range [a, b) as a list of E-tile slices."""
                out = []
                while a < b:
                    t = chunk_of(a)
                    e = min(b, BND[t + 1])
                    out.append(etiles[t][:][:, a - BND[t]:e - BND[t]])
                    a = e
                return out

            def emit_front(t):
                """Score matmuls + one exp for stream chunk t."""
                c0, c1 = BND[t], BND[t + 1]
                sa = spool.tile([100, 1536], F32, tag="S", name=f"S{t % 2}")
                E = epool.tile([100, 1536], F16, tag="E")
                for base, ln, lhsT, rhs3 in segs:
                    a, b = max(c0, base), min(c1, base + ln)
                    while a < b:  # split at this S tile's 512-col banks
                        e = min(b, c0 + ((a - c0) // 512 + 1) * 512)
                        nc.tensor.matmul(
                            sa[:][:, a - c0:e - c0], lhsT,
                            rhs3[:, :, a - base:e - base],
                            start=True, stop=True, perf_mode=DR)
                        a = e
                nc.scalar.activation(E[:][:, 0:c1 - c0], sa[:][:, 0:c1 - c0],
                                     AF.Exp, scale=0.125)
                etiles[t] = E

            def emit_aligned(At, dcols, pairs):
                """At[:, d] = sum_i lhsT_i.T @ E[stream a_i + d] for
                d in [0, dcols). Dest is split at every E-chunk boundary of
                either source range so each dest interval is a complete
                start/stop accumulation group."""
                cuts = {0, dcols}
                for _, a in pairs:
                    for t in range(chunk_of(a) + 1, chunk_of(a + dcols - 1) + 1):
                        cuts.add(BND[t] - a)
                cs = sorted(cuts)
                for d0, d1 in zip(cs, cs[1:]):
                    for i, (lhsT, a) in enumerate(pairs):
                        (sl,) = eslices(a + d0, a + d1)
                        nc.tensor.matmul(At[:][:, d0:d1], lhsT, sl,
                                         start=(i == 0),
                                         stop=(i == len(pairs) - 1),
                                         skip_group_check=True)

            as2_live = {}

            def emit_back(g):
                """Aligned matmuls + copy (+DMA) for finished group g."""
                if g < B // 2:  # path1 pair j
                    j = g
                    vaLs = va_t["vaL"][:, 128 * j:128 * (j + 1)]
                    vaRs = va_t["vaR"][:, 128 * j:128 * (j + 1)]
                    As = mpool.tile([128, 800], BF16, tag="As")
                    for lo, w in ((0, 512), (512, 288)):
                        At = apool.tile([128, 512], F32, tag="A")
                        emit_aligned(At, w,
                                     [(vaLs, 1600 * j + lo),
                                      (vaRs, 1600 * j + 800 + lo)])
                        nc.vector.tensor_copy(As[:][:, lo:lo + w],
                                              At[:][:, 0:w])
                    nc.sync.dma_start(as1o[:, BN * j:BN * (j + 1)], As[:])
                else:  # path2 800-col group
                    gg = g - B // 2
                    p, o8 = gg // 8, (gg % 8) * 800
                    base = 51200 + 6400 * p + o8
                    vbLs = sb["vbL"][:, 128 * p:128 * (p + 1)]
                    vbRs = sb["vbR"][:, 128 * p:128 * (p + 1)]
                    At = apool.tile([128, 512], F32, tag="A")
                    emit_aligned(At, 400, [(vbLs, base), (vbRs, base + 400)])
                    u = (gg % 8) // 2
                    if gg % 2 == 0:
                        as2_live[p] = mpool.tile([128, 800], BF16, tag="As", name="As2")
                    As2 = as2_live[p]
                    nc.vector.tensor_copy(
                        As2[:][:, 400 * (gg % 2):400 * (gg % 2) + 400],
                        At[:][:, 0:400])
                    if gg % 2 == 1:
                        nc.sync.dma_start(
                            as2o[:, 3200 * p + 800 * u:
                                 3200 * p + 800 * (u + 1)], As2[:])

            # group g ready once its last stream column's chunk is emitted
            ends = [1600 * (j + 1) for j in range(B // 2)] + \
                   [51200 + 6400 * (gg // 8) + 800 * (gg % 8) + 800
                    for gg in range(64)]
            ready = [chunk_of(e - 1) for e in ends]
            for t in range(NT + 1):
                if t < NT:
                    emit_front(t)
                for g in range(len(ends)):
                    if ready[g] == t - 1:
                        emit_back(g)

    _split_multi_waits(nc)
    return nc


# ---------------------------------------------------------------- host

_progs = {}


def _install_compile_cache():
    """Persist compiled NEFF-wrapped custom calls across processes: walrus
    compilation takes tens of seconds per program and bass2jax recompiles
    in every fresh process otherwise."""
    import hashlib
    import pathlib
    from concourse import bass2jax
    if getattr(bass2jax, "_ant_disk_cache", False):
        return
    bass2jax._ant_disk_cache = True
    orig = bass2jax.neuronx_cc_hook
    cdir = pathlib.Path(os.environ.get("BASS_NEFF_CACHE",
                                       "/tmp/bass_neff_cache"))
    try:
        cdir.mkdir(parents=True, exist_ok=True)
    except OSError:
        return

    def cached_hook(code, code_format, platform_version, file_prefix):
        try:
            key = hashlib.sha256(
                bytes(code) + b"|" + bytes(code_format)).hexdigest()
            path = cdir / f"{key}.neffcall"
            if path.exists():
                return 0, path.read_bytes()
        except Exception:
            return orig(code, code_format, platform_version, file_prefix)
        rc, blob = orig(code, code_format, platform_version, file_prefix)
        if rc == 0:
            try:
                tmp = path.with_suffix(f".tmp{os.getpid()}")
                tmp.write_bytes(blob)
                tmp.rename(path)
            except OSError:
                pass
        return rc, blob

    bass2jax.neuronx_cc_hook = cached_hook
    try:
        import libneuronxla
        if libneuronxla.neuronx_cc is orig:
            libneuronxla.neuronx_cc = cached_hook
    except ImportError:
        pass


def _get_progs():
    if "p1" not in _progs:
        _install_compile_cache()
        _progs["p1"] = build_prog1()
        _progs["p2"] = build_prog2()
    return _progs["p1"], _progs["p2"]


def _masters():
    import ml_dtypes
    m1 = np.zeros((128, 320), ml_dtypes.bfloat16)
    m1[0:64, 128] = 1.0   # up-plane (rows 0:64 of rhs) -> out row q
    m1[64:128, 129] = 1.0  # down-plane -> out row q+1
    m8 = np.zeros((128, 320), ml_dtypes.bfloat16)
    m8[0:64, 128] = 1.0
    m8[64:128, 136] = 1.0  # down-plane -> out row r0+8
    return m1, m8


def _dr_pack_k(x, pad_to=None):
    """Pack [K, M] (K contraction, even) into DoubleRow layout
    [K//2, 2*M] fp8e4 with k = (K//2)*s + p."""
    import ml_dtypes
    K = x.shape[0]
    h = K // 2
    arr = x.reshape(2, h, *x.shape[1:]).transpose(1, 0, *range(2, x.ndim + 1))
    return np.ascontiguousarray(arr.reshape(h, -1).astype(
        ml_dtypes.float8_e4m3fn))


def _dr_pack_k_padded(x, nblk, blk, pad):
    """[K, nblk*blk] -> DR fp8 [K//2, 2*nblk*pad] with each blk padded."""
    import ml_dtypes
    K = x.shape[0]
    h = K // 2
    a = x.reshape(2, h, nblk, blk).transpose(1, 0, 2, 3)
    z = np.zeros((h, 2, nblk, pad), np.float32)
    z[:, :, :, 0:blk] = a
    return np.ascontiguousarray(z.reshape(h, -1).astype(
        ml_dtypes.float8_e4m3fn))


def kernel(features_a, features_b, Wq1, Wq2, Wk1, Wk2, Wv1, Wv2):
    import ml_dtypes
    nc1, nc2 = _get_progs()
    cc = np.ascontiguousarray
    FP8 = ml_dtypes.float8_e4m3fn

    fa = np.asarray(features_a, np.float32).reshape(B, C, N)
    fb = np.asarray(features_b, np.float32).reshape(B, C, N)

    def feat8(fa_core, fb_core):  # 2x [PB, C, N] -> [128, 8*BN] fp8
        # [sd, b, s, p, n] with cin = 256b + 128s + p -> [p, sd, b, s, n]
        fT = np.stack([fc.transpose(1, 0, 2).reshape(C, BN)
                       for fc in (fa_core, fb_core)])
        a = fT.reshape(2, 2, 2, 128, BN).transpose(3, 0, 1, 2, 4)
        return cc(a.reshape(128, 8 * BN).astype(FP8))

    def wpack(Ws):  # list of [C, M] -> [128, 3*2*2*M] fp8
        a = np.stack([np.asarray(W, np.float32) for W in Ws])
        M = a.shape[-1]
        a = a.reshape(3, 2, 2, 128, M).transpose(3, 0, 1, 2, 4)
        return cc(a.reshape(128, 12 * M).astype(FP8))

    ws = {"w1dr": wpack([Wq1, Wk1, Wv1]), "w2dr": wpack([Wq2, Wk2, Wv2])}
    w1q_b = np.asarray(Wq1, np.float32).reshape(2, 2, 128, C).transpose(
        2, 0, 1, 3).reshape(128, 2, 1024).astype(FP8)  # [p, b, (s c)]

    def hot1(f8c):  # f8c [128, 8*BN]: fuse [w1q-b | fa-b] per DR pass b
        fa4 = f8c.reshape(128, 2, 2, 2, BN)[:, 0].reshape(128, 2, 1600)
        return cc(np.concatenate([w1q_b, fa4], axis=2).reshape(128, 5248))

    in1 = []
    for i in range(CORES):
        f8c = feat8(fa[PB * i:PB * (i + 1)], fb[PB * i:PB * (i + 1)])
        in1.append(dict(f8=f8c, hot1=hot1(f8c), **ws))
    res1 = run_bass_kernel_spmd(nc1, in1, core_ids=list(range(CORES)))

    qaT = np.concatenate([res1.results[i]["qko_a"][0:64]
                          for i in range(CORES)], axis=1)
    kaT = np.concatenate([res1.results[i]["qko_a"][64:128]
                          for i in range(CORES)], axis=1)
    vaT = np.concatenate([res1.results[i]["vo_a"]
                          for i in range(CORES)], axis=1)
    qbT = [res1.results[i]["qko_b"][0:64] for i in range(CORES)]
    kbT = [res1.results[i]["qko_b"][64:128] for i in range(CORES)]
    vbT = [res1.results[i]["vo_b"] for i in range(CORES)]

    # a-side derived tensors (shared by all cores)
    vaT32 = vaT.astype(np.float32)
    va_nm = cc(vaT.T)                       # [B*N, INNER] fp16
    na = np.maximum(np.sqrt((vaT32 * vaT32).sum(0)), EPS)
    vhat_aT = vaT32 / na[None, :]
    vaL = np.zeros((N, (B // 2) * 128), np.float16)
    vaR = np.zeros((N, (B // 2) * 128), np.float16)
    for j in range(B // 2):
        vaL[:, 128 * j:128 * j + 64] = va_nm[N * 2 * j:N * (2 * j + 1)]
        vaR[:, 128 * j + 64:128 * (j + 1)] = va_nm[N * (2 * j + 1):
                                                   N * (2 * j + 2)]
    vhat_aT2 = np.zeros((128, B * N // 2), np.float32)
    for j2 in range(8):
        vhat_aT2[0:64, 400 * j2:400 * (j2 + 1)] = \
            vhat_aT[:, 800 * j2:800 * j2 + 400]
        vhat_aT2[64:128, 400 * j2:400 * (j2 + 1)] = \
            vhat_aT[:, 800 * j2 + 400:800 * (j2 + 1)]
    m1, m8 = _masters()

    kaTdr = _dr_pack_k_padded(kaT.astype(np.float32), B, N, MP)
    qaTdr = _dr_pack_k(qaT.astype(np.float32))
    in2 = []
    vhat_bTs = []
    for i in range(CORES):
        vbT32 = vbT[i].astype(np.float32)
        vb_nm = cc(vbT[i].T)                # [BN, INNER] fp16
        nb = np.maximum(np.sqrt((vbT32 * vbT32).sum(0)), EPS)
        vhat_bT = vbT32 / nb[None, :]
        vbL = np.zeros((N, PB * 128), np.float16)
        vbR = np.zeros((N, PB * 128), np.float16)
        for p in range(PB):
            vbL[:, 128 * p:128 * p + 64] = vb_nm[N * p:N * (p + 1)]
            vbR[:, 128 * p + 64:128 * (p + 1)] = vb_nm[N * p:N * (p + 1)]
        vhat_bTs.append(vhat_bT)
        qbdr_i = _dr_pack_k(qbT[i].astype(np.float32))
        hot2_i = cc(np.concatenate(
            [qbdr_i, kaTdr.reshape(32, 2, B, MP)[:, :, 0:8].reshape(32, 1792)],
            axis=1))
        in2.append(dict(
            kaTdr=kaTdr, qaTdr=qaTdr, hot2=hot2_i,
            qbTdr=qbdr_i,
            kbTdr=_dr_pack_k_padded(kbT[i].astype(np.float32), PB, N, MP),
            vaL=vaL, vaR=vaR, vbL=vbL, vbR=vbR))
    res2 = run_bass_kernel_spmd(nc2, in2, core_ids=list(range(CORES)))

    sim = np.zeros((B, B), np.float32)
    for i in range(CORES):
        r = res2.results[i]
        # path1: As1 col-block 800j = pair j (rows 0:64 -> q=2j,
        # rows 64:128 -> q=2j+1, cols (p, n)); dot/ny2 on host
        as1 = np.asarray(r["as1o"], np.float32).reshape(128, 32, 800)
        vb_h = vhat_bTs[i]                              # [64 i, 800 (p n)]
        ny2_1 = np.empty((64, 800), np.float32)
        dot1 = np.empty((64, 800), np.float32)
        ny2_1[0::2] = (as1[0:64] ** 2).sum(0)
        ny2_1[1::2] = (as1[64:128] ** 2).sum(0)
        dot1[0::2] = np.einsum('ijc,ic->jc', as1[0:64], vb_h)
        dot1[1::2] = np.einsum('ijc,ic->jc', as1[64:128], vb_h)
        cos1 = dot1 / np.maximum(np.sqrt(ny2_1), EPS)
        sim1 = cos1.reshape(64, PB, N).sum(-1)          # [q, p]

        # path2: As2 cols 3200p + 800g + 400h + c; rows 0:64 ->
        # qn = 800*(2g+h)+c, rows 64:128 -> +400; vhat_a [64, (g,h,half,c)]
        as2 = np.asarray(r["as2o"], np.float32).reshape(128, PB, 4, 2, 400)
        va4 = vhat_aT.reshape(64, 4, 2, 2, 400)         # [i, g, h, half, c]
        ny_lo = (as2[0:64] ** 2).sum(0).reshape(PB, 8, 400)
        ny_hi = (as2[64:128] ** 2).sum(0).reshape(PB, 8, 400)
        ny2_2 = np.concatenate([ny_lo, ny_hi], axis=2).reshape(PB, B * N)
        d_lo = np.einsum('ipghc,ighc->pghc', as2[0:64], va4[:, :, :, 0])
        d_hi = np.einsum('ipghc,ighc->pghc', as2[64:128], va4[:, :, :, 1])
        dot2 = np.concatenate([d_lo.reshape(PB, 8, 400),
                               d_hi.reshape(PB, 8, 400)],
                              axis=2).reshape(PB, B * N)
        cos2 = dot2 / np.maximum(np.sqrt(ny2_2), EPS)
        sim2 = cos2.reshape(PB, B, N).sum(-1)           # [p, q]

        sim[PB * i:PB * (i + 1)] = (sim1.T + sim2) / N
    return sim

